# revision 1
# baseline (speedup 1.0000x reference)
"""EnhancedGAT Trainium2 Bass kernel (8 NeuronCores, SPMD).

Strategy:
  - Edges are sorted by destination node on the host; core k owns dst nodes
    [k*N/8, (k+1)*N/8) and every edge targeting them. Per-core edge lists are
    bucketed into 32-node bins and padded to 128-edge chunks with a per-bin
    chunk count shared across cores (SPMD uniformity).
  - Each GAT layer:
      node phase: every core computes a table row [h | a_s | a_d] (bf16,
        padded to a 256-element row so dma_gather's 256B-alignment holds) for
        its own nodes, then an AllGather replicates the full table to every
        core's DRAM.
      edge phase: per 4096-edge superstep one dma_gather pulls the rows for
        the edges' sources; attention coefficients are computed in-place and
        the weighted messages are scattered into per-bin PSUM accumulators via
        one-hot matmuls (lhsT = staircase matrix built once with iota+is_equal).
        Softmax is unnormalized (exp / segment-sum; max-subtraction skipped --
        alphas are O(0.3) here) and the divide happens per node at window
        epilogue, where self-loop contributions are also added.
  - Layer 1 additionally accumulates per-node mean edge-feature attention
    terms and in-degrees (extra matmul columns) used by the self-loops of
    layers 2-4.
  - Final graph mean-pool via one-hot matmuls into a [33, G] accumulator,
    AllReduce across cores, tiny dense readout replicated on every core.
"""
import sys
import numpy as np

sys.path.insert(0, "/opt/trn_rl_repo")

HID = 32
NCORES = 8
P = 128
BIN = 32
SS = 32          # chunks per superstep
CHUNK = 128
ROW = 256        # table row elements (bf16) for layers 1-3
ROW4 = 128       # layer-4 table row elements


# ----------------------------------------------------------------- host prep
def host_prep(inputs):
    x = np.asarray(inputs["x"], np.float32)
    ei = np.asarray(inputs["edge_index"]).astype(np.int64)
    ea = np.asarray(inputs["edge_attr"], np.float32)
    batch = np.asarray(inputs["batch"]).astype(np.int64)
    desc = np.asarray(inputs["descriptors"], np.float32)

    N = x.shape[0]
    E = ei.shape[1]
    Gn = desc.shape[0]
    NPC = N // NCORES
    NW = -(-NPC // P)
    NBINS = -(-NPC // BIN)

    src_all, dst_all = ei[0], ei[1]
    order = np.argsort(dst_all, kind="stable")
    src_s, dst_s = src_all[order], dst_all[order]
    ea_s = ea[order]
    core_of = dst_s // NPC
    local = dst_s - core_of * NPC
    bin_of = local // BIN

    cnt = np.zeros((NCORES, NBINS), np.int64)
    np.add.at(cnt, (core_of, bin_of), 1)
    cpb = np.max(-(-cnt // CHUNK), axis=0)          # chunks per bin (shared)
    C_total = int(cpb.sum())
    padc = (-C_total) % SS
    if C_total == 0:
        padc = SS
    cpb[-1] += padc
    C_total += padc
    off = np.zeros(NBINS, np.int64)
    off[1:] = np.cumsum(cpb)[:-1]
    EP = C_total * CHUNK                            # padded edges per core

    per_core = []
    for k in range(NCORES):
        srck = np.zeros(EP, np.int64)
        dstrk = np.zeros(EP, np.float32)
        maskk = np.zeros(EP, np.float32)
        eak = np.zeros((EP, 4), np.float32)
        sel = core_of == k
        bins_k = bin_of[sel]
        start = np.searchsorted(bins_k, np.arange(NBINS))
        pos = np.arange(bins_k.size) - start[bins_k]
        slot = off[bins_k] * CHUNK + pos
        srck[slot] = src_s[sel]
        dstrk[slot] = (local[sel] - bins_k * BIN).astype(np.float32)
        maskk[slot] = 1.0
        eak[slot] = ea_s[sel]

        # device layouts: edge e = c*128 + p
        src16 = np.tile(srck.reshape(-1, 16).T.astype(np.int16), (8, 1))
        dstr_d = dstrk.reshape(C_total, P).T.copy()
        mask_d = maskk.reshape(C_total, P).T.copy()
        abias_d = ((mask_d - 1.0) * 30.0).astype(np.float32)
        eaT_d = eak.T.copy()                         # [4, EP]

        xk = x[k * NPC:(k + 1) * NPC]
        xT = np.zeros((8, NW * P), np.float32)
        xT[:, :NPC] = xk.T
        bk = np.full(NW * P, Gn + 5, np.float32)
        bk[:NPC] = batch[k * NPC:(k + 1) * NPC].astype(np.float32)
        batch_d = bk.reshape(NW, P).T.copy()

        per_core.append(dict(SRC16=src16, DSTR=dstr_d, MASK=mask_d,
                             ABIAS=abias_d, EAT=eaT_d, XT=xT, BATCH=batch_d))

    # ---- weight folding
    w = {k: np.asarray(v, np.float32) for k, v in inputs.items()
         if k not in ("x", "edge_index", "edge_attr", "batch", "descriptors")}

    def vfold(We, ae, heads):
        Vp = (We.reshape(w["We_enc"].shape[1], heads, HID) * ae[None]).sum(-1)
        return w["We_enc"] @ Vp, w["be_enc"] @ Vp      # [4,heads],[heads]

    V2, bv2 = vfold(w["We2"], w["ae2"], 4)
    V3, bv3 = vfold(w["We3"], w["ae3"], 4)
    V4, bv4 = vfold(w["We4"], w["ae4"], 1)
    W4x9 = np.concatenate([V2, V3, V4], axis=1)        # [4,9]
    be9 = np.concatenate([bv2, bv3, bv4])              # [9]

    def padr(v, n):
        o = np.zeros(n, np.float32)
        o[: v.size] = v
        return o

    # channel-major reorder of the 128-wide (4 heads x 32 ch) dimension:
    # new position c*4+a holds old a*32+c. Keeps per-head broadcasts
    # innermost-packed on DVE (2x mode).
    cm = (np.arange(128) % 4) * 32 + np.arange(128) // 4

    atts = np.stack([padr(w["as1"].reshape(-1)[cm], 128), padr(w["as2"].reshape(-1)[cm], 128),
                     padr(w["as3"].reshape(-1)[cm], 128), padr(w["as4"].reshape(-1), 128)])
    attd = np.stack([padr(w["ad1"].reshape(-1)[cm], 128), padr(w["ad2"].reshape(-1)[cm], 128),
                     padr(w["ad3"].reshape(-1)[cm], 128), padr(w["ad4"].reshape(-1), 128)])
    bout = np.stack([padr(w["b1"][cm], 128), padr(w["b2"][cm], 128),
                     padr(w["b3"][cm], 128), padr(w["b4"], 128)])

    shared = dict(
        W1=w["W1"][:, cm], WL2=w["W2"][cm][:, cm], WL3=w["W3"][cm][:, cm],
        WL4=w["W4"][cm],
        W4x9=W4x9, BE9R=np.tile(be9, 4)[None, :],      # [1,36]
        ATTS=atts, ATTD=attd, BOUT=bout,
        WD=w["Wd"], BD=w["bd"][:, None], WLIN=w["Wl"], DESCT=desc.T.copy(),
    )
    bl = float(np.asarray(w["bl"]).reshape(-1)[0])

    dims = dict(N=N, E=E, Gn=Gn, NPC=NPC, NW=NW, NBINS=NBINS,
                C=C_total, cpb=cpb, off=off, bl=bl)
    return dims, shared, per_core


# ------------------------------------------------------------- program build
def build_program(dims, shared):
    import concourse.bass as bass
    import concourse.mybir as mybir
    import concourse.tile as tile
    import concourse.bacc as bacc
    from concourse.masks import make_identity
    from contextlib import ExitStack

    F32 = mybir.dt.float32
    BF16 = mybir.dt.bfloat16
    I32 = mybir.dt.int32
    I16 = mybir.dt.int16
    AF = mybir.ActivationFunctionType
    ALU = mybir.AluOpType
    AX = mybir.AxisListType

    N, Gn, NPC, NW, NBINS, C = (dims[k] for k in ("N", "Gn", "NPC", "NW", "NBINS", "C"))
    cpb, off, bl = dims["cpb"], dims["off"], dims["bl"]
    NSS = C // SS
    # layer params: h width, heads, rhs width, gather row elems
    LP = [dict(HW=128, AW=4, RW=146, EL=ROW),   # L1 (rhs incl. junk a_d + eterm9 + cnt)
          dict(HW=128, AW=4, RW=132, EL=ROW),
          dict(HW=128, AW=4, RW=132, EL=ROW),
          dict(HW=32, AW=1, RW=33, EL=ROW4)]

    nc = bacc.Bacc(num_swdge_queues=2)
    DEBUG_DUMPS = dims.get("debug", False)
    SIM1 = dims.get("sim1", False)

    # ---- params
    pr = {}
    for nm, shp, dt in [("SRC16", [P, C * 8], I16), ("DSTR", [P, C], F32),
                        ("MASK", [P, C], F32), ("ABIAS", [P, C], F32),
                        ("EAT", [4, C * CHUNK], F32), ("XT", [8, NW * P], F32),
                        ("BATCH", [P, NW], F32), ("W1", [8, 128], F32),
                        ("WL2", [128, 128], F32), ("WL3", [128, 128], F32),
                        ("WL4", [128, 32], F32), ("W4x9", [4, 9], F32),
                        ("BE9R", [1, 36], F32), ("ATTS", [4, 128], F32),
                        ("ATTD", [4, 128], F32), ("BOUT", [4, 128], F32),
                        ("WD", [48, 32], F32), ("BD", [32, 1], F32),
                        ("WLIN", [64, 1], F32), ("DESCT", [48, Gn], F32)]:
        pr[nm] = nc.declare_dram_parameter(nm, shp, dt, isOutput=False)
    out_p = nc.declare_dram_parameter("out", [1, Gn], F32, isOutput=True)
    dbgw = [nc.declare_dram_parameter(f"dbgw{l}", [NW * P, 146], F32, isOutput=True)
            for l in range(3)] if dims.get("debug") else None

    # ---- internal DRAM
    T_loc = [nc.dram_tensor(f"T_loc{l}", [NPC, LP[l]["EL"]], BF16) for l in range(4)]
    T_glob = [nc.dram_tensor(f"T_glob{l}", [N, LP[l]["EL"]], BF16, addr_space="Shared")
              for l in range(4)]
    ar_in = nc.dram_tensor("ar_in", [33, Gn], F32)
    ar_out = nc.dram_tensor("ar_out", [33, Gn], F32, addr_space="Shared")
    cnt_dram = nc.dram_tensor("cnt_dram", [1, Gn], F32)

    # bin/window bookkeeping (compile-time)
    bin_of_chunk = []
    for b in range(NBINS):
        bin_of_chunk += [b] * int(cpb[b])
    win_of_bin = [b // 4 for b in range(NBINS)]
    last_chunk_of_bin = {}
    first_chunk_of_bin = {}
    for c_i, b in enumerate(bin_of_chunk):
        last_chunk_of_bin[b] = c_i
        first_chunk_of_bin.setdefault(b, c_i)
    last_chunk_of_win = {}
    for b in range(NBINS):
        if b in last_chunk_of_bin:
            w_ = win_of_bin[b]
            last_chunk_of_win[w_] = max(last_chunk_of_win.get(w_, -1),
                                        last_chunk_of_bin[b])

    with tile.TileContext(nc) as tc, ExitStack() as ctx:
        cp = ctx.enter_context(tc.tile_pool(name="const", bufs=1))
        wp = ctx.enter_context(tc.tile_pool(name="work", bufs=2))
        vp = ctx.enter_context(tc.tile_pool(name="win", bufs=3))
        pp = ctx.enter_context(tc.tile_pool(name="psum", bufs=2, space="PSUM"))
        bp = ctx.enter_context(tc.tile_pool(name="binp", bufs=2, space="PSUM"))

        sync, gps, vec, act, pe = nc.sync, nc.gpsimd, nc.vector, nc.scalar, nc.tensor

        # ---- resident tiles
        src16 = cp.tile([P, C * 8], I16)
        sync.dma_start(out=src16[:], in_=pr["SRC16"][:, :])
        dstr = cp.tile([P, C], F32)
        sync.dma_start(out=dstr[:], in_=pr["DSTR"][:, :])
        maskt = cp.tile([P, C], F32)
        sync.dma_start(out=maskt[:], in_=pr["MASK"][:, :])
        abias = cp.tile([P, C], F32)
        sync.dma_start(out=abias[:], in_=pr["ABIAS"][:, :])
        batcht = cp.tile([P, NW], F32)
        sync.dma_start(out=batcht[:], in_=pr["BATCH"][:, :])
        xT_sb = cp.tile([8, NW * P], F32)
        sync.dma_start(out=xT_sb[:], in_=pr["XT"][:, :])

        iota32 = cp.tile([P, BIN], I32)
        gps.iota(iota32[:], pattern=[[1, BIN]], base=0, channel_multiplier=0)
        iota32f = cp.tile([P, BIN], F32)
        vec.tensor_copy(iota32f[:], iota32[:])
        iotag_i = cp.tile([P, Gn], I32)
        gps.iota(iotag_i[:], pattern=[[1, Gn]], base=0, channel_multiplier=0)
        iotagf = cp.tile([P, Gn], F32)
        vec.tensor_copy(iotagf[:], iotag_i[:])
        identb = cp.tile([P, P], BF16)
        make_identity(nc, identb[:])

        w1_sb = cp.tile([8, 128], F32)
        sync.dma_start(out=w1_sb[:], in_=pr["W1"][:, :])
        wl_sb = [None,
                 cp.tile([128, 128], BF16, name="wl2", tag="wl2"),
                 cp.tile([128, 128], BF16, name="wl3", tag="wl3"),
                 cp.tile([128, 32], BF16, name="wl4", tag="wl4")]
        gps.dma_start(out=wl_sb[1][:], in_=pr["WL2"][:, :])   # gpsimd casts f32->bf16
        gps.dma_start(out=wl_sb[2][:], in_=pr["WL3"][:, :])
        gps.dma_start(out=wl_sb[3][:], in_=pr["WL4"][:, :])
        w4x9_sb = cp.tile([4, 9], F32)
        sync.dma_start(out=w4x9_sb[:], in_=pr["W4x9"][:, :])
        be9r = cp.tile([P, 36], F32)
        sync.dma_start(out=be9r[:], in_=pr["BE9R"][0:1, :].to_broadcast([P, 36]))
        atts_t, attd_t, bout_t = [], [], []
        for l in range(4):
            t1 = cp.tile([P, 128], BF16, tag=f"atts{l}")
            gps.dma_start(out=t1[:], in_=pr["ATTS"][l:l + 1, :].to_broadcast([P, 128]))
            atts_t.append(t1)
            t2 = cp.tile([P, 128], BF16, tag=f"attd{l}")
            gps.dma_start(out=t2[:], in_=pr["ATTD"][l:l + 1, :].to_broadcast([P, 128]))
            attd_t.append(t2)
            t3 = cp.tile([P, 128], F32, tag=f"bout{l}")
            sync.dma_start(out=t3[:], in_=pr["BOUT"][l:l + 1, :].to_broadcast([P, 128]))
            bout_t.append(t3)

        eterm = cp.tile([P, C, 9], BF16)
        pt_all = cp.tile([P, C, BIN], BF16)
        loop_sb = cp.tile([P, NW, 10], F32)
        gsp = ctx.enter_context(tc.tile_pool(name="gsp", bufs=1, space="PSUM"))
        eap = ctx.enter_context(tc.tile_pool(name="eap", bufs=1))
        gsum_ps = None  # allocated lazily at first L4 epilogue
        n_pool_mm = [0]

        z_prev = None  # [P, NW, 128] bf16 from previous layer

        WG = 5  # max windows per epilogue group
        # non-uniform groups: keep the LAST groups small so the serial
        # layer-boundary tail (last epilogue -> node phase -> AllGather) shrinks
        grp_bounds = []
        w0_ = 0
        while NW - w0_ > WG + 2:
            grp_bounds.append((w0_, WG))
            w0_ += WG
        rem = NW - w0_
        if rem > 2:
            grp_bounds.append((w0_, rem - 2))
            grp_bounds.append((w0_ + rem - 2, 2))
        elif rem > 0:
            grp_bounds.append((w0_, rem))
        NG = len(grp_bounds)
        grp_of_win = {}
        for gi, (gw0, gsz_) in enumerate(grp_bounds):
            for w_ in range(gw0, gw0 + gsz_):
                grp_of_win[w_] = gi
        last_chunk_of_grp = {}
        for b in range(NBINS):
            if b in last_chunk_of_bin:
                g_ = grp_of_win[win_of_bin[b]]
                last_chunk_of_grp[g_] = max(last_chunk_of_grp.get(g_, -1),
                                            last_chunk_of_bin[b])

        for l in range(4):
            HW, AW, RW, EL = (LP[l][k] for k in ("HW", "AW", "RW", "EL"))

            # ============ node phase: build T_loc rows for own nodes
            T_sb = wp.tile([P, NW, EL], BF16, tag="tsb")
            act.memzero(T_sb[:])   # pad cols are DMA'd to the table; keep them finite
            for g_ in range(NG):
                w0, gsz = grp_bounds[g_]
                for w_ in range(w0, w0 + gsz):
                    if l == 0:
                        hps = pp.tile([P, 128], F32, tag="hps")
                        pe.matmul(out=hps[:, 0:HW], lhsT=xT_sb[:, w_ * P:(w_ + 1) * P],
                                  rhs=w1_sb[:], start=True, stop=True)
                    else:
                        ztp = pp.tile([P, P], BF16, tag="ztp", bufs=1)
                        pe.transpose(out=ztp[:], in_=z_prev[:, w_, :], identity=identb[:])
                        zt_sb = wp.tile([P, P], BF16, tag="ztsb")
                        act.copy(out=zt_sb[:], in_=ztp[:])
                        hps = pp.tile([P, 128], F32, tag="hps")
                        pe.matmul(out=hps[:, 0:HW], lhsT=zt_sb[:], rhs=wl_sb[l][:],
                                  start=True, stop=True)
                    act.copy(out=T_sb[:, w_, 0:HW], in_=hps[:, 0:HW])
                # batched a_s / a_d over the window group
                tmpf = wp.tile([P, WG, 128], F32, tag="tmpf")
                asf = wp.tile([P, WG, 8], F32, tag="asf")
                for which, attt in ((0, atts_t[l]), (1, attd_t[l])):
                    vec.tensor_tensor(
                        out=tmpf[:, 0:gsz, 0:HW], in0=T_sb[:, w0:w0 + gsz, 0:HW],
                        in1=attt[:, 0:HW].unsqueeze(1).to_broadcast([P, gsz, HW]),
                        op=ALU.mult)
                    vec.tensor_reduce(
                        out=asf[:, 0:gsz, which * 4:which * 4 + AW],
                        in_=tmpf[:, 0:gsz, 0:HW].rearrange("p g (c a) -> p g a c", a=AW),
                        axis=AX.X, op=ALU.add)
                act.copy(out=T_sb[:, w0:w0 + gsz, HW:HW + AW], in_=asf[:, 0:gsz, 0:AW])
                act.copy(out=T_sb[:, w0:w0 + gsz, HW + AW:HW + 2 * AW],
                         in_=asf[:, 0:gsz, 4:4 + AW])
                for w_ in range(w0, w0 + gsz):
                    nr = min(P, NPC - w_ * P)
                    sync.dma_start(out=T_loc[l][w_ * P:w_ * P + nr, :],
                                   in_=T_sb[0:nr, w_, :])

            if SIM1:
                gps.dma_start(out=T_glob[l][0:NPC, :], in_=T_loc[l][:, :])
            else:
                gps.collective_compute(
                    "AllGather", ALU.bypass, replica_groups=[list(range(NCORES))],
                    ins=[T_loc[l][:, :]], outs=[T_glob[l][:, :]])

            # ============ edge phase
            grp_tiles = {}
            grp_done = set()

            def open_group(g_):
                t = vp.tile([P, WG, 146], F32, name="wingrp", tag="wingrp")
                act.memzero(t[:])
                grp_tiles[g_] = t
                return t

            def epilogue_group(g_):
                w0, gsz = grp_bounds[g_]
                wg = grp_tiles[g_]
                scr = wp.tile([P, WG, 12], F32, name="scr", tag="scr")
                # self-loop alpha -> exp
                vec.tensor_tensor(out=scr[:, 0:gsz, 0:AW],
                                  in0=T_sb[:, w0:w0 + gsz, HW:HW + AW],
                                  in1=T_sb[:, w0:w0 + gsz, HW + AW:HW + 2 * AW],
                                  op=ALU.add)
                if l > 0:
                    sl = [None, (0, 4), (4, 8), (8, 9)][l]
                    vec.tensor_tensor(out=scr[:, 0:gsz, 0:AW], in0=scr[:, 0:gsz, 0:AW],
                                      in1=loop_sb[:, w0:w0 + gsz, sl[0]:sl[1]],
                                      op=ALU.add)
                vec.tensor_scalar_mul(out=scr[:, 0:gsz, 4:4 + AW],
                                      in0=scr[:, 0:gsz, 0:AW], scalar1=0.2)
                vec.tensor_tensor(out=scr[:, 0:gsz, 0:AW], in0=scr[:, 0:gsz, 0:AW],
                                  in1=scr[:, 0:gsz, 4:4 + AW], op=ALU.max)
                act.activation(out=scr[:, 0:gsz, 0:AW], in_=scr[:, 0:gsz, 0:AW],
                               func=AF.Exp)
                # num += h_own * ex_loop
                nt = wp.tile([P, WG, 128], F32, name="nt", tag="nt")
                vec.tensor_tensor(
                    out=nt[:, 0:gsz, 0:HW].rearrange("p g (c a) -> p g c a", a=AW),
                    in0=T_sb[:, w0:w0 + gsz, 0:HW].rearrange("p g (c a) -> p g c a", a=AW),
                    in1=scr[:, 0:gsz, 0:AW].unsqueeze(2)
                        .to_broadcast([P, gsz, HW // AW, AW]),
                    op=ALU.mult)
                vec.tensor_tensor(out=wg[:, 0:gsz, 0:HW], in0=wg[:, 0:gsz, 0:HW],
                                  in1=nt[:, 0:gsz, 0:HW], op=ALU.add)
                # den -> reciprocal
                vec.tensor_tensor(out=scr[:, 0:gsz, 4:4 + AW],
                                  in0=wg[:, 0:gsz, HW:HW + AW],
                                  in1=scr[:, 0:gsz, 0:AW], op=ALU.add)
                vec.tensor_scalar_add(out=scr[:, 0:gsz, 4:4 + AW],
                                      in0=scr[:, 0:gsz, 4:4 + AW], scalar1=1e-16)
                vec.reciprocal(out=scr[:, 0:gsz, 4:4 + AW], in_=scr[:, 0:gsz, 4:4 + AW])
                if l == 0:
                    vec.tensor_scalar_max(out=scr[:, 0:gsz, 8:9],
                                          in0=wg[:, 0:gsz, 145:146], scalar1=1.0)
                    vec.reciprocal(out=scr[:, 0:gsz, 8:9], in_=scr[:, 0:gsz, 8:9])
                    vec.tensor_tensor(
                        out=loop_sb[:, w0:w0 + gsz, 0:9], in0=wg[:, 0:gsz, 136:145],
                        in1=scr[:, 0:gsz, 8:9].to_broadcast([P, gsz, 9]), op=ALU.mult)
                # z = num * recip(den) + bias [+ relu]
                vec.tensor_tensor(
                    out=wg[:, 0:gsz, 0:HW].rearrange("p g (c a) -> p g c a", a=AW),
                    in0=wg[:, 0:gsz, 0:HW].rearrange("p g (c a) -> p g c a", a=AW),
                    in1=scr[:, 0:gsz, 4:4 + AW].unsqueeze(2)
                        .to_broadcast([P, gsz, HW // AW, AW]),
                    op=ALU.mult)
                vec.tensor_tensor(
                    out=wg[:, 0:gsz, 0:HW], in0=wg[:, 0:gsz, 0:HW],
                    in1=bout_t[l][:, 0:HW].unsqueeze(1).to_broadcast([P, gsz, HW]),
                    op=ALU.add)
                if l < 3:
                    act.activation(out=z_next[:, w0:w0 + gsz, :], in_=wg[:, 0:gsz, 0:128],
                                   func=AF.Relu)
                else:
                    nonlocal gsum_ps
                    pool_sb = wp.tile([P, WG, 33], BF16, name="pool_sb", tag="poolsb")
                    act.copy(out=pool_sb[:, 0:gsz, 0:32], in_=wg[:, 0:gsz, 0:32])
                    vec.memset(pool_sb[:, 0:gsz, 32:33], 1.0)
                    bt = wp.tile([P, WG, Gn], BF16, name="bt", tag="bt")
                    vec.tensor_tensor(
                        out=bt[:, 0:gsz, :],
                        in0=batcht[:, w0:w0 + gsz].unsqueeze(2).to_broadcast([P, gsz, Gn]),
                        in1=iotagf[:].unsqueeze(1).to_broadcast([P, gsz, Gn]),
                        op=ALU.is_equal)
                    if gsum_ps is None:
                        gsum_ps = gsp.tile([33, Gn], F32, name="gsum_ps")
                    for j_ in range(gsz):
                        n_pool_mm[0] += 1
                        pe.matmul(out=gsum_ps[:], lhsT=pool_sb[:, j_, :],
                                  rhs=bt[:, j_, :],
                                  start=(n_pool_mm[0] == 1),
                                  stop=(n_pool_mm[0] == NW))
                grp_done.add(g_)

            if l < 3:
                z_next = wp.tile([P, NW, 128], BF16, tag="zsb")

            cur_bin_tile = {}
            for ss in range(NSS):
                Gt = wp.tile([P, SS, EL], BF16, tag="gt", bufs=3)
                gps.dma_gather(
                    out_ap=Gt[:, :, :], in_ap=T_glob[l][:, :],
                    idxs_ap=src16[:, ss * SS * 8:(ss + 1) * SS * 8],
                    num_idxs=SS * CHUNK, num_idxs_reg=SS * CHUNK, elem_size=EL,
                    single_packet=False, queue_num=ss % 2)
                if l == 0:
                    # edge-term precompute (feeds rhs cols 136:145 + later layers)
                    eaT_sl = eap.tile([4, SS * CHUNK], F32, name="easl", tag="eat")
                    half = SS * CHUNK // 2
                    for hf in range(2):
                        sync.dma_start(
                            out=eaT_sl[:, hf * half:(hf + 1) * half],
                            in_=pr["EAT"][:, ss * SS * CHUNK + hf * half:
                                          ss * SS * CHUNK + (hf + 1) * half])
                    for q in range(SS // 4):
                        etp = pp.tile([P, 36], F32, tag="etp", bufs=1)
                        for j in range(4):
                            ci = q * 4 + j
                            pe.matmul(out=etp[:, j * 9:(j + 1) * 9],
                                      lhsT=eaT_sl[:, ci * CHUNK:(ci + 1) * CHUNK],
                                      rhs=w4x9_sb[:], start=True, stop=True)
                        vec.tensor_tensor(
                            out=eterm[:, ss * SS + q * 4:ss * SS + q * 4 + 4, :]
                                .rearrange("p a b -> p (a b)"),
                            in0=etp[:], in1=be9r[:], op=ALU.add)
                    # mask so dummy edges don't pollute the loop-eterm sums
                    vec.tensor_tensor(
                        out=Gt[:, :, 136:145],
                        in0=eterm[:, ss * SS:(ss + 1) * SS, :],
                        in1=maskt[:, ss * SS:(ss + 1) * SS].unsqueeze(2)
                            .to_broadcast([P, SS, 9]),
                        op=ALU.mult)
                    act.copy(out=Gt[:, :, 145:146],
                             in_=maskt[:, ss * SS:(ss + 1) * SS].unsqueeze(2))
                    # staircase one-hots built once, reused by all layers
                    for g in range(SS // 8):
                        s0 = ss * SS + g * 8
                        vec.tensor_tensor(
                            out=pt_all[:, s0:s0 + 8, :],
                            in0=dstr[:, s0:s0 + 8].unsqueeze(2).to_broadcast([P, 8, BIN]),
                            in1=iota32f[:].unsqueeze(1).to_broadcast([P, 8, BIN]),
                            op=ALU.is_equal)
                # alpha
                AT = wp.tile([P, SS, 8], BF16, tag="at", bufs=2)
                vec.tensor_tensor(out=AT[:, :, 0:AW], in0=Gt[:, :, HW:HW + AW],
                                  in1=Gt[:, :, HW + AW:HW + 2 * AW], op=ALU.add)
                if l > 0:
                    sl = [None, (0, 4), (4, 8), (8, 9)][l]
                    vec.tensor_tensor(out=AT[:, :, 0:AW], in0=AT[:, :, 0:AW],
                                      in1=eterm[:, ss * SS:(ss + 1) * SS, sl[0]:sl[1]],
                                      op=ALU.add)
                vec.tensor_scalar_mul(out=AT[:, :, AW:2 * AW], in0=AT[:, :, 0:AW],
                                      scalar1=0.2)
                vec.tensor_tensor(out=AT[:, :, 0:AW], in0=AT[:, :, 0:AW],
                                  in1=AT[:, :, AW:2 * AW], op=ALU.max)
                vec.tensor_tensor(
                    out=AT[:, :, 0:AW], in0=AT[:, :, 0:AW],
                    in1=abias[:, ss * SS:(ss + 1) * SS].unsqueeze(2)
                        .to_broadcast([P, SS, AW]),
                    op=ALU.add)
                act.activation(out=Gt[:, :, HW:HW + AW], in_=AT[:, :, 0:AW],
                               func=AF.Exp)
                vec.tensor_tensor(
                    out=Gt[:, :, 0:HW].rearrange("p s (c a) -> p s c a", a=AW),
                    in0=Gt[:, :, 0:HW].rearrange("p s (c a) -> p s c a", a=AW),
                    in1=Gt[:, :, HW:HW + AW].unsqueeze(2)
                        .to_broadcast([P, SS, HW // AW, AW]),
                    op=ALU.mult)
                # scatter matmuls
                for c_i in range(SS):
                    gc = ss * SS + c_i
                    b = bin_of_chunk[gc]
                    w_ = win_of_bin[b]
                    g_ = grp_of_win[w_]
                    if g_ not in grp_tiles:
                        open_group(g_)
                    if gc == first_chunk_of_bin[b]:
                        cur_bin_tile[b] = bp.tile([BIN, 146], F32, name="binacc", tag="binacc")
                    pe.matmul(out=cur_bin_tile[b][:, 0:RW],
                              lhsT=pt_all[:, gc, :], rhs=Gt[:, c_i, 0:RW],
                              start=(gc == first_chunk_of_bin[b]),
                              stop=(gc == last_chunk_of_bin[b]))
                    if gc == last_chunk_of_bin[b]:
                        j = b % 4
                        wrel = w_ - grp_bounds[g_][0]
                        act.copy(out=grp_tiles[g_][BIN * j:BIN * (j + 1), wrel, 0:RW],
                                 in_=cur_bin_tile[b][:, 0:RW])
                        del cur_bin_tile[b]
                    if gc == last_chunk_of_grp.get(g_, None):
                        epilogue_group(g_)
            # groups never triggered (e.g. all-empty windows)
            for g_ in range(NG):
                if g_ not in grp_done:
                    if g_ not in grp_tiles:
                        open_group(g_)
                    epilogue_group(g_)
            z_prev = z_next if l < 3 else None

        # ============ readout
        gsum_sb = cp.tile([33, Gn], F32)
        act.copy(out=gsum_sb[:], in_=gsum_ps[:])
        gps.dma_start(out=ar_in[:], in_=gsum_sb[:])
        if SIM1:
            gps.dma_start(out=ar_out[:], in_=ar_in[:])
        else:
            gps.collective_compute("AllReduce", ALU.add,
                                   replica_groups=[list(range(NCORES))],
                                   ins=[ar_in[:]], outs=[ar_out[:]])
        gs = cp.tile([33, Gn], F32)
        sync.dma_start(out=gs[:], in_=ar_out[:])
        sync.dma_start(out=cnt_dram[:], in_=gs[32:33, :])
        comb = cp.tile([64, Gn], F32)
        cntb = cp.tile([32, Gn], F32)
        sync.dma_start(out=cntb[:], in_=cnt_dram[0:1, :].to_broadcast([32, Gn]))
        vec.tensor_scalar_max(out=cntb[:], in0=cntb[:], scalar1=1.0)
        vec.reciprocal(out=cntb[:], in_=cntb[:])
        vec.tensor_tensor(out=comb[0:32, :], in0=gs[0:32, :], in1=cntb[:],
                          op=ALU.mult)
        wd_sb = cp.tile([48, 32], F32)
        sync.dma_start(out=wd_sb[:], in_=pr["WD"][:, :])
        desct_sb = cp.tile([48, Gn], F32)
        sync.dma_start(out=desct_sb[:], in_=pr["DESCT"][:, :])
        bd_sb = cp.tile([32, 1], F32)
        sync.dma_start(out=bd_sb[:], in_=pr["BD"][:, :])
        dps = pp.tile([32, Gn], F32, tag="hps")
        pe.matmul(out=dps[:], lhsT=wd_sb[:], rhs=desct_sb[:], start=True, stop=True)
        act.activation(out=comb[32:64, :], in_=dps[:], func=AF.Relu, bias=bd_sb[:])
        wlin_sb = cp.tile([64, 1], F32)
        sync.dma_start(out=wlin_sb[:], in_=pr["WLIN"][:, :])
        fin = pp.tile([1, Gn], F32, tag="hps")
        pe.matmul(out=fin[:], lhsT=wlin_sb[:], rhs=comb[:], start=True, stop=True)
        res_sb = cp.tile([1, Gn], F32)
        vec.tensor_scalar_add(out=res_sb[:], in0=fin[:], scalar1=bl)
        act.activation(out=res_sb[:], in_=res_sb[:], func=AF.Sigmoid)
        sync.dma_start(out=out_p[:, :], in_=res_sb[:])

    nc.finalize()
    return nc


# ------------------------------------------------------------------ entry
def _run(inputs, trace=False, debug=False):
    dims, shared, per_core = host_prep(inputs)
    if debug:
        dims["debug"] = True
    nc = build_program(dims, shared)
    in_maps = [{**shared, **pc} for pc in per_core]
    from concourse.bass_utils import run_bass_kernel_spmd
    return run_bass_kernel_spmd(nc, in_maps, list(range(NCORES)), trace=trace)


def kernel(**inputs):
    res = _run(inputs)
    return res.results[0]["out"].reshape(-1).astype(np.float32)



# revision 8
# speedup vs baseline: 1.2111x; 1.2111x over previous
"""EnhancedGAT Trainium2 Bass kernel (8 NeuronCores, SPMD).

Strategy:
  - Edges are sorted by destination node on the host; core k owns dst nodes
    [k*N/8, (k+1)*N/8) and every edge targeting them. Per-core edge lists are
    bucketed into 64-node bins and padded to 128-edge chunks with a per-bin
    chunk count shared across cores (SPMD uniformity). Dummy (padding) edges
    carry dst-offset 64, which falls outside the 64-wide one-hot used by the
    scatter matmuls, so they contribute exactly nothing.
  - Each GAT layer:
      node phase: every core computes a table row [h(128) | b(4)] for its own
        nodes, where b = per-head <h, att_s + att_d> comes directly out of the
        h matmul via 4 extra weight columns W @ A. Rows live in a [NPC, 256]
        bf16 DRAM table (512B stride for the gather); an AllGather replicates
        it to every core.
      edge phase: per 4096-edge superstep one dma_gather pulls the rows for
        the edges' sources; attention coefficients alpha = b[src] (+ edge
        term) are leaky-relu'd and exp'd in place, messages h*ex are scattered
        into per-bin PSUM accumulators via one-hot matmuls. Softmax is
        unnormalized (max-subtraction skipped; alphas are O(0.3)); the divide
        happens per node at the group epilogue, where self-loop contributions
        are added. As soon as a window-group's epilogue finishes, the NEXT
        layer's node phase for those windows runs (transpose + matmul + table
        write), hiding the layer boundary behind the remaining gathers.
  - Layer 1 additionally computes, per edge, the folded edge-attention terms
    for layers 2-4 (eterm = ea @ V + be, with the padding mask folded in as a
    fifth all-ones/zeros EAT row) plus the per-edge mask into an [C,10] SBUF
    cache, and accumulates per-node mean edge-feature terms and in-degrees
    (extra scatter-matmul columns) used by the self-loops of layers 2-4.
  - Final graph mean-pool via one-hot matmuls into a [33, G] accumulator,
    AllReduce across cores, tiny dense readout replicated on every core.
"""
import sys
import numpy as np

sys.path.insert(0, "/opt/trn_rl_repo")

HID = 32
NCORES = 8
P = 128
BIN = 64
SS = 32          # chunks per superstep
CHUNK = 128
ROW = 256        # table row elements (bf16) for layers 1-3 (512B stride)
ROW4 = 128       # layer-4 table row elements


# ----------------------------------------------------------------- host prep
def host_prep(inputs):
    x = np.asarray(inputs["x"], np.float32)
    ei = np.asarray(inputs["edge_index"]).astype(np.int64)
    ea = np.asarray(inputs["edge_attr"], np.float32)
    batch = np.asarray(inputs["batch"]).astype(np.int64)
    desc = np.asarray(inputs["descriptors"], np.float32)

    N = x.shape[0]
    E = ei.shape[1]
    Gn = desc.shape[0]
    NPC = N // NCORES
    NW = -(-NPC // P)
    NBINS = -(-NPC // BIN)

    src_all, dst_all = ei[0], ei[1]
    order = np.argsort(dst_all, kind="stable")
    src_s, dst_s = src_all[order], dst_all[order]
    ea_s = ea[order]
    core_of = dst_s // NPC
    local = dst_s - core_of * NPC
    bin_of = local // BIN

    cnt = np.zeros((NCORES, NBINS), np.int64)
    np.add.at(cnt, (core_of, bin_of), 1)
    cpb = np.max(-(-cnt // CHUNK), axis=0)          # chunks per bin (shared)
    cpb = np.maximum(cpb, 1)                        # every bin gets a chunk
    C_total = int(cpb.sum())
    padc = (-C_total) % SS
    if padc:
        cpb[-1] += padc
        C_total += padc
    off = np.zeros(NBINS, np.int64)
    off[1:] = np.cumsum(cpb)[:-1]
    EP = C_total * CHUNK                            # padded edges per core

    per_core = []
    for k in range(NCORES):
        srck = np.zeros(EP, np.int64)
        dstrk = np.full(EP, float(BIN), np.float32)  # dummies -> dead one-hot
        maskk = np.zeros(EP, np.float32)
        eak = np.zeros((EP, 4), np.float32)
        sel = core_of == k
        bins_k = bin_of[sel]
        start = np.searchsorted(bins_k, np.arange(NBINS))
        pos = np.arange(bins_k.size) - start[bins_k]
        slot = off[bins_k] * CHUNK + pos
        srck[slot] = src_s[sel]
        dstrk[slot] = (local[sel] - bins_k * BIN).astype(np.float32)
        maskk[slot] = 1.0
        eak[slot] = ea_s[sel]

        # device layouts: edge e = c*128 + p
        src16 = np.tile(srck.reshape(-1, 16).T.astype(np.int16), (8, 1))
        dstr_d = dstrk.reshape(C_total, P).T.copy()
        ea5 = np.concatenate([eak.T, maskk[None, :]], axis=0)  # [5, EP]

        xk = x[k * NPC:(k + 1) * NPC]
        xT = np.zeros((8, NW * P), np.float32)
        xT[:, :NPC] = xk.T
        bk = np.full(NW * P, Gn + 5, np.float32)
        bk[:NPC] = batch[k * NPC:(k + 1) * NPC].astype(np.float32)
        batch_d = bk.reshape(NW, P).T.copy()

        per_core.append(dict(SRC16=src16, DSTR=dstr_d, EAT=ea5,
                             XT=xT, BATCH=batch_d))

    # ---- weight folding
    w = {k: np.asarray(v, np.float32) for k, v in inputs.items()
         if k not in ("x", "edge_index", "edge_attr", "batch", "descriptors")}

    def vfold(We, ae, heads):
        Vp = (We.reshape(w["We_enc"].shape[1], heads, HID) * ae[None]).sum(-1)
        return w["We_enc"] @ Vp, w["be_enc"] @ Vp      # [4,heads],[heads]

    V2, bv2 = vfold(w["We2"], w["ae2"], 4)
    V3, bv3 = vfold(w["We3"], w["ae3"], 4)
    V4, bv4 = vfold(w["We4"], w["ae4"], 1)
    # [5,10]: rows = 4 edge-attr dims + mask; cols = 9 eterms + cnt
    W5x10 = np.zeros((5, 10), np.float32)
    W5x10[0:4, 0:9] = np.concatenate([V2, V3, V4], axis=1)
    W5x10[4, 0:9] = np.concatenate([bv2, bv3, bv4])
    W5x10[4, 9] = 1.0

    def padr(v, n):
        o = np.zeros(n, np.float32)
        o[: v.size] = v
        return o

    # channel-major reorder of the 128-wide (4 heads x 32 ch) dimension:
    # new position c*4+a holds old a*32+c. Keeps per-head broadcasts
    # innermost-packed on DVE (2x mode).
    cm = (np.arange(128) % 4) * 32 + np.arange(128) // 4

    def wext(W, att_s, att_d, heads):
        # append per-head b-columns: b_a = h . (att_s+att_d)_a
        att = (att_s + att_d).reshape(-1)  # [heads*HID] head-major
        if heads == 4:
            attc = att[cm]                 # channel-major to match W cols
            A = np.zeros((128, 4), np.float32)
            A[np.arange(128), np.arange(128) % 4] = attc
        else:
            A = att[:, None]               # [32,1]
        return np.concatenate([W, W @ A], axis=1)

    W1e = wext(w["W1"][:, cm], w["as1"], w["ad1"], 4)            # [8,132]
    W2e = wext(w["W2"][cm][:, cm], w["as2"], w["ad2"], 4)        # [128,132]
    W3e = wext(w["W3"][cm][:, cm], w["as3"], w["ad3"], 4)
    W4e = wext(w["W4"][cm], w["as4"], w["ad4"], 1)               # [128,33]

    bout = np.stack([padr(w["b1"][cm], 128), padr(w["b2"][cm], 128),
                     padr(w["b3"][cm], 128), padr(w["b4"], 128)])

    shared = dict(
        W1=W1e, WL2=W2e, WL3=W3e, WL4=W4e,
        W5X10=W5x10, BOUT=bout,
        WD=w["Wd"], BD=w["bd"][:, None], WLIN=w["Wl"], DESCT=desc.T.copy(),
    )
    bl = float(np.asarray(w["bl"]).reshape(-1)[0])

    dims = dict(N=N, E=E, Gn=Gn, NPC=NPC, NW=NW, NBINS=NBINS,
                C=C_total, cpb=cpb, off=off, bl=bl)
    return dims, shared, per_core


# ------------------------------------------------------------- program build
def build_program(dims, shared):
    import concourse.bass as bass
    import concourse.mybir as mybir
    import concourse.tile as tile
    import concourse.bacc as bacc
    from concourse.masks import make_identity
    from contextlib import ExitStack

    F32 = mybir.dt.float32
    BF16 = mybir.dt.bfloat16
    I32 = mybir.dt.int32
    I16 = mybir.dt.int16
    AF = mybir.ActivationFunctionType
    ALU = mybir.AluOpType
    AX = mybir.AxisListType

    N, Gn, NPC, NW, NBINS, C = (dims[k] for k in ("N", "Gn", "NPC", "NW", "NBINS", "C"))
    cpb, off, bl = dims["cpb"], dims["off"], dims["bl"]
    NSS = C // SS
    # layer params: h width, heads, rhs width, gather row elems
    LP = [dict(HW=128, AW=4, RW=142, EL=ROW),   # L1 rhs: h,ex,eterm9,cnt
          dict(HW=128, AW=4, RW=132, EL=ROW),
          dict(HW=128, AW=4, RW=132, EL=ROW),
          dict(HW=32, AW=1, RW=33, EL=ROW4)]

    nc = bacc.Bacc(num_swdge_queues=2)
    SIM1 = dims.get("sim1", False)

    # ---- params
    pr = {}
    for nm, shp, dt in [("SRC16", [P, C * 8], I16), ("DSTR", [P, C], F32),
                        ("EAT", [5, C * CHUNK], F32), ("XT", [8, NW * P], F32),
                        ("BATCH", [P, NW], F32), ("W1", [8, 132], F32),
                        ("WL2", [128, 132], F32), ("WL3", [128, 132], F32),
                        ("WL4", [128, 33], F32), ("W5X10", [5, 10], F32),
                        ("BOUT", [4, 128], F32),
                        ("WD", [48, 32], F32), ("BD", [32, 1], F32),
                        ("WLIN", [64, 1], F32), ("DESCT", [48, Gn], F32)]:
        pr[nm] = nc.declare_dram_parameter(nm, shp, dt, isOutput=False)
    out_p = nc.declare_dram_parameter("out", [1, Gn], F32, isOutput=True)

    # ---- internal DRAM
    T_loc = [nc.dram_tensor(f"T_loc{l}", [NPC, LP[l]["EL"]], BF16) for l in range(4)]
    T_glob = [nc.dram_tensor(f"T_glob{l}", [N, LP[l]["EL"]], BF16, addr_space="Shared")
              for l in range(4)]
    ar_in = nc.dram_tensor("ar_in", [33, Gn], F32)
    ar_out = nc.dram_tensor("ar_out", [33, Gn], F32, addr_space="Shared")

    # bin/window bookkeeping (compile-time)
    bin_of_chunk = []
    for b in range(NBINS):
        bin_of_chunk += [b] * int(cpb[b])
    BPW = P // BIN  # bins per window
    win_of_bin = [b // BPW for b in range(NBINS)]
    last_chunk_of_bin = {}
    first_chunk_of_bin = {}
    for c_i, b in enumerate(bin_of_chunk):
        last_chunk_of_bin[b] = c_i
        first_chunk_of_bin.setdefault(b, c_i)

    with tile.TileContext(nc) as tc, ExitStack() as ctx:
        cp = ctx.enter_context(tc.tile_pool(name="const", bufs=1))
        wp = ctx.enter_context(tc.tile_pool(name="work", bufs=2))
        vp = ctx.enter_context(tc.tile_pool(name="win", bufs=3))
        pp = ctx.enter_context(tc.tile_pool(name="psum", bufs=2, space="PSUM"))
        bp = ctx.enter_context(tc.tile_pool(name="binp", bufs=2, space="PSUM"))

        sync, gps, vec, act, pe = nc.sync, nc.gpsimd, nc.vector, nc.scalar, nc.tensor

        # ---- resident tiles
        src16 = cp.tile([P, C * 8], I16)
        sync.dma_start(out=src16[:], in_=pr["SRC16"][:, :])
        dstr = cp.tile([P, C], BF16)
        gps.dma_start(out=dstr[:], in_=pr["DSTR"][:, :])   # f32 -> bf16 cast
        batcht = cp.tile([P, NW], F32)
        sync.dma_start(out=batcht[:], in_=pr["BATCH"][:, :])
        xT_sb = cp.tile([8, NW * P], F32)
        sync.dma_start(out=xT_sb[:], in_=pr["XT"][:, :])

        iota_i = cp.tile([P, BIN], I32)
        gps.iota(iota_i[:], pattern=[[1, BIN]], base=0, channel_multiplier=0)
        iotab = cp.tile([P, BIN], BF16)
        vec.tensor_copy(iotab[:], iota_i[:])
        iotag_i = cp.tile([P, Gn], I32)
        gps.iota(iotag_i[:], pattern=[[1, Gn]], base=0, channel_multiplier=0)
        iotagf = cp.tile([P, Gn], F32)
        vec.tensor_copy(iotagf[:], iotag_i[:])
        identb = cp.tile([P, P], BF16)
        make_identity(nc, identb[:])

        w1_sb = cp.tile([8, 132], F32)
        sync.dma_start(out=w1_sb[:], in_=pr["W1"][:, :])
        wl_sb = [None,
                 cp.tile([128, 132], BF16, name="wl2", tag="wl2"),
                 cp.tile([128, 132], BF16, name="wl3", tag="wl3"),
                 cp.tile([128, 33], BF16, name="wl4", tag="wl4")]
        gps.dma_start(out=wl_sb[1][:], in_=pr["WL2"][:, :])   # gpsimd casts f32->bf16
        gps.dma_start(out=wl_sb[2][:], in_=pr["WL3"][:, :])
        gps.dma_start(out=wl_sb[3][:], in_=pr["WL4"][:, :])
        w5x10 = cp.tile([5, 10], BF16)
        gps.dma_start(out=w5x10[:], in_=pr["W5X10"][:, :])
        bout_t = []
        for l in range(4):
            t3 = cp.tile([P, 128], F32, tag=f"bout{l}")
            sync.dma_start(out=t3[:], in_=pr["BOUT"][l:l + 1, :].to_broadcast([P, 128]))
            bout_t.append(t3)

        etc = cp.tile([P, C, 10], BF16)      # eterm9 | cnt  per edge
        pt_all = cp.tile([P, C, BIN], BF16)  # one-hot dst rows per edge
        loop_sb = cp.tile([P, NW, 10], F32)
        gsp = ctx.enter_context(tc.tile_pool(name="gsp", bufs=1, space="PSUM"))
        eap = ctx.enter_context(tc.tile_pool(name="eap", bufs=1))
        gsum_ps = None
        n_pool_mm = [0]

        # ---- readout head start: descriptor branch is input-independent
        comb = cp.tile([64, Gn], F32)
        wd_sb = cp.tile([48, 32], F32)
        sync.dma_start(out=wd_sb[:], in_=pr["WD"][:, :])
        desct_sb = cp.tile([48, Gn], F32)
        sync.dma_start(out=desct_sb[:], in_=pr["DESCT"][:, :])
        bd_sb = cp.tile([32, 1], F32)
        sync.dma_start(out=bd_sb[:], in_=pr["BD"][:, :])
        dps = pp.tile([32, Gn], F32, tag="hps")
        pe.matmul(out=dps[:], lhsT=wd_sb[:], rhs=desct_sb[:], start=True, stop=True)
        act.activation(out=comb[32:64, :], in_=dps[:], func=AF.Relu, bias=bd_sb[:])
        wlin_sb = cp.tile([64, 1], F32)
        sync.dma_start(out=wlin_sb[:], in_=pr["WLIN"][:, :])
        onesf = cp.tile([1, 32], F32)
        vec.memset(onesf[:], 1.0)

        WG = 5  # max windows per epilogue group
        # non-uniform groups: keep the LAST groups small so the serial
        # layer-boundary tail (last epilogue -> node phase -> AllGather) shrinks
        grp_bounds = []
        w0_ = 0
        while NW - w0_ > WG + 2:
            grp_bounds.append((w0_, WG))
            w0_ += WG
        rem = NW - w0_
        if rem > 2:
            grp_bounds.append((w0_, rem - 2))
            grp_bounds.append((w0_ + rem - 2, 2))
        elif rem > 0:
            grp_bounds.append((w0_, rem))
        NG = len(grp_bounds)
        grp_of_win = {}
        for gi, (gw0, gsz_) in enumerate(grp_bounds):
            for w_ in range(gw0, gw0 + gsz_):
                grp_of_win[w_] = gi
        last_chunk_of_grp = {}
        for b in range(NBINS):
            g_ = grp_of_win[win_of_bin[b]]
            last_chunk_of_grp[g_] = max(last_chunk_of_grp.get(g_, -1),
                                        last_chunk_of_bin[b])

        # T_sb tables: [P, NW, 132] (h | b); layer l+1's is built during
        # layer l's edge phase, group by group.
        def node_phase_group(l, g_, T_next, z_src):
            """Build T_next rows for group g_ of layer l (0-based), write T_loc."""
            w0, gsz = grp_bounds[g_]
            HWn = LP[l]["HW"]
            BW = HWn + LP[l]["AW"]  # table row width
            for w_ in range(w0, w0 + gsz):
                if l == 0:
                    hps = pp.tile([P, 132], F32, tag="hps")
                    pe.matmul(out=hps[:, 0:BW], lhsT=xT_sb[:, w_ * P:(w_ + 1) * P],
                              rhs=w1_sb[:], start=True, stop=True)
                else:
                    ztp = pp.tile([P, P], BF16, tag="ztp", bufs=1)
                    pe.transpose(out=ztp[:], in_=z_src[:, w_, :], identity=identb[:])
                    zt_sb = wp.tile([P, P], BF16, tag="ztsb")
                    act.copy(out=zt_sb[:], in_=ztp[:])
                    hps = pp.tile([P, 132], F32, tag="hps")
                    pe.matmul(out=hps[:, 0:BW], lhsT=zt_sb[:], rhs=wl_sb[l][:],
                              start=True, stop=True)
                act.copy(out=T_next[:, w_, 0:BW], in_=hps[:, 0:BW])
                nr = min(P, NPC - w_ * P)
                sync.dma_start(out=T_loc[l][w_ * P:w_ * P + nr, 0:BW],
                               in_=T_next[0:nr, w_, 0:BW])

        # ---- layer 0 node phase (all groups up front)
        T_sb = wp.tile([P, NW, 132], BF16, tag="tsb")
        for g_ in range(NG):
            node_phase_group(0, g_, T_sb, None)
        if SIM1:
            gps.dma_start(out=T_glob[0][0:NPC, :], in_=T_loc[0][:, :])
        else:
            gps.collective_compute(
                "AllGather", ALU.bypass, replica_groups=[list(range(NCORES))],
                ins=[T_loc[0][:, :]], outs=[T_glob[0][:, :]])

        for l in range(4):
            HW, AW, RW, EL = (LP[l][k] for k in ("HW", "AW", "RW", "EL"))
            BW = HW + AW

            T_next = None
            if l < 3:
                T_next = wp.tile([P, NW, 132], BF16, name="tnext", tag="tsb")
                z_next = wp.tile([P, NW, 128], BF16, name="znext", tag="zsb")

            grp_tiles = {}
            grp_done = set()

            def open_group(g_):
                t = vp.tile([P, WG, 142], F32, name="wingrp", tag="wingrp")
                grp_tiles[g_] = t
                return t

            def epilogue_group(g_):
                w0, gsz = grp_bounds[g_]
                wg = grp_tiles[g_]
                scr = wp.tile([P, WG, 12], F32, name="scr", tag="scr")
                # self-loop alpha (= b_own [+ eterm means]) -> exp
                if l > 0:
                    sl = [None, (0, 4), (4, 8), (8, 9)][l]
                    vec.tensor_tensor(out=scr[:, 0:gsz, 0:AW],
                                      in0=T_sb[:, w0:w0 + gsz, HW:HW + AW],
                                      in1=loop_sb[:, w0:w0 + gsz, sl[0]:sl[1]],
                                      op=ALU.add)
                else:
                    act.copy(out=scr[:, 0:gsz, 0:AW],
                             in_=T_sb[:, w0:w0 + gsz, HW:HW + AW])
                vec.tensor_scalar_mul(out=scr[:, 0:gsz, 4:4 + AW],
                                      in0=scr[:, 0:gsz, 0:AW], scalar1=0.2)
                vec.tensor_tensor(out=scr[:, 0:gsz, 0:AW], in0=scr[:, 0:gsz, 0:AW],
                                  in1=scr[:, 0:gsz, 4:4 + AW], op=ALU.max)
                act.activation(out=scr[:, 0:gsz, 0:AW], in_=scr[:, 0:gsz, 0:AW],
                               func=AF.Exp)
                # num += h_own * ex_loop
                nt = wp.tile([P, WG, 128], F32, name="nt", tag="nt")
                vec.tensor_tensor(
                    out=nt[:, 0:gsz, 0:HW].rearrange("p g (c a) -> p g c a", a=AW),
                    in0=T_sb[:, w0:w0 + gsz, 0:HW].rearrange("p g (c a) -> p g c a", a=AW),
                    in1=scr[:, 0:gsz, 0:AW].unsqueeze(2)
                        .to_broadcast([P, gsz, HW // AW, AW]),
                    op=ALU.mult)
                vec.tensor_tensor(out=wg[:, 0:gsz, 0:HW], in0=wg[:, 0:gsz, 0:HW],
                                  in1=nt[:, 0:gsz, 0:HW], op=ALU.add)
                # den -> reciprocal
                vec.tensor_tensor(out=scr[:, 0:gsz, 4:4 + AW],
                                  in0=wg[:, 0:gsz, HW:HW + AW],
                                  in1=scr[:, 0:gsz, 0:AW], op=ALU.add)
                vec.tensor_scalar_add(out=scr[:, 0:gsz, 4:4 + AW],
                                      in0=scr[:, 0:gsz, 4:4 + AW], scalar1=1e-16)
                vec.reciprocal(out=scr[:, 0:gsz, 4:4 + AW], in_=scr[:, 0:gsz, 4:4 + AW])
                if l == 0:
                    vec.tensor_scalar_max(out=scr[:, 0:gsz, 8:9],
                                          in0=wg[:, 0:gsz, 141:142], scalar1=1.0)
                    vec.reciprocal(out=scr[:, 0:gsz, 8:9], in_=scr[:, 0:gsz, 8:9])
                    vec.tensor_tensor(
                        out=loop_sb[:, w0:w0 + gsz, 0:9], in0=wg[:, 0:gsz, 132:141],
                        in1=scr[:, 0:gsz, 8:9].to_broadcast([P, gsz, 9]), op=ALU.mult)
                # z = num * recip(den) + bias [+ relu]
                vec.tensor_tensor(
                    out=wg[:, 0:gsz, 0:HW].rearrange("p g (c a) -> p g c a", a=AW),
                    in0=wg[:, 0:gsz, 0:HW].rearrange("p g (c a) -> p g c a", a=AW),
                    in1=scr[:, 0:gsz, 4:4 + AW].unsqueeze(2)
                        .to_broadcast([P, gsz, HW // AW, AW]),
                    op=ALU.mult)
                vec.tensor_tensor(
                    out=wg[:, 0:gsz, 0:HW], in0=wg[:, 0:gsz, 0:HW],
                    in1=bout_t[l][:, 0:HW].unsqueeze(1).to_broadcast([P, gsz, HW]),
                    op=ALU.add)
                if l < 3:
                    act.activation(out=z_next[:, w0:w0 + gsz, :], in_=wg[:, 0:gsz, 0:128],
                                   func=AF.Relu)
                    node_phase_group(l + 1, g_, T_next, z_next)
                else:
                    nonlocal gsum_ps
                    pool_sb = wp.tile([P, WG, 33], BF16, name="pool_sb", tag="poolsb")
                    act.copy(out=pool_sb[:, 0:gsz, 0:32], in_=wg[:, 0:gsz, 0:32])
                    vec.memset(pool_sb[:, 0:gsz, 32:33], 1.0)
                    bt = wp.tile([P, WG, Gn], BF16, name="bt", tag="bt")
                    vec.tensor_tensor(
                        out=bt[:, 0:gsz, :],
                        in0=batcht[:, w0:w0 + gsz].unsqueeze(2).to_broadcast([P, gsz, Gn]),
                        in1=iotagf[:].unsqueeze(1).to_broadcast([P, gsz, Gn]),
                        op=ALU.is_equal)
                    if gsum_ps is None:
                        gsum_ps = gsp.tile([33, Gn], F32, name="gsum_ps")
                    for j_ in range(gsz):
                        n_pool_mm[0] += 1
                        pe.matmul(out=gsum_ps[:], lhsT=pool_sb[:, j_, :],
                                  rhs=bt[:, j_, :],
                                  start=(n_pool_mm[0] == 1),
                                  stop=(n_pool_mm[0] == NW))
                grp_done.add(g_)

            cur_bin_tile = {}
            for ss in range(NSS):
                Gt = wp.tile([P, SS, EL], BF16, tag="gt", bufs=3)
                gps.dma_gather(
                    out_ap=Gt[:, :, :], in_ap=T_glob[l][:, :],
                    idxs_ap=src16[:, ss * SS * 8:(ss + 1) * SS * 8],
                    num_idxs=SS * CHUNK, num_idxs_reg=SS * CHUNK, elem_size=EL,
                    single_packet=False, queue_num=ss % 2)
                if l == 0:
                    # edge-term + mask precompute (feeds rhs cols 132:142 +
                    # later layers' alpha); mask folded into EAT row 5.
                    eaT_sl = eap.tile([5, SS * CHUNK], BF16, name="easl", tag="eat")
                    gps.dma_start(
                        out=eaT_sl[:],
                        in_=pr["EAT"][:, ss * SS * CHUNK:(ss + 1) * SS * CHUNK])
                    for q in range(SS // 4):
                        etp = pp.tile([P, 40], F32, tag="etp", bufs=1)
                        for j in range(4):
                            ci = q * 4 + j
                            pe.matmul(out=etp[:, j * 10:(j + 1) * 10],
                                      lhsT=eaT_sl[:, ci * CHUNK:(ci + 1) * CHUNK],
                                      rhs=w5x10[:], start=True, stop=True)
                        act.copy(out=etc[:, ss * SS + q * 4:ss * SS + q * 4 + 4, :]
                                 .rearrange("p a b -> p (a b)"),
                                 in_=etp[:])
                    # staircase one-hots built once, reused by all layers
                    for g in range(SS // 8):
                        s0 = ss * SS + g * 8
                        vec.tensor_tensor(
                            out=pt_all[:, s0:s0 + 8, :],
                            in0=dstr[:, s0:s0 + 8].unsqueeze(2).to_broadcast([P, 8, BIN]),
                            in1=iotab[:].unsqueeze(1).to_broadcast([P, 8, BIN]),
                            op=ALU.is_equal)
                # alpha = b[src] (+ eterm) -> leaky relu -> exp
                AT = wp.tile([P, SS, 8], BF16, tag="at", bufs=2)
                if l > 0:
                    sl = [None, (0, 4), (4, 8), (8, 9)][l]
                    vec.tensor_tensor(out=AT[:, :, 0:AW], in0=Gt[:, :, HW:HW + AW],
                                      in1=etc[:, ss * SS:(ss + 1) * SS, sl[0]:sl[1]],
                                      op=ALU.add)
                    vec.tensor_scalar_mul(out=AT[:, :, AW:2 * AW], in0=AT[:, :, 0:AW],
                                          scalar1=0.2)
                    vec.tensor_tensor(out=AT[:, :, 0:AW], in0=AT[:, :, 0:AW],
                                      in1=AT[:, :, AW:2 * AW], op=ALU.max)
                else:
                    vec.tensor_scalar_mul(out=AT[:, :, AW:2 * AW],
                                          in0=Gt[:, :, HW:HW + AW], scalar1=0.2)
                    vec.tensor_tensor(out=AT[:, :, 0:AW], in0=Gt[:, :, HW:HW + AW],
                                      in1=AT[:, :, AW:2 * AW], op=ALU.max)
                act.activation(out=Gt[:, :, HW:HW + AW], in_=AT[:, :, 0:AW],
                               func=AF.Exp)
                vec.tensor_tensor(
                    out=Gt[:, :, 0:HW].rearrange("p s (c a) -> p s c a", a=AW),
                    in0=Gt[:, :, 0:HW].rearrange("p s (c a) -> p s c a", a=AW),
                    in1=Gt[:, :, HW:HW + AW].unsqueeze(2)
                        .to_broadcast([P, SS, HW // AW, AW]),
                    op=ALU.mult)
                if l == 0:
                    # append eterm9|cnt as rhs cols 132:142
                    act.copy(out=Gt[:, :, 132:142],
                             in_=etc[:, ss * SS:(ss + 1) * SS, :])
                # scatter matmuls
                for c_i in range(SS):
                    gc = ss * SS + c_i
                    b = bin_of_chunk[gc]
                    w_ = win_of_bin[b]
                    g_ = grp_of_win[w_]
                    if g_ not in grp_tiles:
                        open_group(g_)
                    if gc == first_chunk_of_bin[b]:
                        cur_bin_tile[b] = bp.tile([BIN, 142], F32, name="binacc",
                                                  tag="binacc")
                    pe.matmul(out=cur_bin_tile[b][:, 0:RW],
                              lhsT=pt_all[:, gc, :], rhs=Gt[:, c_i, 0:RW],
                              start=(gc == first_chunk_of_bin[b]),
                              stop=(gc == last_chunk_of_bin[b]))
                    if gc == last_chunk_of_bin[b]:
                        j = b % BPW
                        wrel = w_ - grp_bounds[g_][0]
                        act.copy(out=grp_tiles[g_][BIN * j:BIN * (j + 1), wrel, 0:RW],
                                 in_=cur_bin_tile[b][:, 0:RW])
                        del cur_bin_tile[b]
                    if gc == last_chunk_of_grp.get(g_, None):
                        epilogue_group(g_)
            # groups never triggered (e.g. all-empty windows)
            for g_ in range(NG):
                if g_ not in grp_done:
                    if g_ not in grp_tiles:
                        open_group(g_)
                    epilogue_group(g_)
            if l < 3:
                if SIM1:
                    gps.dma_start(out=T_glob[l + 1][0:NPC, :], in_=T_loc[l + 1][:, :])
                else:
                    gps.collective_compute(
                        "AllGather", ALU.bypass, replica_groups=[list(range(NCORES))],
                        ins=[T_loc[l + 1][:, :]], outs=[T_glob[l + 1][:, :]])
                T_sb = T_next

        # ============ readout
        gsum_sb = cp.tile([33, Gn], F32)
        act.copy(out=gsum_sb[:], in_=gsum_ps[:])
        gps.dma_start(out=ar_in[:], in_=gsum_sb[:])
        if SIM1:
            gps.dma_start(out=ar_out[:], in_=ar_in[:])
        else:
            gps.collective_compute("AllReduce", ALU.add,
                                   replica_groups=[list(range(NCORES))],
                                   ins=[ar_in[:]], outs=[ar_out[:]])
        gs = cp.tile([33, Gn], F32)
        sync.dma_start(out=gs[:], in_=ar_out[:])
        # broadcast cnt row to 32 partitions via ones-matmul (no DRAM roundtrip)
        cnt1 = cp.tile([1, Gn], F32)
        sync.dma_start(out=cnt1[:], in_=gs[32:33, :])
        cps = pp.tile([32, Gn], F32, tag="hps")
        pe.matmul(out=cps[:], lhsT=onesf[:], rhs=cnt1[:], start=True, stop=True)
        cntb = cp.tile([32, Gn], F32)
        vec.tensor_scalar_max(out=cntb[:], in0=cps[:], scalar1=1.0)
        vec.reciprocal(out=cntb[:], in_=cntb[:])
        vec.tensor_tensor(out=comb[0:32, :], in0=gs[0:32, :], in1=cntb[:],
                          op=ALU.mult)
        fin = pp.tile([1, Gn], F32, tag="hps")
        pe.matmul(out=fin[:], lhsT=wlin_sb[:], rhs=comb[:], start=True, stop=True)
        res_sb = cp.tile([1, Gn], F32)
        vec.tensor_scalar_add(out=res_sb[:], in0=fin[:], scalar1=bl)
        act.activation(out=res_sb[:], in_=res_sb[:], func=AF.Sigmoid)
        sync.dma_start(out=out_p[:, :], in_=res_sb[:])

    nc.finalize()
    return nc


# ------------------------------------------------------------------ entry
def _run(inputs, trace=False, debug=False):
    dims, shared, per_core = host_prep(inputs)
    nc = build_program(dims, shared)
    in_maps = [{**shared, **pc} for pc in per_core]
    from concourse.bass_utils import run_bass_kernel_spmd
    return run_bass_kernel_spmd(nc, in_maps, list(range(NCORES)), trace=trace)


def kernel(**inputs):
    res = _run(inputs)
    return res.results[0]["out"].reshape(-1).astype(np.float32)


# revision 13
# speedup vs baseline: 1.3007x; 1.0739x over previous
"""EnhancedGAT Trainium2 Bass kernel (8 NeuronCores, SPMD).

Strategy:
  - Edges are sorted by destination node on the host; core k owns dst nodes
    [k*N/8, (k+1)*N/8) and every edge targeting them. Per-core edge lists are
    bucketed into 64-node bins and padded to 128-edge chunks with a per-bin
    chunk count shared across cores (SPMD uniformity). Dummy (padding) edges
    carry dst-offset 64, which falls outside the 64-wide one-hot used by the
    scatter matmuls, so they contribute exactly nothing.
  - Each GAT layer:
      node phase: every core computes a table row [h(128) | b(4)] for its own
        nodes, where b = per-head <h, att_s + att_d> comes directly out of the
        h matmul via 4 extra weight columns W @ A. Rows live in a [NPC, 256]
        bf16 DRAM table (512B stride for the gather); an AllGather replicates
        it to every core.
      edge phase: per 4096-edge superstep one dma_gather pulls the rows for
        the edges' sources; attention coefficients alpha = b[src] (+ edge
        term) are leaky-relu'd and exp'd in place, messages h*ex are scattered
        into per-bin PSUM accumulators via one-hot matmuls. Softmax is
        unnormalized (max-subtraction skipped; alphas are O(0.3)); the divide
        happens per node at the group epilogue, where self-loop contributions
        are added. As soon as a window-group's epilogue finishes, the NEXT
        layer's node phase for those windows runs (transpose + matmul + table
        write), hiding the layer boundary behind the remaining gathers.
  - Layer 1 additionally computes, per edge, the folded edge-attention terms
    for layers 2-4 (eterm = ea @ V + be, with the padding mask folded in as a
    fifth all-ones/zeros EAT row) plus the per-edge mask into an [C,10] SBUF
    cache, and accumulates per-node mean edge-feature terms and in-degrees
    (extra scatter-matmul columns) used by the self-loops of layers 2-4.
  - Final graph mean-pool via one-hot matmuls into a [33, G] accumulator,
    AllReduce across cores, tiny dense readout replicated on every core.
"""
import sys
import numpy as np

sys.path.insert(0, "/opt/trn_rl_repo")

HID = 32
NCORES = 8
P = 128
BIN = 64
SS = 32          # chunks per superstep
CHUNK = 128
ROW = 256        # table row elements (bf16) for layers 1-3 (512B stride)
ROW4 = 128       # layer-4 table row elements


# ----------------------------------------------------------------- host prep
def host_prep(inputs):
    x = np.asarray(inputs["x"], np.float32)
    ei = np.asarray(inputs["edge_index"]).astype(np.int64)
    ea = np.asarray(inputs["edge_attr"], np.float32)
    batch = np.asarray(inputs["batch"]).astype(np.int64)
    desc = np.asarray(inputs["descriptors"], np.float32)

    N = x.shape[0]
    E = ei.shape[1]
    Gn = desc.shape[0]
    NPC = N // NCORES
    NW = -(-NPC // P)
    NBINS = -(-NPC // BIN)

    src_all, dst_all = ei[0], ei[1]
    order = np.argsort(dst_all, kind="stable")
    src_s, dst_s = src_all[order], dst_all[order]
    ea_s = ea[order]
    core_of = dst_s // NPC
    local = dst_s - core_of * NPC
    bin_of = local // BIN

    cnt = np.zeros((NCORES, NBINS), np.int64)
    np.add.at(cnt, (core_of, bin_of), 1)
    cpb = np.max(-(-cnt // CHUNK), axis=0)          # chunks per bin (shared)
    cpb = np.maximum(cpb, 1)                        # every bin gets a chunk
    C_total = int(cpb.sum())
    padc = (-C_total) % SS
    if padc:
        cpb[-1] += padc
        C_total += padc
    off = np.zeros(NBINS, np.int64)
    off[1:] = np.cumsum(cpb)[:-1]
    EP = C_total * CHUNK                            # padded edges per core

    per_core = []
    for k in range(NCORES):
        srck = np.zeros(EP, np.int64)
        dstrk = np.full(EP, float(BIN), np.float32)  # dummies -> dead one-hot
        maskk = np.zeros(EP, np.float32)
        eak = np.zeros((EP, 4), np.float32)
        sel = core_of == k
        bins_k = bin_of[sel]
        start = np.searchsorted(bins_k, np.arange(NBINS))
        pos = np.arange(bins_k.size) - start[bins_k]
        slot = off[bins_k] * CHUNK + pos
        srck[slot] = src_s[sel]
        dstrk[slot] = (local[sel] - bins_k * BIN).astype(np.float32)
        maskk[slot] = 1.0
        eak[slot] = ea_s[sel]

        # device layouts: edge e = c*128 + p
        src16 = np.tile(srck.reshape(-1, 16).T.astype(np.int16), (8, 1))
        dstr_d = dstrk.reshape(C_total, P).T.copy()
        import ml_dtypes
        ea5 = np.concatenate([eak.T, maskk[None, :]], axis=0).astype(ml_dtypes.bfloat16)

        xk = x[k * NPC:(k + 1) * NPC]
        xT = np.zeros((8, NW * P), np.float32)
        xT[:, :NPC] = xk.T
        bk = np.full(NW * P, Gn + 5, np.float32)
        bk[:NPC] = batch[k * NPC:(k + 1) * NPC].astype(np.float32)
        batch_d = bk.reshape(NW, P).T.copy()

        per_core.append(dict(SRC16=src16, DSTR=dstr_d, EAT=ea5,
                             XT=xT, BATCH=batch_d))

    # ---- weight folding
    w = {k: np.asarray(v, np.float32) for k, v in inputs.items()
         if k not in ("x", "edge_index", "edge_attr", "batch", "descriptors")}

    def vfold(We, ae, heads):
        Vp = (We.reshape(w["We_enc"].shape[1], heads, HID) * ae[None]).sum(-1)
        return w["We_enc"] @ Vp, w["be_enc"] @ Vp      # [4,heads],[heads]

    V2, bv2 = vfold(w["We2"], w["ae2"], 4)
    V3, bv3 = vfold(w["We3"], w["ae3"], 4)
    V4, bv4 = vfold(w["We4"], w["ae4"], 1)
    # [5,10]: rows = 4 edge-attr dims + mask; cols = 9 eterms + cnt
    W5x10 = np.zeros((5, 10), np.float32)
    W5x10[0:4, 0:9] = np.concatenate([V2, V3, V4], axis=1)
    W5x10[4, 0:9] = np.concatenate([bv2, bv3, bv4])
    W5x10[4, 9] = 1.0

    def padr(v, n):
        o = np.zeros(n, np.float32)
        o[: v.size] = v
        return o

    # channel-major reorder of the 128-wide (4 heads x 32 ch) dimension:
    # new position c*4+a holds old a*32+c. Keeps per-head broadcasts
    # innermost-packed on DVE (2x mode).
    cm = (np.arange(128) % 4) * 32 + np.arange(128) // 4

    def wext(W, att_s, att_d, heads):
        # append per-head b-columns: b_a = h . (att_s+att_d)_a
        att = (att_s + att_d).reshape(-1)  # [heads*HID] head-major
        if heads == 4:
            attc = att[cm]                 # channel-major to match W cols
            A = np.zeros((128, 4), np.float32)
            A[np.arange(128), np.arange(128) % 4] = attc
        else:
            A = att[:, None]               # [32,1]
        return np.concatenate([W, W @ A], axis=1)

    W1e = wext(w["W1"][:, cm], w["as1"], w["ad1"], 4)            # [8,132]
    W2e = wext(w["W2"][cm][:, cm], w["as2"], w["ad2"], 4)        # [128,132]
    W3e = wext(w["W3"][cm][:, cm], w["as3"], w["ad3"], 4)
    W4e = wext(w["W4"][cm], w["as4"], w["ad4"], 1)               # [128,33]

    bout = np.stack([padr(w["b1"][cm], 128), padr(w["b2"][cm], 128),
                     padr(w["b3"][cm], 128), padr(w["b4"], 128)])

    gcnt = np.bincount(batch, minlength=Gn).astype(np.float32)
    cntr = (1.0 / np.maximum(gcnt, 1.0))[None, :]           # [1, Gn]
    shared = dict(
        W1=W1e, WL2=W2e, WL3=W3e, WL4=W4e,
        W5X10=W5x10, BOUT=bout, CNTR=cntr,
        WD=w["Wd"], BD=w["bd"][:, None], WLIN=w["Wl"], DESCT=desc.T.copy(),
    )
    bl = float(np.asarray(w["bl"]).reshape(-1)[0])

    dims = dict(N=N, E=E, Gn=Gn, NPC=NPC, NW=NW, NBINS=NBINS,
                C=C_total, cpb=cpb, off=off, bl=bl)
    return dims, shared, per_core


# ------------------------------------------------------------- program build
def build_program(dims, shared):
    import concourse.bass as bass
    import concourse.mybir as mybir
    import concourse.tile as tile
    import concourse.bacc as bacc
    from concourse.masks import make_identity
    from contextlib import ExitStack

    F32 = mybir.dt.float32
    BF16 = mybir.dt.bfloat16
    I32 = mybir.dt.int32
    I16 = mybir.dt.int16
    AF = mybir.ActivationFunctionType
    ALU = mybir.AluOpType
    AX = mybir.AxisListType

    N, Gn, NPC, NW, NBINS, C = (dims[k] for k in ("N", "Gn", "NPC", "NW", "NBINS", "C"))
    cpb, off, bl = dims["cpb"], dims["off"], dims["bl"]
    NSS = C // SS
    # layer params: h width, heads, rhs width, gather row elems
    LP = [dict(HW=128, AW=4, RW=142, EL=ROW),   # L1 rhs: h,ex,eterm9,cnt
          dict(HW=128, AW=4, RW=132, EL=ROW),
          dict(HW=128, AW=4, RW=132, EL=ROW),
          dict(HW=32, AW=1, RW=33, EL=ROW4)]

    nc = bacc.Bacc(num_swdge_queues=2)
    SIM1 = dims.get("sim1", False)

    # ---- params
    pr = {}
    for nm, shp, dt in [("SRC16", [P, C * 8], I16), ("DSTR", [P, C], F32),
                        ("EAT", [5, C * CHUNK], BF16), ("XT", [8, NW * P], F32),
                        ("BATCH", [P, NW], F32), ("W1", [8, 132], F32),
                        ("WL2", [128, 132], F32), ("WL3", [128, 132], F32),
                        ("WL4", [128, 33], F32), ("W5X10", [5, 10], F32),
                        ("BOUT", [4, 128], F32),
                        ("WD", [48, 32], F32), ("BD", [32, 1], F32),
                        ("WLIN", [64, 1], F32), ("DESCT", [48, Gn], F32),
                        ("CNTR", [1, Gn], F32)]:
        pr[nm] = nc.declare_dram_parameter(nm, shp, dt, isOutput=False)
    out_p = nc.declare_dram_parameter("out", [1, Gn], F32, isOutput=True)

    # ---- internal DRAM
    T_loc = [nc.dram_tensor(f"T_loc{l}", [NPC, LP[l]["EL"]], BF16) for l in range(4)]
    T_glob = [nc.dram_tensor(f"T_glob{l}", [N, LP[l]["EL"]], BF16, addr_space="Shared")
              for l in range(4)]
    ar_in = nc.dram_tensor("ar_in", [32, Gn], F32)
    ar_out = nc.dram_tensor("ar_out", [32, Gn], F32, addr_space="Shared")

    # bin/window bookkeeping (compile-time)
    bin_of_chunk = []
    for b in range(NBINS):
        bin_of_chunk += [b] * int(cpb[b])
    BPW = P // BIN  # bins per window
    win_of_bin = [b // BPW for b in range(NBINS)]
    last_chunk_of_bin = {}
    first_chunk_of_bin = {}
    for c_i, b in enumerate(bin_of_chunk):
        last_chunk_of_bin[b] = c_i
        first_chunk_of_bin.setdefault(b, c_i)

    with tile.TileContext(nc) as tc, ExitStack() as ctx:
        cp = ctx.enter_context(tc.tile_pool(name="const", bufs=1))
        wp = ctx.enter_context(tc.tile_pool(name="work", bufs=2))
        vp = ctx.enter_context(tc.tile_pool(name="win", bufs=3))
        pp = ctx.enter_context(tc.tile_pool(name="psum", bufs=2, space="PSUM"))
        bp = ctx.enter_context(tc.tile_pool(name="binp", bufs=2, space="PSUM"))

        sync, gps, vec, act, pe = nc.sync, nc.gpsimd, nc.vector, nc.scalar, nc.tensor

        # ---- resident tiles
        src16 = cp.tile([P, C * 8], I16)
        sync.dma_start(out=src16[:], in_=pr["SRC16"][:, :])
        dstr = cp.tile([P, C], BF16)
        gps.dma_start(out=dstr[:], in_=pr["DSTR"][:, :])   # f32 -> bf16 cast
        batcht = cp.tile([P, NW], F32)
        sync.dma_start(out=batcht[:], in_=pr["BATCH"][:, :])
        xT_sb = cp.tile([8, NW * P], F32)
        sync.dma_start(out=xT_sb[:], in_=pr["XT"][:, :])

        iota_i = cp.tile([P, BIN], I32)
        gps.iota(iota_i[:], pattern=[[1, BIN]], base=0, channel_multiplier=0)
        iotab = cp.tile([P, BIN], BF16)
        vec.tensor_copy(iotab[:], iota_i[:])
        iotag_i = cp.tile([P, Gn], I32)
        gps.iota(iotag_i[:], pattern=[[1, Gn]], base=0, channel_multiplier=0)
        iotagf = cp.tile([P, Gn], F32)
        vec.tensor_copy(iotagf[:], iotag_i[:])
        identb = cp.tile([P, P], BF16)
        make_identity(nc, identb[:])

        w1_sb = cp.tile([8, 132], F32)
        sync.dma_start(out=w1_sb[:], in_=pr["W1"][:, :])
        wl_sb = [None,
                 cp.tile([128, 132], BF16, name="wl2", tag="wl2"),
                 cp.tile([128, 132], BF16, name="wl3", tag="wl3"),
                 cp.tile([128, 33], BF16, name="wl4", tag="wl4")]
        gps.dma_start(out=wl_sb[1][:], in_=pr["WL2"][:, :])   # gpsimd casts f32->bf16
        gps.dma_start(out=wl_sb[2][:], in_=pr["WL3"][:, :])
        gps.dma_start(out=wl_sb[3][:], in_=pr["WL4"][:, :])
        w5x10 = cp.tile([5, 10], BF16)
        gps.dma_start(out=w5x10[:], in_=pr["W5X10"][:, :])
        bout_t = []
        for l in range(4):
            t3 = cp.tile([P, 128], F32, tag=f"bout{l}")
            sync.dma_start(out=t3[:], in_=pr["BOUT"][l:l + 1, :].to_broadcast([P, 128]))
            bout_t.append(t3)

        etc = cp.tile([P, C, 10], BF16)      # eterm9 | cnt  per edge
        pt_all = cp.tile([P, C, BIN], BF16)  # one-hot dst rows per edge
        loop_sb = cp.tile([P, NW, 10], F32)
        gsp = ctx.enter_context(tc.tile_pool(name="gsp", bufs=1, space="PSUM"))
        eap = ctx.enter_context(tc.tile_pool(name="eap", bufs=1))
        gsum_ps = None
        n_pool_mm = [0]

        # ---- readout head start: descriptor branch is input-independent
        comb = cp.tile([64, Gn], F32)
        wd_sb = cp.tile([48, 32], F32)
        sync.dma_start(out=wd_sb[:], in_=pr["WD"][:, :])
        desct_sb = cp.tile([48, Gn], F32)
        sync.dma_start(out=desct_sb[:], in_=pr["DESCT"][:, :])
        bd_sb = cp.tile([32, 1], F32)
        sync.dma_start(out=bd_sb[:], in_=pr["BD"][:, :])
        dps = pp.tile([32, Gn], F32, tag="hps", bufs=3)
        pe.matmul(out=dps[:], lhsT=wd_sb[:], rhs=desct_sb[:], start=True, stop=True)
        act.activation(out=comb[32:64, :], in_=dps[:], func=AF.Relu, bias=bd_sb[:])
        wlin_sb = cp.tile([64, 1], F32)
        sync.dma_start(out=wlin_sb[:], in_=pr["WLIN"][:, :])
        cntrb = cp.tile([32, Gn], F32)
        sync.dma_start(out=cntrb[:], in_=pr["CNTR"][0:1, :].to_broadcast([32, Gn]))

        WG = 5  # max windows per epilogue group
        # non-uniform groups: taper toward the end so the serial layer-boundary
        # tail (last epilogue -> node phase -> AllGather) shrinks
        grp_bounds = []
        w0_ = 0
        while NW - w0_ > 10:
            grp_bounds.append((w0_, WG))
            w0_ += WG
        for t_ in (4, 3, 2, 1):
            if NW - w0_ > t_:
                grp_bounds.append((w0_, t_))
                w0_ += t_
        if NW > w0_:
            grp_bounds.append((w0_, NW - w0_))
        NG = len(grp_bounds)
        grp_of_win = {}
        for gi, (gw0, gsz_) in enumerate(grp_bounds):
            for w_ in range(gw0, gw0 + gsz_):
                grp_of_win[w_] = gi
        last_chunk_of_grp = {}
        for b in range(NBINS):
            g_ = grp_of_win[win_of_bin[b]]
            last_chunk_of_grp[g_] = max(last_chunk_of_grp.get(g_, -1),
                                        last_chunk_of_bin[b])

        # T_sb tables: [P, NW, 132] (h | b); layer l+1's is built during
        # layer l's edge phase, group by group.
        def node_phase_group(l, g_, T_next, z_src):
            """Build T_next rows for group g_ of layer l (0-based), write T_loc."""
            w0, gsz = grp_bounds[g_]
            HWn = LP[l]["HW"]
            BW = HWn + LP[l]["AW"]  # table row width
            for w_ in range(w0, w0 + gsz):
                if l == 0:
                    hps = pp.tile([P, 132], F32, tag="hps", bufs=3)
                    pe.matmul(out=hps[:, 0:BW], lhsT=xT_sb[:, w_ * P:(w_ + 1) * P],
                              rhs=w1_sb[:], start=True, stop=True)
                else:
                    ztp = pp.tile([P, P], BF16, tag="ztp", bufs=1)
                    pe.transpose(out=ztp[:], in_=z_src[:, w_, :], identity=identb[:])
                    zt_sb = wp.tile([P, P], BF16, tag="ztsb")
                    act.copy(out=zt_sb[:], in_=ztp[:])
                    hps = pp.tile([P, 132], F32, tag="hps", bufs=3)
                    pe.matmul(out=hps[:, 0:BW], lhsT=zt_sb[:], rhs=wl_sb[l][:],
                              start=True, stop=True)
                act.copy(out=T_next[:, w_, 0:BW], in_=hps[:, 0:BW])
            # batched table write: full windows in one DMA, ragged tail apart
            wfull = gsz - (1 if (w0 + gsz) * P > NPC else 0)
            for w_ in range(w0, w0 + gsz):           # BISECT: per-window
                nr = min(P, NPC - w_ * P)
                sync.dma_start(out=T_loc[l][w_ * P:w_ * P + nr, 0:BW],
                               in_=T_next[0:nr, w_, 0:BW])

        # ---- layer 0 node phase (all groups up front)
        T_sb = wp.tile([P, NW, 132], BF16, tag="tsb")
        for g_ in range(NG):
            node_phase_group(0, g_, T_sb, None)
        if SIM1:
            gps.dma_start(out=T_glob[0][0:NPC, :], in_=T_loc[0][:, :])
        else:
            gps.collective_compute(
                "AllGather", ALU.bypass, replica_groups=[list(range(NCORES))],
                ins=[T_loc[0][:, :]], outs=[T_glob[0][:, :]])

        for l in range(4):
            HW, AW, RW, EL = (LP[l][k] for k in ("HW", "AW", "RW", "EL"))
            BW = HW + AW

            T_next = None
            if l < 3:
                T_next = wp.tile([P, NW, 132], BF16, name="tnext", tag="tsb")
                z_next = wp.tile([P, NW, 128], BF16, name="znext", tag="zsb")

            grp_tiles = {}
            grp_done = set()

            def open_group(g_):
                t = vp.tile([P, WG, 142], F32, name="wingrp", tag="wingrp")
                grp_tiles[g_] = t
                return t

            def epilogue_group(g_):
                w0, gsz = grp_bounds[g_]
                wg = grp_tiles[g_]
                scr = wp.tile([P, WG, 12], F32, name="scr", tag="scr")
                # self-loop alpha (= b_own [+ eterm means]) -> exp
                if l > 0:
                    sl = [None, (0, 4), (4, 8), (8, 9)][l]
                    vec.tensor_tensor(out=scr[:, 0:gsz, 0:AW],
                                      in0=T_sb[:, w0:w0 + gsz, HW:HW + AW],
                                      in1=loop_sb[:, w0:w0 + gsz, sl[0]:sl[1]],
                                      op=ALU.add)
                else:
                    act.copy(out=scr[:, 0:gsz, 0:AW],
                             in_=T_sb[:, w0:w0 + gsz, HW:HW + AW])
                vec.tensor_scalar_mul(out=scr[:, 0:gsz, 4:4 + AW],
                                      in0=scr[:, 0:gsz, 0:AW], scalar1=0.2)
                vec.tensor_tensor(out=scr[:, 0:gsz, 0:AW], in0=scr[:, 0:gsz, 0:AW],
                                  in1=scr[:, 0:gsz, 4:4 + AW], op=ALU.max)
                act.activation(out=scr[:, 0:gsz, 0:AW], in_=scr[:, 0:gsz, 0:AW],
                               func=AF.Exp)
                # num += h_own * ex_loop
                nt = wp.tile([P, WG, 128], F32, name="nt", tag="nt")
                vec.tensor_tensor(
                    out=nt[:, 0:gsz, 0:HW].rearrange("p g (c a) -> p g c a", a=AW),
                    in0=T_sb[:, w0:w0 + gsz, 0:HW].rearrange("p g (c a) -> p g c a", a=AW),
                    in1=scr[:, 0:gsz, 0:AW].unsqueeze(2)
                        .to_broadcast([P, gsz, HW // AW, AW]),
                    op=ALU.mult)
                vec.tensor_tensor(out=wg[:, 0:gsz, 0:HW], in0=wg[:, 0:gsz, 0:HW],
                                  in1=nt[:, 0:gsz, 0:HW], op=ALU.add)
                # den -> reciprocal
                vec.tensor_tensor(out=scr[:, 0:gsz, 4:4 + AW],
                                  in0=wg[:, 0:gsz, HW:HW + AW],
                                  in1=scr[:, 0:gsz, 0:AW], op=ALU.add)
                vec.tensor_scalar_add(out=scr[:, 0:gsz, 4:4 + AW],
                                      in0=scr[:, 0:gsz, 4:4 + AW], scalar1=1e-16)
                vec.reciprocal(out=scr[:, 0:gsz, 4:4 + AW], in_=scr[:, 0:gsz, 4:4 + AW])
                if l == 0:
                    vec.tensor_scalar_max(out=scr[:, 0:gsz, 8:9],
                                          in0=wg[:, 0:gsz, 141:142], scalar1=1.0)
                    vec.reciprocal(out=scr[:, 0:gsz, 8:9], in_=scr[:, 0:gsz, 8:9])
                    vec.tensor_tensor(
                        out=loop_sb[:, w0:w0 + gsz, 0:9], in0=wg[:, 0:gsz, 132:141],
                        in1=scr[:, 0:gsz, 8:9].to_broadcast([P, gsz, 9]), op=ALU.mult)
                # z = num * recip(den) + bias [+ relu]
                vec.tensor_tensor(
                    out=wg[:, 0:gsz, 0:HW].rearrange("p g (c a) -> p g c a", a=AW),
                    in0=wg[:, 0:gsz, 0:HW].rearrange("p g (c a) -> p g c a", a=AW),
                    in1=scr[:, 0:gsz, 4:4 + AW].unsqueeze(2)
                        .to_broadcast([P, gsz, HW // AW, AW]),
                    op=ALU.mult)
                vec.tensor_tensor(
                    out=wg[:, 0:gsz, 0:HW], in0=wg[:, 0:gsz, 0:HW],
                    in1=bout_t[l][:, 0:HW].unsqueeze(1).to_broadcast([P, gsz, HW]),
                    op=ALU.add)
                if l < 3:
                    act.activation(out=z_next[:, w0:w0 + gsz, :], in_=wg[:, 0:gsz, 0:128],
                                   func=AF.Relu)
                    node_phase_group(l + 1, g_, T_next, z_next)
                else:
                    nonlocal gsum_ps
                    pool_sb = wp.tile([P, WG, 32], BF16, name="pool_sb", tag="poolsb")
                    act.copy(out=pool_sb[:, 0:gsz, 0:32], in_=wg[:, 0:gsz, 0:32])
                    bt = wp.tile([P, WG, Gn], BF16, name="bt", tag="bt")
                    vec.tensor_tensor(
                        out=bt[:, 0:gsz, :],
                        in0=batcht[:, w0:w0 + gsz].unsqueeze(2).to_broadcast([P, gsz, Gn]),
                        in1=iotagf[:].unsqueeze(1).to_broadcast([P, gsz, Gn]),
                        op=ALU.is_equal)
                    if gsum_ps is None:
                        gsum_ps = gsp.tile([32, Gn], F32, name="gsum_ps")
                    for j_ in range(gsz):
                        n_pool_mm[0] += 1
                        pe.matmul(out=gsum_ps[:], lhsT=pool_sb[:, j_, :],
                                  rhs=bt[:, j_, :],
                                  start=(n_pool_mm[0] == 1),
                                  stop=(n_pool_mm[0] == NW))
                grp_done.add(g_)

            cur_bin_tile = {}
            for ss in range(NSS):
                Gt = wp.tile([P, SS, EL], BF16, tag="gt", bufs=3)
                gps.dma_gather(
                    out_ap=Gt[:, :, :], in_ap=T_glob[l][:, :],
                    idxs_ap=src16[:, ss * SS * 8:(ss + 1) * SS * 8],
                    num_idxs=SS * CHUNK, num_idxs_reg=SS * CHUNK, elem_size=EL,
                    single_packet=False, queue_num=ss % 2)
                if l == 0:
                    # edge-term + mask precompute (feeds rhs cols 132:142 +
                    # later layers' alpha); mask folded into EAT row 5.
                    eaT_sl = eap.tile([5, SS * CHUNK], BF16, name="easl", tag="eat")
                    sync.dma_start(
                        out=eaT_sl[:],
                        in_=pr["EAT"][:, ss * SS * CHUNK:(ss + 1) * SS * CHUNK])
                    for q in range(SS // 16):
                        etp = pp.tile([P, 160], F32, tag="etp", bufs=1)
                        for j in range(16):
                            ci = q * 16 + j
                            pe.matmul(out=etp[:, j * 10:(j + 1) * 10],
                                      lhsT=eaT_sl[:, ci * CHUNK:(ci + 1) * CHUNK],
                                      rhs=w5x10[:], start=True, stop=True)
                        act.copy(out=etc[:, ss * SS + q * 16:ss * SS + q * 16 + 16, :]
                                 .rearrange("p a b -> p (a b)"),
                                 in_=etp[:])
                    # staircase one-hots built once, reused by all layers
                    # (split across DVE and GpSimd to keep both under the
                    # gather roofline)
                    for g in range(SS // 8):
                        s0 = ss * SS + g * 8
                        eng = vec
                        eng.tensor_tensor(
                            out=pt_all[:, s0:s0 + 8, :],
                            in0=dstr[:, s0:s0 + 8].unsqueeze(2).to_broadcast([P, 8, BIN]),
                            in1=iotab[:].unsqueeze(1).to_broadcast([P, 8, BIN]),
                            op=ALU.is_equal)
                # alpha = b[src] (+ eterm) -> leaky relu -> exp
                AT = wp.tile([P, SS, 8], BF16, tag="at", bufs=2)
                if l > 0:
                    sl = [None, (0, 4), (4, 8), (8, 9)][l]
                    vec.tensor_tensor(out=AT[:, :, 0:AW], in0=Gt[:, :, HW:HW + AW],
                                      in1=etc[:, ss * SS:(ss + 1) * SS, sl[0]:sl[1]],
                                      op=ALU.add)
                    vec.tensor_scalar_mul(out=AT[:, :, AW:2 * AW], in0=AT[:, :, 0:AW],
                                          scalar1=0.2)
                    vec.tensor_tensor(out=AT[:, :, 0:AW], in0=AT[:, :, 0:AW],
                                      in1=AT[:, :, AW:2 * AW], op=ALU.max)
                else:
                    vec.tensor_scalar_mul(out=AT[:, :, AW:2 * AW],
                                          in0=Gt[:, :, HW:HW + AW], scalar1=0.2)
                    vec.tensor_tensor(out=AT[:, :, 0:AW], in0=Gt[:, :, HW:HW + AW],
                                      in1=AT[:, :, AW:2 * AW], op=ALU.max)
                act.activation(out=Gt[:, :, HW:HW + AW], in_=AT[:, :, 0:AW],
                               func=AF.Exp)
                vec.tensor_tensor(
                    out=Gt[:, :, 0:HW].rearrange("p s (c a) -> p s c a", a=AW),
                    in0=Gt[:, :, 0:HW].rearrange("p s (c a) -> p s c a", a=AW),
                    in1=Gt[:, :, HW:HW + AW].unsqueeze(2)
                        .to_broadcast([P, SS, HW // AW, AW]),
                    op=ALU.mult)
                if l == 0:
                    # append eterm9|cnt as rhs cols 132:142
                    act.copy(out=Gt[:, :, 132:142],
                             in_=etc[:, ss * SS:(ss + 1) * SS, :])
                # scatter matmuls
                for c_i in range(SS):
                    gc = ss * SS + c_i
                    b = bin_of_chunk[gc]
                    w_ = win_of_bin[b]
                    g_ = grp_of_win[w_]
                    if g_ not in grp_tiles:
                        open_group(g_)
                    if gc == first_chunk_of_bin[b]:
                        cur_bin_tile[b] = bp.tile([BIN, 142], F32, name="binacc",
                                                  tag="binacc")
                    pe.matmul(out=cur_bin_tile[b][:, 0:RW],
                              lhsT=pt_all[:, gc, :], rhs=Gt[:, c_i, 0:RW],
                              start=(gc == first_chunk_of_bin[b]),
                              stop=(gc == last_chunk_of_bin[b]))
                    if gc == last_chunk_of_bin[b]:
                        j = b % BPW
                        wrel = w_ - grp_bounds[g_][0]
                        act.copy(out=grp_tiles[g_][BIN * j:BIN * (j + 1), wrel, 0:RW],
                                 in_=cur_bin_tile[b][:, 0:RW])
                        del cur_bin_tile[b]
                    if gc == last_chunk_of_grp.get(g_, None):
                        epilogue_group(g_)
            # groups never triggered (e.g. all-empty windows)
            for g_ in range(NG):
                if g_ not in grp_done:
                    if g_ not in grp_tiles:
                        open_group(g_)
                    epilogue_group(g_)
            if l < 3:
                if SIM1:
                    gps.dma_start(out=T_glob[l + 1][0:NPC, :], in_=T_loc[l + 1][:, :])
                else:
                    gps.collective_compute(
                        "AllGather", ALU.bypass, replica_groups=[list(range(NCORES))],
                        ins=[T_loc[l + 1][:, :]], outs=[T_glob[l + 1][:, :]])
                T_sb = T_next

        # ============ readout
        gsum_sb = cp.tile([32, Gn], F32)
        act.copy(out=gsum_sb[:], in_=gsum_ps[:])
        sync.dma_start(out=ar_in[:], in_=gsum_sb[:])
        if SIM1:
            gps.dma_start(out=ar_out[:], in_=ar_in[:])
        else:
            gps.collective_compute("AllReduce", ALU.add,
                                   replica_groups=[list(range(NCORES))],
                                   ins=[ar_in[:]], outs=[ar_out[:]])
        gs = cp.tile([32, Gn], F32)
        sync.dma_start(out=gs[:], in_=ar_out[:])
        vec.tensor_tensor(out=comb[0:32, :], in0=gs[:, :], in1=cntrb[:],
                          op=ALU.mult)
        fin = pp.tile([1, Gn], F32, tag="hps", bufs=3)
        pe.matmul(out=fin[:], lhsT=wlin_sb[:], rhs=comb[:], start=True, stop=True)
        res_sb = cp.tile([1, Gn], F32)
        vec.tensor_scalar_add(out=res_sb[:], in0=fin[:], scalar1=bl)
        act.activation(out=res_sb[:], in_=res_sb[:], func=AF.Sigmoid)
        sync.dma_start(out=out_p[:, :], in_=res_sb[:])

    nc.finalize()
    return nc


# ------------------------------------------------------------------ entry
def _run(inputs, trace=False, debug=False):
    dims, shared, per_core = host_prep(inputs)
    nc = build_program(dims, shared)
    in_maps = [{**shared, **pc} for pc in per_core]
    from concourse.bass_utils import run_bass_kernel_spmd
    return run_bass_kernel_spmd(nc, in_maps, list(range(NCORES)), trace=trace)


def kernel(**inputs):
    res = _run(inputs)
    return res.results[0]["out"].reshape(-1).astype(np.float32)


# revision 20
# speedup vs baseline: 1.3614x; 1.0467x over previous
"""EnhancedGAT Trainium2 Bass kernel (8 NeuronCores, SPMD).

Strategy:
  - Edges are sorted by destination node on the host; core k owns dst nodes
    [k*N/8, (k+1)*N/8) and every edge targeting them. Per-core edge lists are
    bucketed into 64-node bins and padded to 128-edge chunks with a per-bin
    chunk count shared across cores (SPMD uniformity). Dummy (padding) edges
    carry dst-offset 64, which falls outside the 64-wide one-hot used by the
    scatter matmuls, so they contribute exactly nothing.
  - Each GAT layer:
      node phase: every core computes a table row [h(128) | b(4)] for its own
        nodes, where b = per-head <h, att_s + att_d> comes directly out of the
        h matmul via 4 extra weight columns W @ A. Rows live in a [NPC, 256]
        bf16 DRAM table (512B stride for the gather); an AllGather replicates
        it to every core.
      edge phase: per 4096-edge superstep one dma_gather pulls the rows for
        the edges' sources; attention coefficients alpha = b[src] (+ edge
        term) are leaky-relu'd and exp'd in place, messages h*ex are scattered
        into per-bin PSUM accumulators via one-hot matmuls. Softmax is
        unnormalized (max-subtraction skipped; alphas are O(0.3)); the divide
        happens per node at the group epilogue, where self-loop contributions
        are added. As soon as a window-group's epilogue finishes, the NEXT
        layer's node phase for those windows runs (transpose + matmul + table
        write), hiding the layer boundary behind the remaining gathers.
  - Layer 1 additionally computes, per edge, the folded edge-attention terms
    for layers 2-4 (eterm = ea @ V + be, with the padding mask folded in as a
    fifth all-ones/zeros EAT row) plus the per-edge mask into an [C,10] SBUF
    cache, and accumulates per-node mean edge-feature terms and in-degrees
    (extra scatter-matmul columns) used by the self-loops of layers 2-4.
  - Final graph mean-pool via one-hot matmuls into a [33, G] accumulator,
    AllReduce across cores, tiny dense readout replicated on every core.
"""
import sys
import numpy as np

sys.path.insert(0, "/opt/trn_rl_repo")

HID = 32
NCORES = 8
P = 128
BIN = 64
SS = 32          # chunks per superstep
CHUNK = 128
ROW = 256        # table row elements (bf16) for layers 1-3 (512B stride)
ROW4 = 128       # layer-4 table row elements


# ----------------------------------------------------------------- host prep
def host_prep(inputs):
    x = np.asarray(inputs["x"], np.float32)
    ei = np.asarray(inputs["edge_index"]).astype(np.int64)
    ea = np.asarray(inputs["edge_attr"], np.float32)
    batch = np.asarray(inputs["batch"]).astype(np.int64)
    desc = np.asarray(inputs["descriptors"], np.float32)

    N = x.shape[0]
    E = ei.shape[1]
    Gn = desc.shape[0]
    NPC = N // NCORES
    NW = -(-NPC // P)
    NBINS = -(-NPC // BIN)

    src_all, dst_all = ei[0], ei[1]
    order = np.argsort(dst_all, kind="stable")
    src_s, dst_s = src_all[order], dst_all[order]
    ea_s = ea[order]
    core_of = dst_s // NPC
    local = dst_s - core_of * NPC
    bin_of = local // BIN

    cnt = np.zeros((NCORES, NBINS), np.int64)
    np.add.at(cnt, (core_of, bin_of), 1)
    cpb = np.max(-(-cnt // CHUNK), axis=0)          # chunks per bin (shared)
    cpb = np.maximum(cpb, 1)                        # every bin gets a chunk
    C_total = int(cpb.sum())
    off = np.zeros(NBINS, np.int64)
    off[1:] = np.cumsum(cpb)[:-1]
    EP = C_total * CHUNK                            # padded edges per core

    per_core = []
    for k in range(NCORES):
        srck = np.zeros(EP, np.int64)
        dstrk = np.full(EP, float(BIN), np.float32)  # dummies -> dead one-hot
        maskk = np.zeros(EP, np.float32)
        eak = np.zeros((EP, 4), np.float32)
        sel = core_of == k
        bins_k = bin_of[sel]
        start = np.searchsorted(bins_k, np.arange(NBINS))
        pos = np.arange(bins_k.size) - start[bins_k]
        slot = off[bins_k] * CHUNK + pos
        srck[slot] = src_s[sel]
        dstrk[slot] = (local[sel] - bins_k * BIN).astype(np.float32)
        maskk[slot] = 1.0
        eak[slot] = ea_s[sel]

        # device layouts: edge e = c*128 + p
        src16 = np.tile(srck.reshape(-1, 16).T.astype(np.int16), (8, 1))
        dstr_d = dstrk.reshape(C_total, P).T.copy()
        import ml_dtypes
        ea5 = np.concatenate([eak.T, maskk[None, :]], axis=0).astype(ml_dtypes.bfloat16)

        xk = x[k * NPC:(k + 1) * NPC]
        xT = np.zeros((8, NW * P), np.float32)
        xT[:, :NPC] = xk.T
        bk = np.full(NW * P, Gn + 5, np.float32)
        bk[:NPC] = batch[k * NPC:(k + 1) * NPC].astype(np.float32)
        batch_d = bk.reshape(NW, P).T.copy()

        per_core.append(dict(SRC16=src16, DSTR=dstr_d, EAT=ea5,
                             XT=xT, BATCH=batch_d))

    # ---- weight folding
    w = {k: np.asarray(v, np.float32) for k, v in inputs.items()
         if k not in ("x", "edge_index", "edge_attr", "batch", "descriptors")}

    def vfold(We, ae, heads):
        Vp = (We.reshape(w["We_enc"].shape[1], heads, HID) * ae[None]).sum(-1)
        return w["We_enc"] @ Vp, w["be_enc"] @ Vp      # [4,heads],[heads]

    V2, bv2 = vfold(w["We2"], w["ae2"], 4)
    V3, bv3 = vfold(w["We3"], w["ae3"], 4)
    V4, bv4 = vfold(w["We4"], w["ae4"], 1)
    # [5,10]: rows = 4 edge-attr dims + mask; cols = 9 eterms + cnt
    W5x10 = np.zeros((5, 10), np.float32)
    W5x10[0:4, 0:9] = np.concatenate([V2, V3, V4], axis=1)
    W5x10[4, 0:9] = np.concatenate([bv2, bv3, bv4])
    W5x10[4, 9] = 1.0

    def padr(v, n):
        o = np.zeros(n, np.float32)
        o[: v.size] = v
        return o

    # channel-major reorder of the 128-wide (4 heads x 32 ch) dimension:
    # new position c*4+a holds old a*32+c. Keeps per-head broadcasts
    # innermost-packed on DVE (2x mode).
    cm = (np.arange(128) % 4) * 32 + np.arange(128) // 4

    def wext(W, att_s, att_d, heads):
        # append per-head b-columns: b_a = h . (att_s+att_d)_a
        att = (att_s + att_d).reshape(-1)  # [heads*HID] head-major
        if heads == 4:
            attc = att[cm]                 # channel-major to match W cols
            A = np.zeros((128, 4), np.float32)
            A[np.arange(128), np.arange(128) % 4] = attc
        else:
            A = att[:, None]               # [32,1]
        return np.concatenate([W, W @ A], axis=1)

    W1e = wext(w["W1"][:, cm], w["as1"], w["ad1"], 4)            # [8,132]
    W2e = wext(w["W2"][cm][:, cm], w["as2"], w["ad2"], 4)        # [128,132]
    W3e = wext(w["W3"][cm][:, cm], w["as3"], w["ad3"], 4)
    W4e = wext(w["W4"][cm], w["as4"], w["ad4"], 1)               # [128,33]

    bout = np.stack([padr(w["b1"][cm], 128), padr(w["b2"][cm], 128),
                     padr(w["b3"][cm], 128), padr(w["b4"], 128)])

    gcnt = np.bincount(batch, minlength=Gn).astype(np.float32)
    cntr = (1.0 / np.maximum(gcnt, 1.0))[None, :]           # [1, Gn]
    shared = dict(
        W1=W1e, WL2=W2e, WL3=W3e, WL4=W4e,
        W5X10=W5x10, BOUT=bout, CNTR=cntr,
        WD=w["Wd"], BD=w["bd"][:, None], WLIN=w["Wl"], DESCT=desc.T.copy(),
    )
    bl = float(np.asarray(w["bl"]).reshape(-1)[0])

    dims = dict(N=N, E=E, Gn=Gn, NPC=NPC, NW=NW, NBINS=NBINS,
                C=C_total, cpb=cpb, off=off, bl=bl)
    return dims, shared, per_core


# ------------------------------------------------------------- program build
def build_program(dims, shared):
    import concourse.bass as bass
    import concourse.mybir as mybir
    import concourse.tile as tile
    import concourse.bacc as bacc
    from concourse.masks import make_identity
    from contextlib import ExitStack

    F32 = mybir.dt.float32
    BF16 = mybir.dt.bfloat16
    I32 = mybir.dt.int32
    I16 = mybir.dt.int16
    AF = mybir.ActivationFunctionType
    ALU = mybir.AluOpType
    AX = mybir.AxisListType

    N, Gn, NPC, NW, NBINS, C = (dims[k] for k in ("N", "Gn", "NPC", "NW", "NBINS", "C"))
    cpb, off, bl = dims["cpb"], dims["off"], dims["bl"]
    NSS = C // SS
    # layer params: h width, heads, rhs width, gather row elems
    LP = [dict(HW=128, AW=4, RW=142, EL=ROW),   # L1 rhs: h,ex,eterm9,cnt
          dict(HW=128, AW=4, RW=132, EL=ROW),
          dict(HW=128, AW=4, RW=132, EL=ROW),
          dict(HW=32, AW=1, RW=33, EL=ROW4)]

    nc = bacc.Bacc(num_swdge_queues=2)
    SIM1 = dims.get("sim1", False)

    # ---- params
    pr = {}
    for nm, shp, dt in [("SRC16", [P, C * 8], I16), ("DSTR", [P, C], F32),
                        ("EAT", [5, C * CHUNK], BF16), ("XT", [8, NW * P], F32),
                        ("BATCH", [P, NW], F32), ("W1", [8, 132], F32),
                        ("WL2", [128, 132], F32), ("WL3", [128, 132], F32),
                        ("WL4", [128, 33], F32), ("W5X10", [5, 10], F32),
                        ("BOUT", [4, 128], F32),
                        ("WD", [48, 32], F32), ("BD", [32, 1], F32),
                        ("WLIN", [64, 1], F32), ("DESCT", [48, Gn], F32),
                        ("CNTR", [1, Gn], F32)]:
        pr[nm] = nc.declare_dram_parameter(nm, shp, dt, isOutput=False)
    out_p = nc.declare_dram_parameter("out", [1, Gn], F32, isOutput=True)

    # ---- internal DRAM
    T_loc = [nc.dram_tensor(f"T_loc{l}", [NPC, LP[l]["EL"]], BF16) for l in range(4)]
    T_glob = [nc.dram_tensor(f"T_glob{l}", [N, LP[l]["EL"]], BF16, addr_space="Shared")
              for l in range(4)]
    ar_in = nc.dram_tensor("ar_in", [32, Gn], F32)
    ar_out = nc.dram_tensor("ar_out", [32, Gn], F32, addr_space="Shared")

    # bin/window bookkeeping (compile-time)
    bin_of_chunk = []
    for b in range(NBINS):
        bin_of_chunk += [b] * int(cpb[b])
    BPW = P // BIN  # bins per window
    win_of_bin = [b // BPW for b in range(NBINS)]
    last_chunk_of_bin = {}
    first_chunk_of_bin = {}
    for c_i, b in enumerate(bin_of_chunk):
        last_chunk_of_bin[b] = c_i
        first_chunk_of_bin.setdefault(b, c_i)

    with tile.TileContext(nc) as tc, ExitStack() as ctx:
        cp = ctx.enter_context(tc.tile_pool(name="const", bufs=1))
        wp = ctx.enter_context(tc.tile_pool(name="work", bufs=2))
        vp = ctx.enter_context(tc.tile_pool(name="win", bufs=2))
        pp = ctx.enter_context(tc.tile_pool(name="psum", bufs=2, space="PSUM"))
        bp = ctx.enter_context(tc.tile_pool(name="binp", bufs=2, space="PSUM"))

        sync, gps, vec, act, pe = nc.sync, nc.gpsimd, nc.vector, nc.scalar, nc.tensor
        ZTPB = dims.get("ztpb", 1)

        # ---- resident tiles
        src16 = cp.tile([P, C * 8], I16)
        sync.dma_start(out=src16[:], in_=pr["SRC16"][:, :])
        dstr = cp.tile([P, C], BF16)
        gps.dma_start(out=dstr[:], in_=pr["DSTR"][:, :])   # f32 -> bf16 cast
        batcht = cp.tile([P, NW], F32)
        sync.dma_start(out=batcht[:], in_=pr["BATCH"][:, :])
        xT_sb = cp.tile([8, NW * P], F32)
        sync.dma_start(out=xT_sb[:], in_=pr["XT"][:, :])

        iota_i = cp.tile([P, BIN], I32)
        gps.iota(iota_i[:], pattern=[[1, BIN]], base=0, channel_multiplier=0)
        iotab = cp.tile([P, BIN], BF16)
        vec.tensor_copy(iotab[:], iota_i[:])
        iotag_i = cp.tile([P, Gn], I32)
        gps.iota(iotag_i[:], pattern=[[1, Gn]], base=0, channel_multiplier=0)
        iotagf = cp.tile([P, Gn], F32)
        vec.tensor_copy(iotagf[:], iotag_i[:])
        identf = cp.tile([P, P], F32)
        make_identity(nc, identf[:])

        w1_sb = cp.tile([8, 132], F32)
        sync.dma_start(out=w1_sb[:], in_=pr["W1"][:, :])
        wl_sb = [None,
                 cp.tile([128, 132], BF16, name="wl2", tag="wl2"),
                 cp.tile([128, 132], BF16, name="wl3", tag="wl3"),
                 cp.tile([128, 33], BF16, name="wl4", tag="wl4")]
        gps.dma_start(out=wl_sb[1][:], in_=pr["WL2"][:, :])   # gpsimd casts f32->bf16
        gps.dma_start(out=wl_sb[2][:], in_=pr["WL3"][:, :])
        gps.dma_start(out=wl_sb[3][:], in_=pr["WL4"][:, :])
        w5x10 = cp.tile([5, 10], BF16)
        gps.dma_start(out=w5x10[:], in_=pr["W5X10"][:, :])
        bout_t = []
        for l in range(4):
            t3 = cp.tile([P, 128], F32, tag=f"bout{l}")
            sync.dma_start(out=t3[:], in_=pr["BOUT"][l:l + 1, :].to_broadcast([P, 128]))
            bout_t.append(t3)

        etc = cp.tile([P, C, 10], BF16)      # eterm9 | cnt  per edge
        pt_all = cp.tile([P, C, BIN], BF16)  # one-hot dst rows per edge
        loop_sb = cp.tile([P, NW, 10], F32)
        gsp = ctx.enter_context(tc.tile_pool(name="gsp", bufs=1, space="PSUM"))
        eap = ctx.enter_context(tc.tile_pool(name="eap", bufs=1))
        gsum_ps = None
        n_pool_mm = [0]

        # ---- readout head start: descriptor branch is input-independent
        comb = cp.tile([64, Gn], F32)
        wd_sb = cp.tile([48, 32], F32)
        sync.dma_start(out=wd_sb[:], in_=pr["WD"][:, :])
        desct_sb = cp.tile([48, Gn], F32)
        sync.dma_start(out=desct_sb[:], in_=pr["DESCT"][:, :])
        bd_sb = cp.tile([32, 1], F32)
        sync.dma_start(out=bd_sb[:], in_=pr["BD"][:, :])
        dps = pp.tile([32, Gn], F32, tag="hps", bufs=4 - ZTPB)
        pe.matmul(out=dps[:], lhsT=wd_sb[:], rhs=desct_sb[:], start=True, stop=True)
        act.activation(out=comb[32:64, :], in_=dps[:], func=AF.Relu, bias=bd_sb[:])
        wlin_sb = cp.tile([64, 1], F32)
        sync.dma_start(out=wlin_sb[:], in_=pr["WLIN"][:, :])
        cntrb = cp.tile([32, Gn], F32)
        sync.dma_start(out=cntrb[:], in_=pr["CNTR"][0:1, :].to_broadcast([32, Gn]))

        WG = dims.get("wg", 5)  # max windows per epilogue group
        # non-uniform groups: taper toward the end so the serial layer-boundary
        # tail (last epilogue -> node phase -> AllGather) shrinks
        grp_bounds = []
        w0_ = 0
        while NW - w0_ > 10:
            grp_bounds.append((w0_, WG))
            w0_ += WG
        for t_ in (4, 3, 2, 1):
            if NW - w0_ > t_:
                grp_bounds.append((w0_, t_))
                w0_ += t_
        if NW > w0_:
            grp_bounds.append((w0_, NW - w0_))
        NG = len(grp_bounds)
        grp_of_win = {}
        for gi, (gw0, gsz_) in enumerate(grp_bounds):
            for w_ in range(gw0, gw0 + gsz_):
                grp_of_win[w_] = gi
        last_chunk_of_grp = {}
        for b in range(NBINS):
            g_ = grp_of_win[win_of_bin[b]]
            last_chunk_of_grp[g_] = max(last_chunk_of_grp.get(g_, -1),
                                        last_chunk_of_bin[b])

        # T_sb tables: [P, NW, 132] (h | b); layer l+1's is built during
        # layer l's edge phase, group by group.
        def node_phase_group(l, g_, T_next, z_src):
            """Build T_next rows for group g_ of layer l (0-based), write T_loc."""
            w0, gsz = grp_bounds[g_]
            HWn = LP[l]["HW"]
            BW = HWn + LP[l]["AW"]  # table row width
            for w_ in range(w0, w0 + gsz):
                if l == 0:
                    hps = pp.tile([P, 132], F32, tag="hps", bufs=4 - ZTPB)
                    pe.matmul(out=hps[:, 0:BW], lhsT=xT_sb[:, w_ * P:(w_ + 1) * P],
                              rhs=w1_sb[:], start=True, stop=True)
                else:
                    ztp = pp.tile([P, P], F32, tag="ztp", bufs=ZTPB)
                    pe.transpose(out=ztp[:], in_=z_src[:, w_ - w0, 0:128],
                                 identity=identf[:])
                    zt_sb = wp.tile([P, P], BF16, tag="ztsb")
                    act.copy(out=zt_sb[:], in_=ztp[:])
                    hps = pp.tile([P, 132], F32, tag="hps", bufs=4 - ZTPB)
                    pe.matmul(out=hps[:, 0:BW], lhsT=zt_sb[:], rhs=wl_sb[l][:],
                              start=True, stop=True)
                act.copy(out=T_next[:, w_, 0:BW], in_=hps[:, 0:BW])
            # batched table write: full windows in one DMA, ragged tail apart
            wfull = gsz - (1 if (w0 + gsz) * P > NPC else 0)
            if wfull > 0:
                sync.dma_start(
                    out=T_loc[l][w0 * P:(w0 + wfull) * P, 0:BW]
                        .rearrange("(w p) e -> p w e", p=P),
                    in_=T_next[:, w0:w0 + wfull, 0:BW])
            if wfull < gsz:
                w_ = w0 + wfull
                nr = NPC - w_ * P
                sync.dma_start(out=T_loc[l][w_ * P:w_ * P + nr, 0:BW],
                               in_=T_next[0:nr, w_, 0:BW])

        PT_AHEAD = dims.get("pt_ahead", 64)
        # prebuild the one-hot cache for the first chunks while the layer-0
        # node phase occupies PE/Act
        for g in range(0, PT_AHEAD, 8):
            vec.tensor_tensor(
                out=pt_all[:, g:g + 8, :],
                in0=dstr[:, g:g + 8].unsqueeze(2).to_broadcast([P, 8, BIN]),
                in1=iotab[:].unsqueeze(1).to_broadcast([P, 8, BIN]),
                op=ALU.is_equal)

        # ---- layer 0 node phase (all groups up front)
        T_sb = wp.tile([P, NW, 132], BF16, tag="tsb")
        for g_ in range(NG):
            node_phase_group(0, g_, T_sb, None)
        if SIM1:
            sync.dma_start(out=T_glob[0][0:NPC, :], in_=T_loc[0][:, :])
        else:
            gps.collective_compute(
                "AllGather", ALU.bypass, replica_groups=[list(range(NCORES))],
                ins=[T_loc[0][:, :]], outs=[T_glob[0][:, :]])

        for l in range(4):
            HW, AW, RW, EL = (LP[l][k] for k in ("HW", "AW", "RW", "EL"))
            BW = HW + AW

            T_next = None
            if l < 3:
                T_next = wp.tile([P, NW, 132], BF16, name="tnext", tag="tsb")

            grp_tiles = {}
            grp_done = set()

            def open_group(g_):
                t = vp.tile([P, WG, 142], F32, name="wingrp", tag="wingrp")
                grp_tiles[g_] = t
                return t

            def epilogue_group(g_):
                w0, gsz = grp_bounds[g_]
                wg = grp_tiles[g_]
                scr = wp.tile([P, WG, 12], F32, name="scr", tag="scr")
                # self-loop alpha (= b_own [+ eterm means]) -> exp
                if l > 0:
                    sl = [None, (0, 4), (4, 8), (8, 9)][l]
                    vec.tensor_tensor(out=scr[:, 0:gsz, 0:AW],
                                      in0=T_sb[:, w0:w0 + gsz, HW:HW + AW],
                                      in1=loop_sb[:, w0:w0 + gsz, sl[0]:sl[1]],
                                      op=ALU.add)
                else:
                    act.copy(out=scr[:, 0:gsz, 0:AW],
                             in_=T_sb[:, w0:w0 + gsz, HW:HW + AW])
                vec.tensor_scalar_mul(out=scr[:, 0:gsz, 4:4 + AW],
                                      in0=scr[:, 0:gsz, 0:AW], scalar1=0.2)
                vec.tensor_tensor(out=scr[:, 0:gsz, 0:AW], in0=scr[:, 0:gsz, 0:AW],
                                  in1=scr[:, 0:gsz, 4:4 + AW], op=ALU.max)
                act.activation(out=scr[:, 0:gsz, 0:AW], in_=scr[:, 0:gsz, 0:AW],
                               func=AF.Exp)
                # num += h_own * ex_loop
                nt = wp.tile([P, WG, 128], BF16, name="nt", tag="nt")
                vec.tensor_tensor(
                    out=nt[:, 0:gsz, 0:HW].rearrange("p g (c a) -> p g c a", a=AW),
                    in0=T_sb[:, w0:w0 + gsz, 0:HW].rearrange("p g (c a) -> p g c a", a=AW),
                    in1=scr[:, 0:gsz, 0:AW].unsqueeze(2)
                        .to_broadcast([P, gsz, HW // AW, AW]),
                    op=ALU.mult)
                vec.tensor_tensor(out=wg[:, 0:gsz, 0:HW], in0=wg[:, 0:gsz, 0:HW],
                                  in1=nt[:, 0:gsz, 0:HW], op=ALU.add)
                # den -> reciprocal
                vec.tensor_tensor(out=scr[:, 0:gsz, 4:4 + AW],
                                  in0=wg[:, 0:gsz, HW:HW + AW],
                                  in1=scr[:, 0:gsz, 0:AW], op=ALU.add)
                vec.tensor_scalar_add(out=scr[:, 0:gsz, 4:4 + AW],
                                      in0=scr[:, 0:gsz, 4:4 + AW], scalar1=1e-16)
                vec.reciprocal(out=scr[:, 0:gsz, 4:4 + AW], in_=scr[:, 0:gsz, 4:4 + AW])
                if l == 0:
                    vec.tensor_scalar_max(out=scr[:, 0:gsz, 8:9],
                                          in0=wg[:, 0:gsz, 141:142], scalar1=1.0)
                    vec.reciprocal(out=scr[:, 0:gsz, 8:9], in_=scr[:, 0:gsz, 8:9])
                    vec.tensor_tensor(
                        out=loop_sb[:, w0:w0 + gsz, 0:9], in0=wg[:, 0:gsz, 132:141],
                        in1=scr[:, 0:gsz, 8:9].to_broadcast([P, gsz, 9]), op=ALU.mult)
                # z = num * recip(den) + bias [+ relu]
                vec.tensor_tensor(
                    out=wg[:, 0:gsz, 0:HW].rearrange("p g (c a) -> p g c a", a=AW),
                    in0=wg[:, 0:gsz, 0:HW].rearrange("p g (c a) -> p g c a", a=AW),
                    in1=scr[:, 0:gsz, 4:4 + AW].unsqueeze(2)
                        .to_broadcast([P, gsz, HW // AW, AW]),
                    op=ALU.mult)
                vec.tensor_tensor(
                    out=wg[:, 0:gsz, 0:HW], in0=wg[:, 0:gsz, 0:HW],
                    in1=bout_t[l][:, 0:HW].unsqueeze(1).to_broadcast([P, gsz, HW]),
                    op=ALU.add)
                if l < 3:
                    act.activation(out=wg[:, 0:gsz, 0:128], in_=wg[:, 0:gsz, 0:128],
                                   func=AF.Relu)
                    node_phase_group(l + 1, g_, T_next, wg)
                else:
                    nonlocal gsum_ps
                    pool_sb = wp.tile([P, WG, 32], BF16, name="pool_sb", tag="poolsb")
                    act.copy(out=pool_sb[:, 0:gsz, 0:32], in_=wg[:, 0:gsz, 0:32])
                    bt = wp.tile([P, WG, Gn], BF16, name="bt", tag="bt", bufs=1)
                    vec.tensor_tensor(
                        out=bt[:, 0:gsz, :],
                        in0=batcht[:, w0:w0 + gsz].unsqueeze(2).to_broadcast([P, gsz, Gn]),
                        in1=iotagf[:].unsqueeze(1).to_broadcast([P, gsz, Gn]),
                        op=ALU.is_equal)
                    if gsum_ps is None:
                        gsum_ps = gsp.tile([32, Gn], F32, name="gsum_ps")
                    for j_ in range(gsz):
                        n_pool_mm[0] += 1
                        pe.matmul(out=gsum_ps[:], lhsT=pool_sb[:, j_, :],
                                  rhs=bt[:, j_, :],
                                  start=(n_pool_mm[0] == 1),
                                  stop=(n_pool_mm[0] == NW))
                grp_done.add(g_)

            cur_bin_tile = {}
            ss_plan = []
            rem_ = C
            while rem_ > 0:
                n_ = min(SS, rem_)
                ss_plan.append(n_)
                rem_ -= n_
            for t_ in dims.get("ss_tail", (8,)):
                if ss_plan[-1] > t_:
                    ss_plan[-1] -= t_
                    ss_plan.append(t_)
            s0 = 0
            for ss, NCH in enumerate(ss_plan):
                Gt = wp.tile([P, SS, EL], BF16, tag="gt", bufs=4)
                gps.dma_gather(
                    out_ap=Gt[:, 0:NCH, :], in_ap=T_glob[l][:, :],
                    idxs_ap=src16[:, s0 * 8:(s0 + NCH) * 8],
                    num_idxs=NCH * CHUNK, num_idxs_reg=NCH * CHUNK, elem_size=EL,
                    single_packet=False, queue_num=ss % 2)
                if l == 0:
                    # edge-term + mask precompute (feeds rhs cols 132:142 +
                    # later layers' alpha); mask folded into EAT row 5.
                    eaT_sl = eap.tile([5, SS * CHUNK], BF16, name="easl", tag="eat")
                    sync.dma_start(
                        out=eaT_sl[:, 0:NCH * CHUNK],
                        in_=pr["EAT"][:, s0 * CHUNK:(s0 + NCH) * CHUNK])
                    for q0 in range(0, NCH, 16):
                        qn = min(16, NCH - q0)
                        etp = pp.tile([P, 160], F32, tag="etp", bufs=1)
                        for j in range(qn):
                            ci = q0 + j
                            pe.matmul(out=etp[:, j * 10:(j + 1) * 10],
                                      lhsT=eaT_sl[:, ci * CHUNK:(ci + 1) * CHUNK],
                                      rhs=w5x10[:], start=True, stop=True)
                        act.copy(out=etc[:, s0 + q0:s0 + q0 + qn, :]
                                 .rearrange("p a b -> p (a b)"),
                                 in_=etp[:, 0:qn * 10])
                    # staircase one-hots built once, reused by all layers;
                    # built PT_AHEAD chunks ahead so the DVE cost sits in the
                    # pipeline's slack instead of its critical phase
                    pb0 = PT_AHEAD + s0
                    pb1 = min(pb0 + NCH, C)
                    for g in range(pb0, pb1, 8):
                        gn = min(8, pb1 - g)
                        vec.tensor_tensor(
                            out=pt_all[:, g:g + gn, :],
                            in0=dstr[:, g:g + gn].unsqueeze(2).to_broadcast([P, gn, BIN]),
                            in1=iotab[:].unsqueeze(1).to_broadcast([P, gn, BIN]),
                            op=ALU.is_equal)
                # alpha = b[src] (+ eterm) -> leaky relu -> exp
                AT = wp.tile([P, SS, 8], BF16, tag="at", bufs=2)
                if l > 0:
                    sl = [None, (0, 4), (4, 8), (8, 9)][l]
                    vec.tensor_tensor(out=AT[:, 0:NCH, 0:AW],
                                      in0=Gt[:, 0:NCH, HW:HW + AW],
                                      in1=etc[:, s0:s0 + NCH, sl[0]:sl[1]],
                                      op=ALU.add)
                    vec.tensor_scalar_mul(out=AT[:, 0:NCH, AW:2 * AW],
                                          in0=AT[:, 0:NCH, 0:AW], scalar1=0.2)
                    vec.tensor_tensor(out=AT[:, 0:NCH, 0:AW], in0=AT[:, 0:NCH, 0:AW],
                                      in1=AT[:, 0:NCH, AW:2 * AW], op=ALU.max)
                else:
                    vec.tensor_scalar_mul(out=AT[:, 0:NCH, AW:2 * AW],
                                          in0=Gt[:, 0:NCH, HW:HW + AW], scalar1=0.2)
                    vec.tensor_tensor(out=AT[:, 0:NCH, 0:AW],
                                      in0=Gt[:, 0:NCH, HW:HW + AW],
                                      in1=AT[:, 0:NCH, AW:2 * AW], op=ALU.max)
                act.activation(out=Gt[:, 0:NCH, HW:HW + AW], in_=AT[:, 0:NCH, 0:AW],
                               func=AF.Exp)
                vec.tensor_tensor(
                    out=Gt[:, 0:NCH, 0:HW].rearrange("p s (c a) -> p s c a", a=AW),
                    in0=Gt[:, 0:NCH, 0:HW].rearrange("p s (c a) -> p s c a", a=AW),
                    in1=Gt[:, 0:NCH, HW:HW + AW].unsqueeze(2)
                        .to_broadcast([P, NCH, HW // AW, AW]),
                    op=ALU.mult)
                if l == 0:
                    # append eterm9|cnt as rhs cols 132:142
                    act.copy(out=Gt[:, 0:NCH, 132:142],
                             in_=etc[:, s0:s0 + NCH, :])
                # scatter matmuls
                for c_i in range(NCH):
                    gc = s0 + c_i
                    b = bin_of_chunk[gc]
                    w_ = win_of_bin[b]
                    g_ = grp_of_win[w_]
                    if g_ not in grp_tiles:
                        open_group(g_)
                    if gc == first_chunk_of_bin[b]:
                        cur_bin_tile[b] = bp.tile([BIN, 142], F32, name="binacc",
                                                  tag="binacc")
                    pe.matmul(out=cur_bin_tile[b][:, 0:RW],
                              lhsT=pt_all[:, gc, :], rhs=Gt[:, c_i, 0:RW],
                              start=(gc == first_chunk_of_bin[b]),
                              stop=(gc == last_chunk_of_bin[b]))
                    if gc == last_chunk_of_bin[b]:
                        j = b % BPW
                        wrel = w_ - grp_bounds[g_][0]
                        act.copy(out=grp_tiles[g_][BIN * j:BIN * (j + 1), wrel, 0:RW],
                                 in_=cur_bin_tile[b][:, 0:RW])
                        del cur_bin_tile[b]
                    if gc == last_chunk_of_grp.get(g_, None):
                        epilogue_group(g_)
                s0 += NCH
            # groups never triggered (e.g. all-empty windows)
            for g_ in range(NG):
                if g_ not in grp_done:
                    if g_ not in grp_tiles:
                        open_group(g_)
                    epilogue_group(g_)
            if l < 3:
                if SIM1:
                    sync.dma_start(out=T_glob[l + 1][0:NPC, :], in_=T_loc[l + 1][:, :])
                else:
                    gps.collective_compute(
                        "AllGather", ALU.bypass, replica_groups=[list(range(NCORES))],
                        ins=[T_loc[l + 1][:, :]], outs=[T_glob[l + 1][:, :]])
                T_sb = T_next

        # ============ readout
        gsum_sb = cp.tile([32, Gn], F32)
        act.copy(out=gsum_sb[:], in_=gsum_ps[:])
        sync.dma_start(out=ar_in[:], in_=gsum_sb[:])
        if SIM1:
            sync.dma_start(out=ar_out[:], in_=ar_in[:])
        else:
            gps.collective_compute("AllReduce", ALU.add,
                                   replica_groups=[list(range(NCORES))],
                                   ins=[ar_in[:]], outs=[ar_out[:]])
        gs = cp.tile([32, Gn], F32)
        sync.dma_start(out=gs[:], in_=ar_out[:])
        vec.tensor_tensor(out=comb[0:32, :], in0=gs[:, :], in1=cntrb[:],
                          op=ALU.mult)
        blt = cp.tile([1, 1], F32)
        vec.memset(blt[:], bl)
        fin = pp.tile([1, Gn], F32, tag="hps", bufs=4 - ZTPB)
        pe.matmul(out=fin[:], lhsT=wlin_sb[:], rhs=comb[:], start=True, stop=True)
        res_sb = cp.tile([1, Gn], F32)
        act.activation(out=res_sb[:], in_=fin[:], func=AF.Sigmoid, bias=blt[:])
        sync.dma_start(out=out_p[:, :], in_=res_sb[:])

    nc.finalize()
    return nc


# ------------------------------------------------------------------ entry
def _run(inputs, trace=False, debug=False):
    dims, shared, per_core = host_prep(inputs)
    nc = build_program(dims, shared)
    in_maps = [{**shared, **pc} for pc in per_core]
    from concourse.bass_utils import run_bass_kernel_spmd
    return run_bass_kernel_spmd(nc, in_maps, list(range(NCORES)), trace=trace)


def kernel(**inputs):
    res = _run(inputs)
    return res.results[0]["out"].reshape(-1).astype(np.float32)


# revision 22
# speedup vs baseline: 1.4082x; 1.0343x over previous
"""EnhancedGAT Trainium2 Bass kernel (8 NeuronCores, SPMD).

Strategy:
  - Edges are sorted by destination node on the host; core k owns dst nodes
    [k*N/8, (k+1)*N/8) and every edge targeting them. Per-core edge lists are
    bucketed into 64-node bins and padded to 128-edge chunks with a per-bin
    chunk count shared across cores (SPMD uniformity). Dummy (padding) edges
    carry dst-offset 64, which falls outside the 64-wide one-hot used by the
    scatter matmuls, so they contribute exactly nothing.
  - Each GAT layer:
      node phase: every core computes a table row [h(128) | b(4)] for its own
        nodes, where b = per-head <h, att_s + att_d> comes directly out of the
        h matmul via 4 extra weight columns W @ A. Rows live in a [NPC, 256]
        bf16 DRAM table (512B stride for the gather); an AllGather replicates
        it to every core.
      edge phase: per 4096-edge superstep one dma_gather pulls the rows for
        the edges' sources; attention coefficients alpha = b[src] (+ edge
        term) are leaky-relu'd and exp'd in place, messages h*ex are scattered
        into per-bin PSUM accumulators via one-hot matmuls. Softmax is
        unnormalized (max-subtraction skipped; alphas are O(0.3)); the divide
        happens per node at the group epilogue, where self-loop contributions
        are added. As soon as a window-group's epilogue finishes, the NEXT
        layer's node phase for those windows runs (transpose + matmul + table
        write), hiding the layer boundary behind the remaining gathers.
  - Layer 1 additionally computes, per edge, the folded edge-attention terms
    for layers 2-4 (eterm = ea @ V + be, with the padding mask folded in as a
    fifth all-ones/zeros EAT row) plus the per-edge mask into an [C,10] SBUF
    cache, and accumulates per-node mean edge-feature terms and in-degrees
    (extra scatter-matmul columns) used by the self-loops of layers 2-4.
  - Final graph mean-pool via one-hot matmuls into a [33, G] accumulator,
    AllReduce across cores, tiny dense readout replicated on every core.
"""
import sys
import numpy as np

sys.path.insert(0, "/opt/trn_rl_repo")

HID = 32
NCORES = 8
P = 128
BIN = 64
SS = 32          # chunks per superstep
CHUNK = 128
ROW = 256        # table row elements (bf16) for layers 1-3 (512B stride)
ROW4 = 128       # layer-4 table row elements


# ----------------------------------------------------------------- host prep
def host_prep(inputs):
    x = np.asarray(inputs["x"], np.float32)
    ei = np.asarray(inputs["edge_index"]).astype(np.int64)
    ea = np.asarray(inputs["edge_attr"], np.float32)
    batch = np.asarray(inputs["batch"]).astype(np.int64)
    desc = np.asarray(inputs["descriptors"], np.float32)

    N = x.shape[0]
    E = ei.shape[1]
    Gn = desc.shape[0]
    NPC = N // NCORES
    NW = -(-NPC // P)
    NBINS = -(-NPC // BIN)

    src_all, dst_all = ei[0], ei[1]
    order = np.argsort(dst_all, kind="stable")
    src_s, dst_s = src_all[order], dst_all[order]
    ea_s = ea[order]
    core_of = dst_s // NPC
    local = dst_s - core_of * NPC
    bin_of = local // BIN

    cnt = np.zeros((NCORES, NBINS), np.int64)
    np.add.at(cnt, (core_of, bin_of), 1)
    cpb = np.max(-(-cnt // CHUNK), axis=0)          # chunks per bin (shared)
    cpb = np.maximum(cpb, 1)                        # every bin gets a chunk
    C_total = int(cpb.sum())
    off = np.zeros(NBINS, np.int64)
    off[1:] = np.cumsum(cpb)[:-1]
    EP = C_total * CHUNK                            # padded edges per core

    per_core = []
    for k in range(NCORES):
        srck = np.zeros(EP, np.int64)
        dstrk = np.full(EP, float(BIN), np.float32)  # dummies -> dead one-hot
        maskk = np.zeros(EP, np.float32)
        eak = np.zeros((EP, 4), np.float32)
        sel = core_of == k
        bins_k = bin_of[sel]
        start = np.searchsorted(bins_k, np.arange(NBINS))
        pos = np.arange(bins_k.size) - start[bins_k]
        slot = off[bins_k] * CHUNK + pos
        srck[slot] = src_s[sel]
        dstrk[slot] = (local[sel] - bins_k * BIN).astype(np.float32)
        maskk[slot] = 1.0
        eak[slot] = ea_s[sel]

        # device layouts: edge e = c*128 + p
        src16 = np.tile(srck.reshape(-1, 16).T.astype(np.int16), (8, 1))
        dstr_d = dstrk.reshape(C_total, P).T.copy()
        import ml_dtypes
        ea5 = np.concatenate([eak.T, maskk[None, :]], axis=0).astype(ml_dtypes.bfloat16)

        xk = x[k * NPC:(k + 1) * NPC]
        xT = np.zeros((8, NW * P), np.float32)
        xT[:, :NPC] = xk.T
        bk = np.full(NW * P, Gn + 5, np.float32)
        bk[:NPC] = batch[k * NPC:(k + 1) * NPC].astype(np.float32)
        batch_d = bk.reshape(NW, P).T.copy()

        per_core.append(dict(SRC16=src16, DSTR=dstr_d, EAT=ea5,
                             XT=xT, BATCH=batch_d))

    # ---- weight folding
    w = {k: np.asarray(v, np.float32) for k, v in inputs.items()
         if k not in ("x", "edge_index", "edge_attr", "batch", "descriptors")}

    def vfold(We, ae, heads):
        Vp = (We.reshape(w["We_enc"].shape[1], heads, HID) * ae[None]).sum(-1)
        return w["We_enc"] @ Vp, w["be_enc"] @ Vp      # [4,heads],[heads]

    V2, bv2 = vfold(w["We2"], w["ae2"], 4)
    V3, bv3 = vfold(w["We3"], w["ae3"], 4)
    V4, bv4 = vfold(w["We4"], w["ae4"], 1)
    # [5,10]: rows = 4 edge-attr dims + mask; cols = 9 eterms + cnt
    W5x10 = np.zeros((5, 10), np.float32)
    W5x10[0:4, 0:9] = np.concatenate([V2, V3, V4], axis=1)
    W5x10[4, 0:9] = np.concatenate([bv2, bv3, bv4])
    W5x10[4, 9] = 1.0

    def padr(v, n):
        o = np.zeros(n, np.float32)
        o[: v.size] = v
        return o

    # channel-major reorder of the 128-wide (4 heads x 32 ch) dimension:
    # new position c*4+a holds old a*32+c. Keeps per-head broadcasts
    # innermost-packed on DVE (2x mode).
    cm = (np.arange(128) % 4) * 32 + np.arange(128) // 4

    def wext(W, att_s, att_d, heads):
        # append per-head b-columns: b_a = h . (att_s+att_d)_a
        att = (att_s + att_d).reshape(-1)  # [heads*HID] head-major
        if heads == 4:
            attc = att[cm]                 # channel-major to match W cols
            A = np.zeros((128, 4), np.float32)
            A[np.arange(128), np.arange(128) % 4] = attc
        else:
            A = att[:, None]               # [32,1]
        return np.concatenate([W, W @ A], axis=1)

    W1e = wext(w["W1"][:, cm], w["as1"], w["ad1"], 4)            # [8,132]
    W2e = wext(w["W2"][cm][:, cm], w["as2"], w["ad2"], 4)        # [128,132]
    W3e = wext(w["W3"][cm][:, cm], w["as3"], w["ad3"], 4)
    W4e = wext(w["W4"][cm], w["as4"], w["ad4"], 1)               # [128,33]

    bout = np.stack([padr(w["b1"][cm], 128), padr(w["b2"][cm], 128),
                     padr(w["b3"][cm], 128), padr(w["b4"], 128)])

    import ml_dtypes
    T0 = np.zeros((N, ROW), np.float32)
    T0[:, 0:132] = x @ W1e
    TG0 = T0.astype(ml_dtypes.bfloat16)

    gcnt = np.bincount(batch, minlength=Gn).astype(np.float32)
    cntr = (1.0 / np.maximum(gcnt, 1.0))[None, :]           # [1, Gn]
    shared = dict(
        W1=W1e, WL2=W2e, WL3=W3e, WL4=W4e, TG0=TG0,
        W5X10=W5x10, BOUT=bout, CNTR=cntr,
        WD=w["Wd"], BD=w["bd"][:, None], WLIN=w["Wl"], DESCT=desc.T.copy(),
    )
    bl = float(np.asarray(w["bl"]).reshape(-1)[0])

    dims = dict(N=N, E=E, Gn=Gn, NPC=NPC, NW=NW, NBINS=NBINS,
                C=C_total, cpb=cpb, off=off, bl=bl)
    return dims, shared, per_core


# ------------------------------------------------------------- program build
def build_program(dims, shared):
    import concourse.bass as bass
    import concourse.mybir as mybir
    import concourse.tile as tile
    import concourse.bacc as bacc
    from concourse.masks import make_identity
    from contextlib import ExitStack

    F32 = mybir.dt.float32
    BF16 = mybir.dt.bfloat16
    I32 = mybir.dt.int32
    I16 = mybir.dt.int16
    AF = mybir.ActivationFunctionType
    ALU = mybir.AluOpType
    AX = mybir.AxisListType

    N, Gn, NPC, NW, NBINS, C = (dims[k] for k in ("N", "Gn", "NPC", "NW", "NBINS", "C"))
    cpb, off, bl = dims["cpb"], dims["off"], dims["bl"]
    NSS = C // SS
    # layer params: h width, heads, rhs width, gather row elems
    LP = [dict(HW=128, AW=4, RW=142, EL=ROW),   # L1 rhs: h,ex,eterm9,cnt
          dict(HW=128, AW=4, RW=132, EL=ROW),
          dict(HW=128, AW=4, RW=132, EL=ROW),
          dict(HW=32, AW=1, RW=33, EL=ROW4)]

    nc = bacc.Bacc(num_swdge_queues=2)
    SIM1 = dims.get("sim1", False)

    # ---- params
    pr = {}
    for nm, shp, dt in [("SRC16", [P, C * 8], I16), ("DSTR", [P, C], F32),
                        ("EAT", [5, C * CHUNK], BF16), ("XT", [8, NW * P], F32),
                        ("BATCH", [P, NW], F32), ("W1", [8, 132], F32),
                        ("WL2", [128, 132], F32), ("WL3", [128, 132], F32),
                        ("WL4", [128, 33], F32), ("W5X10", [5, 10], F32),
                        ("BOUT", [4, 128], F32),
                        ("WD", [48, 32], F32), ("BD", [32, 1], F32),
                        ("WLIN", [64, 1], F32), ("DESCT", [48, Gn], F32),
                        ("CNTR", [1, Gn], F32), ("TG0", [N, ROW], BF16)]:
        pr[nm] = nc.declare_dram_parameter(nm, shp, dt, isOutput=False)
    out_p = nc.declare_dram_parameter("out", [1, Gn], F32, isOutput=True)
    pr_TG0_ph = pr["TG0"]

    # ---- internal DRAM
    T_loc = [None] + [nc.dram_tensor(f"T_loc{l}", [NPC, LP[l]["EL"]], BF16)
                      for l in range(1, 4)]
    T_glob = [pr_TG0_ph] + [nc.dram_tensor(f"T_glob{l}", [N, LP[l]["EL"]], BF16,
                                           addr_space="Shared")
                            for l in range(1, 4)]
    ar_in = nc.dram_tensor("ar_in", [32, Gn], F32)
    ar_out = nc.dram_tensor("ar_out", [32, Gn], F32, addr_space="Shared")

    # bin/window bookkeeping (compile-time)
    bin_of_chunk = []
    for b in range(NBINS):
        bin_of_chunk += [b] * int(cpb[b])
    BPW = P // BIN  # bins per window
    win_of_bin = [b // BPW for b in range(NBINS)]
    last_chunk_of_bin = {}
    first_chunk_of_bin = {}
    for c_i, b in enumerate(bin_of_chunk):
        last_chunk_of_bin[b] = c_i
        first_chunk_of_bin.setdefault(b, c_i)

    with tile.TileContext(nc) as tc, ExitStack() as ctx:
        cp = ctx.enter_context(tc.tile_pool(name="const", bufs=1))
        wp = ctx.enter_context(tc.tile_pool(name="work", bufs=2))
        vp = ctx.enter_context(tc.tile_pool(name="win", bufs=2))
        pp = ctx.enter_context(tc.tile_pool(name="psum", bufs=2, space="PSUM"))
        bp = ctx.enter_context(tc.tile_pool(name="binp", bufs=2, space="PSUM"))

        sync, gps, vec, act, pe = nc.sync, nc.gpsimd, nc.vector, nc.scalar, nc.tensor
        ZTPB = dims.get("ztpb", 1)

        # ---- resident tiles
        src16 = cp.tile([P, C * 8], I16)
        sync.dma_start(out=src16[:], in_=pr["SRC16"][:, :])
        dstr = cp.tile([P, C], BF16)
        gps.dma_start(out=dstr[:], in_=pr["DSTR"][:, :])   # f32 -> bf16 cast
        batcht = cp.tile([P, NW], F32)
        sync.dma_start(out=batcht[:], in_=pr["BATCH"][:, :])
        xT_sb = cp.tile([8, NW * P], F32)
        sync.dma_start(out=xT_sb[:], in_=pr["XT"][:, :])

        iota_i = cp.tile([P, BIN], I32)
        gps.iota(iota_i[:], pattern=[[1, BIN]], base=0, channel_multiplier=0)
        iotab = cp.tile([P, BIN], BF16)
        vec.tensor_copy(iotab[:], iota_i[:])
        iotag_i = cp.tile([P, Gn], I32)
        gps.iota(iotag_i[:], pattern=[[1, Gn]], base=0, channel_multiplier=0)
        iotagf = cp.tile([P, Gn], F32)
        vec.tensor_copy(iotagf[:], iotag_i[:])
        identf = cp.tile([P, P], F32)
        make_identity(nc, identf[:])

        w1_sb = cp.tile([8, 132], F32)
        sync.dma_start(out=w1_sb[:], in_=pr["W1"][:, :])
        wl_sb = [None,
                 cp.tile([128, 132], BF16, name="wl2", tag="wl2"),
                 cp.tile([128, 132], BF16, name="wl3", tag="wl3"),
                 cp.tile([128, 33], BF16, name="wl4", tag="wl4")]
        gps.dma_start(out=wl_sb[1][:], in_=pr["WL2"][:, :])   # gpsimd casts f32->bf16
        gps.dma_start(out=wl_sb[2][:], in_=pr["WL3"][:, :])
        gps.dma_start(out=wl_sb[3][:], in_=pr["WL4"][:, :])
        w5x10 = cp.tile([5, 10], BF16)
        gps.dma_start(out=w5x10[:], in_=pr["W5X10"][:, :])
        bout_t = []
        for l in range(4):
            t3 = cp.tile([P, 128], F32, tag=f"bout{l}")
            sync.dma_start(out=t3[:], in_=pr["BOUT"][l:l + 1, :].to_broadcast([P, 128]))
            bout_t.append(t3)

        etc = cp.tile([P, C, 10], BF16)      # eterm9 | cnt  per edge
        pt_all = cp.tile([P, C, BIN], BF16)  # one-hot dst rows per edge
        loop_sb = cp.tile([P, NW, 10], F32)
        gsp = ctx.enter_context(tc.tile_pool(name="gsp", bufs=1, space="PSUM"))
        eap = ctx.enter_context(tc.tile_pool(name="eap", bufs=1))
        gsum_ps = None
        n_pool_mm = [0]

        # ---- readout head start: descriptor branch is input-independent
        comb = cp.tile([64, Gn], F32)
        wd_sb = cp.tile([48, 32], F32)
        sync.dma_start(out=wd_sb[:], in_=pr["WD"][:, :])
        desct_sb = cp.tile([48, Gn], F32)
        sync.dma_start(out=desct_sb[:], in_=pr["DESCT"][:, :])
        bd_sb = cp.tile([32, 1], F32)
        sync.dma_start(out=bd_sb[:], in_=pr["BD"][:, :])
        dps = pp.tile([32, Gn], F32, tag="hps", bufs=4 - ZTPB)
        pe.matmul(out=dps[:], lhsT=wd_sb[:], rhs=desct_sb[:], start=True, stop=True)
        act.activation(out=comb[32:64, :], in_=dps[:], func=AF.Relu, bias=bd_sb[:])
        wlin_sb = cp.tile([64, 1], F32)
        sync.dma_start(out=wlin_sb[:], in_=pr["WLIN"][:, :])
        cntrb = cp.tile([32, Gn], F32)
        sync.dma_start(out=cntrb[:], in_=pr["CNTR"][0:1, :].to_broadcast([32, Gn]))

        WG = dims.get("wg", 5)  # max windows per epilogue group
        # non-uniform groups: taper toward the end so the serial layer-boundary
        # tail (last epilogue -> node phase -> AllGather) shrinks
        grp_bounds = []
        w0_ = 0
        while NW - w0_ > 10:
            grp_bounds.append((w0_, WG))
            w0_ += WG
        for t_ in dims.get("taper", (4, 3, 2, 1)):
            if NW - w0_ > t_:
                grp_bounds.append((w0_, t_))
                w0_ += t_
        if NW > w0_:
            grp_bounds.append((w0_, NW - w0_))
        NG = len(grp_bounds)
        grp_of_win = {}
        for gi, (gw0, gsz_) in enumerate(grp_bounds):
            for w_ in range(gw0, gw0 + gsz_):
                grp_of_win[w_] = gi
        last_chunk_of_grp = {}
        for b in range(NBINS):
            g_ = grp_of_win[win_of_bin[b]]
            last_chunk_of_grp[g_] = max(last_chunk_of_grp.get(g_, -1),
                                        last_chunk_of_bin[b])

        # T_sb tables: [P, NW, 132] (h | b); layer l+1's is built during
        # layer l's edge phase, group by group.
        def node_phase_group(l, g_, T_next, z_src):
            """Build T_next rows for group g_ of layer l (0-based), write T_loc."""
            w0, gsz = grp_bounds[g_]
            HWn = LP[l]["HW"]
            BW = HWn + LP[l]["AW"]  # table row width
            for w_ in range(w0, w0 + gsz):
                if l == 0:
                    hps = pp.tile([P, 132], F32, tag="hps", bufs=4 - ZTPB)
                    pe.matmul(out=hps[:, 0:BW], lhsT=xT_sb[:, w_ * P:(w_ + 1) * P],
                              rhs=w1_sb[:], start=True, stop=True)
                else:
                    ztp = pp.tile([P, P], F32, tag="ztp", bufs=ZTPB)
                    pe.transpose(out=ztp[:], in_=z_src[:, w_ - w0, 0:128],
                                 identity=identf[:])
                    zt_sb = wp.tile([P, P], BF16, tag="ztsb")
                    act.copy(out=zt_sb[:], in_=ztp[:])
                    hps = pp.tile([P, 132], F32, tag="hps", bufs=4 - ZTPB)
                    pe.matmul(out=hps[:, 0:BW], lhsT=zt_sb[:], rhs=wl_sb[l][:],
                              start=True, stop=True)
                act.copy(out=T_next[:, w_, 0:BW], in_=hps[:, 0:BW])
            if l == 0:
                return  # layer-1 table ships as the TG0 param; SBUF copy only
            # batched table write: full windows in one DMA, ragged tail apart
            wfull = gsz - (1 if (w0 + gsz) * P > NPC else 0)
            if wfull > 0:
                sync.dma_start(
                    out=T_loc[l][w0 * P:(w0 + wfull) * P, 0:BW]
                        .rearrange("(w p) e -> p w e", p=P),
                    in_=T_next[:, w0:w0 + wfull, 0:BW])
            if wfull < gsz:
                w_ = w0 + wfull
                nr = NPC - w_ * P
                sync.dma_start(out=T_loc[l][w_ * P:w_ * P + nr, 0:BW],
                               in_=T_next[0:nr, w_, 0:BW])

        PT_AHEAD = dims.get("pt_ahead", 64)
        # prebuild the one-hot cache for the first chunks while the layer-0
        # node phase occupies PE/Act
        for g in range(0, PT_AHEAD, 8):
            vec.tensor_tensor(
                out=pt_all[:, g:g + 8, :],
                in0=dstr[:, g:g + 8].unsqueeze(2).to_broadcast([P, 8, BIN]),
                in1=iotab[:].unsqueeze(1).to_broadcast([P, 8, BIN]),
                op=ALU.is_equal)

        # ---- layer 0 node phase (all groups up front)
        T_sb = wp.tile([P, NW, 132], BF16, tag="tsb")
        for g_ in range(NG):
            node_phase_group(0, g_, T_sb, None)

        for l in range(4):
            HW, AW, RW, EL = (LP[l][k] for k in ("HW", "AW", "RW", "EL"))
            BW = HW + AW

            T_next = None
            if l < 3:
                T_next = wp.tile([P, NW, 132], BF16, name="tnext", tag="tsb")

            grp_tiles = {}
            grp_done = set()

            def open_group(g_):
                t = vp.tile([P, WG, 142], F32, name="wingrp", tag="wingrp")
                grp_tiles[g_] = t
                return t

            def epilogue_group(g_):
                w0, gsz = grp_bounds[g_]
                wg = grp_tiles[g_]
                scr = wp.tile([P, WG, 12], F32, name="scr", tag="scr")
                # self-loop alpha (= b_own [+ eterm means]) -> exp
                if l > 0:
                    sl = [None, (0, 4), (4, 8), (8, 9)][l]
                    vec.tensor_tensor(out=scr[:, 0:gsz, 0:AW],
                                      in0=T_sb[:, w0:w0 + gsz, HW:HW + AW],
                                      in1=loop_sb[:, w0:w0 + gsz, sl[0]:sl[1]],
                                      op=ALU.add)
                else:
                    act.copy(out=scr[:, 0:gsz, 0:AW],
                             in_=T_sb[:, w0:w0 + gsz, HW:HW + AW])
                vec.tensor_scalar_mul(out=scr[:, 0:gsz, 4:4 + AW],
                                      in0=scr[:, 0:gsz, 0:AW], scalar1=0.2)
                vec.tensor_tensor(out=scr[:, 0:gsz, 0:AW], in0=scr[:, 0:gsz, 0:AW],
                                  in1=scr[:, 0:gsz, 4:4 + AW], op=ALU.max)
                act.activation(out=scr[:, 0:gsz, 0:AW], in_=scr[:, 0:gsz, 0:AW],
                               func=AF.Exp)
                # num += h_own * ex_loop
                nt = wp.tile([P, WG, 128], BF16, name="nt", tag="nt")
                vec.tensor_tensor(
                    out=nt[:, 0:gsz, 0:HW].rearrange("p g (c a) -> p g c a", a=AW),
                    in0=T_sb[:, w0:w0 + gsz, 0:HW].rearrange("p g (c a) -> p g c a", a=AW),
                    in1=scr[:, 0:gsz, 0:AW].unsqueeze(2)
                        .to_broadcast([P, gsz, HW // AW, AW]),
                    op=ALU.mult)
                vec.tensor_tensor(out=wg[:, 0:gsz, 0:HW], in0=wg[:, 0:gsz, 0:HW],
                                  in1=nt[:, 0:gsz, 0:HW], op=ALU.add)
                # den -> reciprocal
                vec.tensor_tensor(out=scr[:, 0:gsz, 4:4 + AW],
                                  in0=wg[:, 0:gsz, HW:HW + AW],
                                  in1=scr[:, 0:gsz, 0:AW], op=ALU.add)
                vec.tensor_scalar_add(out=scr[:, 0:gsz, 4:4 + AW],
                                      in0=scr[:, 0:gsz, 4:4 + AW], scalar1=1e-16)
                vec.reciprocal(out=scr[:, 0:gsz, 4:4 + AW], in_=scr[:, 0:gsz, 4:4 + AW])
                if l == 0:
                    vec.tensor_scalar_max(out=scr[:, 0:gsz, 8:9],
                                          in0=wg[:, 0:gsz, 141:142], scalar1=1.0)
                    vec.reciprocal(out=scr[:, 0:gsz, 8:9], in_=scr[:, 0:gsz, 8:9])
                    vec.tensor_tensor(
                        out=loop_sb[:, w0:w0 + gsz, 0:9], in0=wg[:, 0:gsz, 132:141],
                        in1=scr[:, 0:gsz, 8:9].to_broadcast([P, gsz, 9]), op=ALU.mult)
                # z = num * recip(den) + bias [+ relu]
                vec.tensor_tensor(
                    out=wg[:, 0:gsz, 0:HW].rearrange("p g (c a) -> p g c a", a=AW),
                    in0=wg[:, 0:gsz, 0:HW].rearrange("p g (c a) -> p g c a", a=AW),
                    in1=scr[:, 0:gsz, 4:4 + AW].unsqueeze(2)
                        .to_broadcast([P, gsz, HW // AW, AW]),
                    op=ALU.mult)
                vec.tensor_tensor(
                    out=wg[:, 0:gsz, 0:HW], in0=wg[:, 0:gsz, 0:HW],
                    in1=bout_t[l][:, 0:HW].unsqueeze(1).to_broadcast([P, gsz, HW]),
                    op=ALU.add)
                if l < 3:
                    act.activation(out=wg[:, 0:gsz, 0:128], in_=wg[:, 0:gsz, 0:128],
                                   func=AF.Relu)
                    node_phase_group(l + 1, g_, T_next, wg)
                else:
                    nonlocal gsum_ps
                    pool_sb = wp.tile([P, WG, 32], BF16, name="pool_sb", tag="poolsb")
                    act.copy(out=pool_sb[:, 0:gsz, 0:32], in_=wg[:, 0:gsz, 0:32])
                    bt = wp.tile([P, WG, Gn], BF16, name="bt", tag="bt", bufs=1)
                    vec.tensor_tensor(
                        out=bt[:, 0:gsz, :],
                        in0=batcht[:, w0:w0 + gsz].unsqueeze(2).to_broadcast([P, gsz, Gn]),
                        in1=iotagf[:].unsqueeze(1).to_broadcast([P, gsz, Gn]),
                        op=ALU.is_equal)
                    if gsum_ps is None:
                        gsum_ps = gsp.tile([32, Gn], F32, name="gsum_ps")
                    for j_ in range(gsz):
                        n_pool_mm[0] += 1
                        pe.matmul(out=gsum_ps[:], lhsT=pool_sb[:, j_, :],
                                  rhs=bt[:, j_, :],
                                  start=(n_pool_mm[0] == 1),
                                  stop=(n_pool_mm[0] == NW))
                grp_done.add(g_)

            cur_bin_tile = {}
            ss_plan = []
            rem_ = C
            while rem_ > 0:
                n_ = min(SS, rem_)
                ss_plan.append(n_)
                rem_ -= n_
            for t_ in dims.get("ss_tail", (8,)):
                if ss_plan[-1] > t_:
                    ss_plan[-1] -= t_
                    ss_plan.append(t_)
            s0 = 0
            for ss, NCH in enumerate(ss_plan):
                Gt = wp.tile([P, SS, EL], BF16, tag="gt", bufs=4)
                gps.dma_gather(
                    out_ap=Gt[:, 0:NCH, :], in_ap=T_glob[l][:, :],
                    idxs_ap=src16[:, s0 * 8:(s0 + NCH) * 8],
                    num_idxs=NCH * CHUNK, num_idxs_reg=NCH * CHUNK, elem_size=EL,
                    single_packet=False, queue_num=ss % 2)
                if l == 0:
                    # edge-term + mask precompute (feeds rhs cols 132:142 +
                    # later layers' alpha); mask folded into EAT row 5.
                    eaT_sl = eap.tile([5, SS * CHUNK], BF16, name="easl", tag="eat")
                    sync.dma_start(
                        out=eaT_sl[:, 0:NCH * CHUNK],
                        in_=pr["EAT"][:, s0 * CHUNK:(s0 + NCH) * CHUNK])
                    for q0 in range(0, NCH, 16):
                        qn = min(16, NCH - q0)
                        etp = pp.tile([P, 160], F32, tag="etp", bufs=1)
                        for j in range(qn):
                            ci = q0 + j
                            pe.matmul(out=etp[:, j * 10:(j + 1) * 10],
                                      lhsT=eaT_sl[:, ci * CHUNK:(ci + 1) * CHUNK],
                                      rhs=w5x10[:], start=True, stop=True)
                        act.copy(out=etc[:, s0 + q0:s0 + q0 + qn, :]
                                 .rearrange("p a b -> p (a b)"),
                                 in_=etp[:, 0:qn * 10])
                    # staircase one-hots built once, reused by all layers;
                    # built PT_AHEAD chunks ahead so the DVE cost sits in the
                    # pipeline's slack instead of its critical phase
                    pb0 = PT_AHEAD + s0
                    pb1 = min(pb0 + NCH, C)
                    for g in range(pb0, pb1, 8):
                        gn = min(8, pb1 - g)
                        vec.tensor_tensor(
                            out=pt_all[:, g:g + gn, :],
                            in0=dstr[:, g:g + gn].unsqueeze(2).to_broadcast([P, gn, BIN]),
                            in1=iotab[:].unsqueeze(1).to_broadcast([P, gn, BIN]),
                            op=ALU.is_equal)
                # alpha = b[src] (+ eterm) -> leaky relu -> exp
                AT = wp.tile([P, SS, 8], BF16, tag="at", bufs=2)
                if l > 0:
                    sl = [None, (0, 4), (4, 8), (8, 9)][l]
                    vec.tensor_tensor(out=AT[:, 0:NCH, 0:AW],
                                      in0=Gt[:, 0:NCH, HW:HW + AW],
                                      in1=etc[:, s0:s0 + NCH, sl[0]:sl[1]],
                                      op=ALU.add)
                    vec.tensor_scalar_mul(out=AT[:, 0:NCH, AW:2 * AW],
                                          in0=AT[:, 0:NCH, 0:AW], scalar1=0.2)
                    vec.tensor_tensor(out=AT[:, 0:NCH, 0:AW], in0=AT[:, 0:NCH, 0:AW],
                                      in1=AT[:, 0:NCH, AW:2 * AW], op=ALU.max)
                else:
                    vec.tensor_scalar_mul(out=AT[:, 0:NCH, AW:2 * AW],
                                          in0=Gt[:, 0:NCH, HW:HW + AW], scalar1=0.2)
                    vec.tensor_tensor(out=AT[:, 0:NCH, 0:AW],
                                      in0=Gt[:, 0:NCH, HW:HW + AW],
                                      in1=AT[:, 0:NCH, AW:2 * AW], op=ALU.max)
                act.activation(out=Gt[:, 0:NCH, HW:HW + AW], in_=AT[:, 0:NCH, 0:AW],
                               func=AF.Exp)
                HX = HW if (l > 0 or not dims.get("hex_split")) else dims["hex_split"]
                vec.tensor_tensor(
                    out=Gt[:, 0:NCH, 0:HX].rearrange("p s (c a) -> p s c a", a=AW),
                    in0=Gt[:, 0:NCH, 0:HX].rearrange("p s (c a) -> p s c a", a=AW),
                    in1=Gt[:, 0:NCH, HW:HW + AW].unsqueeze(2)
                        .to_broadcast([P, NCH, HX // AW, AW]),
                    op=ALU.mult)
                if HX < HW:
                    gps.tensor_tensor(
                        out=Gt[:, 0:NCH, HX:HW].rearrange("p s (c a) -> p s c a", a=AW),
                        in0=Gt[:, 0:NCH, HX:HW].rearrange("p s (c a) -> p s c a", a=AW),
                        in1=Gt[:, 0:NCH, HW:HW + AW].unsqueeze(2)
                            .to_broadcast([P, NCH, (HW - HX) // AW, AW]),
                        op=ALU.mult)
                if l == 0:
                    # append eterm9|cnt as rhs cols 132:142
                    act.copy(out=Gt[:, 0:NCH, 132:142],
                             in_=etc[:, s0:s0 + NCH, :])
                # scatter matmuls
                for c_i in range(NCH):
                    gc = s0 + c_i
                    b = bin_of_chunk[gc]
                    w_ = win_of_bin[b]
                    g_ = grp_of_win[w_]
                    if g_ not in grp_tiles:
                        open_group(g_)
                    if gc == first_chunk_of_bin[b]:
                        cur_bin_tile[b] = bp.tile([BIN, 142], F32, name="binacc",
                                                  tag="binacc")
                    pe.matmul(out=cur_bin_tile[b][:, 0:RW],
                              lhsT=pt_all[:, gc, :], rhs=Gt[:, c_i, 0:RW],
                              start=(gc == first_chunk_of_bin[b]),
                              stop=(gc == last_chunk_of_bin[b]))
                    if gc == last_chunk_of_bin[b]:
                        j = b % BPW
                        wrel = w_ - grp_bounds[g_][0]
                        act.copy(out=grp_tiles[g_][BIN * j:BIN * (j + 1), wrel, 0:RW],
                                 in_=cur_bin_tile[b][:, 0:RW])
                        del cur_bin_tile[b]
                    if gc == last_chunk_of_grp.get(g_, None):
                        epilogue_group(g_)
                s0 += NCH
            # groups never triggered (e.g. all-empty windows)
            for g_ in range(NG):
                if g_ not in grp_done:
                    if g_ not in grp_tiles:
                        open_group(g_)
                    epilogue_group(g_)
            if l < 3:
                if SIM1:
                    sync.dma_start(out=T_glob[l + 1][0:NPC, :], in_=T_loc[l + 1][:, :])
                else:
                    gps.collective_compute(
                        "AllGather", ALU.bypass, replica_groups=[list(range(NCORES))],
                        ins=[T_loc[l + 1][:, :]], outs=[T_glob[l + 1][:, :]])
                T_sb = T_next

        # ============ readout
        gsum_sb = cp.tile([32, Gn], F32)
        act.copy(out=gsum_sb[:], in_=gsum_ps[:])
        sync.dma_start(out=ar_in[:], in_=gsum_sb[:])
        if SIM1:
            sync.dma_start(out=ar_out[:], in_=ar_in[:])
        else:
            gps.collective_compute("AllReduce", ALU.add,
                                   replica_groups=[list(range(NCORES))],
                                   ins=[ar_in[:]], outs=[ar_out[:]])
        gs = cp.tile([32, Gn], F32)
        sync.dma_start(out=gs[:], in_=ar_out[:])
        vec.tensor_tensor(out=comb[0:32, :], in0=gs[:, :], in1=cntrb[:],
                          op=ALU.mult)
        blt = cp.tile([1, 1], F32)
        vec.memset(blt[:], bl)
        fin = pp.tile([1, Gn], F32, tag="hps", bufs=4 - ZTPB)
        pe.matmul(out=fin[:], lhsT=wlin_sb[:], rhs=comb[:], start=True, stop=True)
        res_sb = cp.tile([1, Gn], F32)
        act.activation(out=res_sb[:], in_=fin[:], func=AF.Sigmoid, bias=blt[:])
        sync.dma_start(out=out_p[:, :], in_=res_sb[:])

    nc.finalize()
    return nc


# ------------------------------------------------------------------ entry
def _run(inputs, trace=False, debug=False):
    dims, shared, per_core = host_prep(inputs)
    nc = build_program(dims, shared)
    in_maps = [{**shared, **pc} for pc in per_core]
    from concourse.bass_utils import run_bass_kernel_spmd
    return run_bass_kernel_spmd(nc, in_maps, list(range(NCORES)), trace=trace)


def kernel(**inputs):
    res = _run(inputs)
    return res.results[0]["out"].reshape(-1).astype(np.float32)


# revision 23
# speedup vs baseline: 1.5426x; 1.0955x over previous
"""EnhancedGAT Trainium2 Bass kernel (8 NeuronCores, SPMD).

Strategy:
  - Edges are sorted by destination node on the host; core k owns dst nodes
    [k*N/8, (k+1)*N/8) and every edge targeting them. Per-core edge lists are
    bucketed into 64-node bins and padded to 128-edge chunks with a per-bin
    chunk count shared across cores (SPMD uniformity). Dummy (padding) edges
    carry dst-offset 64, which falls outside the 64-wide one-hot used by the
    scatter matmuls, so they contribute exactly nothing.
  - Each GAT layer:
      node phase: every core computes a table row [h(128) | b(4)] for its own
        nodes, where b = per-head <h, att_s + att_d> comes directly out of the
        h matmul via 4 extra weight columns W @ A. Rows live in a [NPC, 256]
        bf16 DRAM table (512B stride for the gather); an AllGather replicates
        it to every core.
      edge phase: per 4096-edge superstep one dma_gather pulls the rows for
        the edges' sources; attention coefficients alpha = b[src] (+ edge
        term) are leaky-relu'd and exp'd in place, messages h*ex are scattered
        into per-bin PSUM accumulators via one-hot matmuls. Softmax is
        unnormalized (max-subtraction skipped; alphas are O(0.3)); the divide
        happens per node at the group epilogue, where self-loop contributions
        are added. As soon as a window-group's epilogue finishes, the NEXT
        layer's node phase for those windows runs (transpose + matmul + table
        write), hiding the layer boundary behind the remaining gathers.
  - Layer 1 additionally computes, per edge, the folded edge-attention terms
    for layers 2-4 (eterm = ea @ V + be, with the padding mask folded in as a
    fifth all-ones/zeros EAT row) plus the per-edge mask into an [C,10] SBUF
    cache, and accumulates per-node mean edge-feature terms and in-degrees
    (extra scatter-matmul columns) used by the self-loops of layers 2-4.
  - Final graph mean-pool via one-hot matmuls into a [33, G] accumulator,
    AllReduce across cores, tiny dense readout replicated on every core.
"""
import sys
import numpy as np

sys.path.insert(0, "/opt/trn_rl_repo")

HID = 32
NCORES = 8
P = 128
BIN = 64
SS = 32          # chunks per superstep
CHUNK = 128
ROW = 256        # table row elements (bf16) for layers 1-3 (512B stride)
ROW4 = 128       # layer-4 table row elements


# ----------------------------------------------------------------- host prep
def host_prep(inputs):
    x = np.asarray(inputs["x"], np.float32)
    ei = np.asarray(inputs["edge_index"]).astype(np.int64)
    ea = np.asarray(inputs["edge_attr"], np.float32)
    batch = np.asarray(inputs["batch"]).astype(np.int64)
    desc = np.asarray(inputs["descriptors"], np.float32)

    N = x.shape[0]
    E = ei.shape[1]
    Gn = desc.shape[0]
    NPC = N // NCORES
    NW = -(-NPC // P)
    NBINS = -(-NPC // BIN)

    src_all, dst_all = ei[0], ei[1]
    order = np.argsort(dst_all, kind="stable")
    src_s, dst_s = src_all[order], dst_all[order]
    ea_s = ea[order]
    core_of = dst_s // NPC
    local = dst_s - core_of * NPC
    bin_of = local // BIN

    cnt = np.zeros((NCORES, NBINS), np.int64)
    np.add.at(cnt, (core_of, bin_of), 1)
    cpb = np.max(-(-cnt // CHUNK), axis=0)          # chunks per bin (shared)
    cpb = np.maximum(cpb, 1)                        # every bin gets a chunk
    C_total = int(cpb.sum())
    off = np.zeros(NBINS, np.int64)
    off[1:] = np.cumsum(cpb)[:-1]
    EP = C_total * CHUNK                            # padded edges per core

    per_core = []
    for k in range(NCORES):
        srck = np.zeros(EP, np.int64)
        dstrk = np.full(EP, float(BIN), np.float32)  # dummies -> dead one-hot
        maskk = np.zeros(EP, np.float32)
        eak = np.zeros((EP, 4), np.float32)
        sel = core_of == k
        bins_k = bin_of[sel]
        start = np.searchsorted(bins_k, np.arange(NBINS))
        pos = np.arange(bins_k.size) - start[bins_k]
        slot = off[bins_k] * CHUNK + pos
        srck[slot] = src_s[sel]
        dstrk[slot] = (local[sel] - bins_k * BIN).astype(np.float32)
        maskk[slot] = 1.0
        eak[slot] = ea_s[sel]

        # device layouts: edge e = c*128 + p
        src16 = np.tile(srck.reshape(-1, 16).T.astype(np.int16), (8, 1))
        dstr_d = dstrk.reshape(C_total, P).T.copy()
        import ml_dtypes
        ea5 = np.concatenate([eak.T, maskk[None, :]], axis=0).astype(ml_dtypes.bfloat16)

        xk = x[k * NPC:(k + 1) * NPC]
        xT = np.zeros((8, NW * P), np.float32)
        xT[:, :NPC] = xk.T
        bk = np.full(NW * P, Gn + 5, np.float32)
        bk[:NPC] = batch[k * NPC:(k + 1) * NPC].astype(np.float32)
        batch_d = bk.reshape(NW, P).T.copy()

        per_core.append(dict(SRC16=src16, DSTR=dstr_d, EAT=ea5,
                             XT=xT, BATCH=batch_d))

    # ---- weight folding
    w = {k: np.asarray(v, np.float32) for k, v in inputs.items()
         if k not in ("x", "edge_index", "edge_attr", "batch", "descriptors")}

    def vfold(We, ae, heads):
        Vp = (We.reshape(w["We_enc"].shape[1], heads, HID) * ae[None]).sum(-1)
        return w["We_enc"] @ Vp, w["be_enc"] @ Vp      # [4,heads],[heads]

    V2, bv2 = vfold(w["We2"], w["ae2"], 4)
    V3, bv3 = vfold(w["We3"], w["ae3"], 4)
    V4, bv4 = vfold(w["We4"], w["ae4"], 1)
    # [5,10]: rows = 4 edge-attr dims + mask; cols = 9 eterms + cnt
    W5x10 = np.zeros((5, 10), np.float32)
    W5x10[0:4, 0:9] = np.concatenate([V2, V3, V4], axis=1)
    W5x10[4, 0:9] = np.concatenate([bv2, bv3, bv4])
    W5x10[4, 9] = 1.0

    def padr(v, n):
        o = np.zeros(n, np.float32)
        o[: v.size] = v
        return o

    # channel-major reorder of the 128-wide (4 heads x 32 ch) dimension:
    # new position c*4+a holds old a*32+c. Keeps per-head broadcasts
    # innermost-packed on DVE (2x mode).
    cm = (np.arange(128) % 4) * 32 + np.arange(128) // 4

    def wext(W, att_s, att_d, heads):
        # append per-head b-columns: b_a = h . (att_s+att_d)_a
        att = (att_s + att_d).reshape(-1)  # [heads*HID] head-major
        if heads == 4:
            attc = att[cm]                 # channel-major to match W cols
            A = np.zeros((128, 4), np.float32)
            A[np.arange(128), np.arange(128) % 4] = attc
        else:
            A = att[:, None]               # [32,1]
        return np.concatenate([W, W @ A], axis=1)

    W1e = wext(w["W1"][:, cm], w["as1"], w["ad1"], 4)            # [8,132]
    W2e = wext(w["W2"][cm][:, cm], w["as2"], w["ad2"], 4)        # [128,132]
    W3e = wext(w["W3"][cm][:, cm], w["as3"], w["ad3"], 4)
    W4e = wext(w["W4"][cm], w["as4"], w["ad4"], 1)               # [128,33]

    bout = np.stack([padr(w["b1"][cm], 128), padr(w["b2"][cm], 128),
                     padr(w["b3"][cm], 128), padr(w["b4"], 128)])

    import ml_dtypes
    T0 = np.zeros((N, ROW), np.float32)
    T0[:, 0:132] = x @ W1e
    TG0 = T0.astype(ml_dtypes.bfloat16)

    gcnt = np.bincount(batch, minlength=Gn).astype(np.float32)
    cntr = (1.0 / np.maximum(gcnt, 1.0))[None, :]           # [1, Gn]
    shared = dict(
        W1=W1e, WL2=W2e, WL3=W3e, WL4=W4e, TG0=TG0,
        W5X10=W5x10, BOUT=bout, CNTR=cntr,
        WD=w["Wd"], BD=w["bd"][:, None], WLIN=w["Wl"], DESCT=desc.T.copy(),
    )
    bl = float(np.asarray(w["bl"]).reshape(-1)[0])

    dims = dict(N=N, E=E, Gn=Gn, NPC=NPC, NW=NW, NBINS=NBINS,
                C=C_total, cpb=cpb, off=off, bl=bl)
    return dims, shared, per_core


# ------------------------------------------------------------- program build
def build_program(dims, shared):
    import concourse.bass as bass
    import concourse.mybir as mybir
    import concourse.tile as tile
    import concourse.bacc as bacc
    from concourse.masks import make_identity
    from contextlib import ExitStack

    F32 = mybir.dt.float32
    BF16 = mybir.dt.bfloat16
    I32 = mybir.dt.int32
    I16 = mybir.dt.int16
    AF = mybir.ActivationFunctionType
    ALU = mybir.AluOpType
    AX = mybir.AxisListType

    N, Gn, NPC, NW, NBINS, C = (dims[k] for k in ("N", "Gn", "NPC", "NW", "NBINS", "C"))
    cpb, off, bl = dims["cpb"], dims["off"], dims["bl"]
    NSS = C // SS
    # layer params: h width, heads, rhs width, gather row elems
    LP = [dict(HW=128, AW=4, RW=142, EL=ROW, GEL=ROW),  # L1 rhs: h,ex,eterm9,cnt
          dict(HW=128, AW=4, RW=132, EL=ROW, GEL=ROW),
          dict(HW=128, AW=4, RW=132, EL=ROW, GEL=ROW),
          dict(HW=32, AW=1, RW=33, EL=ROW4, GEL=34)]

    nc = bacc.Bacc(num_swdge_queues=2)
    SIM1 = dims.get("sim1", False)

    # ---- params
    pr = {}
    for nm, shp, dt in [("SRC16", [P, C * 8], I16), ("DSTR", [P, C], F32),
                        ("EAT", [5, C * CHUNK], BF16), ("XT", [8, NW * P], F32),
                        ("BATCH", [P, NW], F32), ("W1", [8, 132], F32),
                        ("WL2", [128, 132], F32), ("WL3", [128, 132], F32),
                        ("WL4", [128, 33], F32), ("W5X10", [5, 10], F32),
                        ("BOUT", [4, 128], F32),
                        ("WD", [48, 32], F32), ("BD", [32, 1], F32),
                        ("WLIN", [64, 1], F32), ("DESCT", [48, Gn], F32),
                        ("CNTR", [1, Gn], F32), ("TG0", [N, ROW], BF16)]:
        pr[nm] = nc.declare_dram_parameter(nm, shp, dt, isOutput=False)
    out_p = nc.declare_dram_parameter("out", [1, Gn], F32, isOutput=True)
    pr_TG0_ph = pr["TG0"]

    # ---- internal DRAM
    T_loc = [None] + [nc.dram_tensor(f"T_loc{l}", [NPC, LP[l]["EL"]], BF16)
                      for l in range(1, 4)]
    T_glob = [pr_TG0_ph] + [nc.dram_tensor(f"T_glob{l}", [N, LP[l]["EL"]], BF16,
                                           addr_space="Shared")
                            for l in range(1, 4)]
    ar_in = nc.dram_tensor("ar_in", [32, Gn], F32)
    ar_out = nc.dram_tensor("ar_out", [32, Gn], F32, addr_space="Shared")

    # bin/window bookkeeping (compile-time)
    bin_of_chunk = []
    for b in range(NBINS):
        bin_of_chunk += [b] * int(cpb[b])
    BPW = P // BIN  # bins per window
    win_of_bin = [b // BPW for b in range(NBINS)]
    last_chunk_of_bin = {}
    first_chunk_of_bin = {}
    for c_i, b in enumerate(bin_of_chunk):
        last_chunk_of_bin[b] = c_i
        first_chunk_of_bin.setdefault(b, c_i)

    with tile.TileContext(nc) as tc, ExitStack() as ctx:
        cp = ctx.enter_context(tc.tile_pool(name="const", bufs=1))
        wp = ctx.enter_context(tc.tile_pool(name="work", bufs=2))
        vp = ctx.enter_context(tc.tile_pool(name="win", bufs=2))
        pp = ctx.enter_context(tc.tile_pool(name="psum", bufs=2, space="PSUM"))
        bp = ctx.enter_context(tc.tile_pool(name="binp", bufs=2, space="PSUM"))

        sync, gps, vec, act, pe = nc.sync, nc.gpsimd, nc.vector, nc.scalar, nc.tensor

        def dma_gather_short(out_ap, in_ap, idxs_ap, num_idxs, elem_size,
                             elem_step, queue_num):
            from concourse.bass import exact_div
            eng = gps
            _in_ap = eng.lower_ap_dma(in_ap, for_custom_bir_dma=True)
            _idxs_ap = eng.lower_ap(idxs_ap)
            _out_ap = eng.lower_ap(out_ap)
            stride_bytes_256 = exact_div(elem_step * 2, 256)
            return eng.add_instruction(
                mybir.InstDMAGatherAnt(
                    name=eng.bass.get_next_instruction_name(),
                    ins=[*_in_ap, _idxs_ap,
                         eng.lower_val_access(eng.to_reg(num_idxs))],
                    outs=[_out_ap],
                    transpose=False, num_idxs=num_idxs, elem_size=elem_size,
                    stride_bytes_256=stride_bytes_256, gen_mode=0,
                    single_packet=False, queue_num=queue_num,
                    sbuf_tokens_per_rank=0, sbuf_free_dim_per_rank=0,
                    sbuf_free_dim_pad_per_rank=0, sbuf_byte_offset=0))
        ZTPB = dims.get("ztpb", 1)

        # ---- resident tiles
        src16 = cp.tile([P, C * 8], I16)
        sync.dma_start(out=src16[:], in_=pr["SRC16"][:, :])
        dstr = cp.tile([P, C], BF16)
        gps.dma_start(out=dstr[:], in_=pr["DSTR"][:, :])   # f32 -> bf16 cast
        batcht = cp.tile([P, NW], F32)
        sync.dma_start(out=batcht[:], in_=pr["BATCH"][:, :])
        xT_sb = cp.tile([8, NW * P], F32)
        sync.dma_start(out=xT_sb[:], in_=pr["XT"][:, :])

        iota_i = cp.tile([P, BIN], I32)
        gps.iota(iota_i[:], pattern=[[1, BIN]], base=0, channel_multiplier=0)
        iotab = cp.tile([P, BIN], BF16)
        vec.tensor_copy(iotab[:], iota_i[:])
        iotag_i = cp.tile([P, Gn], I32)
        gps.iota(iotag_i[:], pattern=[[1, Gn]], base=0, channel_multiplier=0)
        iotagf = cp.tile([P, Gn], F32)
        vec.tensor_copy(iotagf[:], iotag_i[:])
        identf = cp.tile([P, P], F32)
        make_identity(nc, identf[:])

        w1_sb = cp.tile([8, 132], F32)
        sync.dma_start(out=w1_sb[:], in_=pr["W1"][:, :])
        wl_sb = [None,
                 cp.tile([128, 132], BF16, name="wl2", tag="wl2"),
                 cp.tile([128, 132], BF16, name="wl3", tag="wl3"),
                 cp.tile([128, 33], BF16, name="wl4", tag="wl4")]
        gps.dma_start(out=wl_sb[1][:], in_=pr["WL2"][:, :])   # gpsimd casts f32->bf16
        gps.dma_start(out=wl_sb[2][:], in_=pr["WL3"][:, :])
        gps.dma_start(out=wl_sb[3][:], in_=pr["WL4"][:, :])
        w5x10 = cp.tile([5, 10], BF16)
        gps.dma_start(out=w5x10[:], in_=pr["W5X10"][:, :])
        bout_t = []
        for l in range(4):
            t3 = cp.tile([P, 128], F32, tag=f"bout{l}")
            sync.dma_start(out=t3[:], in_=pr["BOUT"][l:l + 1, :].to_broadcast([P, 128]))
            bout_t.append(t3)

        etc = cp.tile([P, C, 10], BF16)      # eterm9 | cnt  per edge
        pt_all = cp.tile([P, C, BIN], BF16)  # one-hot dst rows per edge
        loop_sb = cp.tile([P, NW, 10], F32)
        gsp = ctx.enter_context(tc.tile_pool(name="gsp", bufs=1, space="PSUM"))
        eap = ctx.enter_context(tc.tile_pool(name="eap", bufs=1))
        gsum_ps = None
        n_pool_mm = [0]

        # ---- readout head start: descriptor branch is input-independent
        comb = cp.tile([64, Gn], F32)
        wd_sb = cp.tile([48, 32], F32)
        sync.dma_start(out=wd_sb[:], in_=pr["WD"][:, :])
        desct_sb = cp.tile([48, Gn], F32)
        sync.dma_start(out=desct_sb[:], in_=pr["DESCT"][:, :])
        bd_sb = cp.tile([32, 1], F32)
        sync.dma_start(out=bd_sb[:], in_=pr["BD"][:, :])
        dps = pp.tile([32, Gn], F32, tag="hps", bufs=4 - ZTPB)
        pe.matmul(out=dps[:], lhsT=wd_sb[:], rhs=desct_sb[:], start=True, stop=True)
        act.activation(out=comb[32:64, :], in_=dps[:], func=AF.Relu, bias=bd_sb[:])
        wlin_sb = cp.tile([64, 1], F32)
        sync.dma_start(out=wlin_sb[:], in_=pr["WLIN"][:, :])
        cntrb = cp.tile([32, Gn], F32)
        sync.dma_start(out=cntrb[:], in_=pr["CNTR"][0:1, :].to_broadcast([32, Gn]))

        WG = dims.get("wg", 5)  # max windows per epilogue group
        # non-uniform groups: taper toward the end so the serial layer-boundary
        # tail (last epilogue -> node phase -> AllGather) shrinks
        grp_bounds = []
        w0_ = 0
        while NW - w0_ > 10:
            grp_bounds.append((w0_, WG))
            w0_ += WG
        for t_ in dims.get("taper", (4, 3, 2, 1)):
            if NW - w0_ > t_:
                grp_bounds.append((w0_, t_))
                w0_ += t_
        if NW > w0_:
            grp_bounds.append((w0_, NW - w0_))
        NG = len(grp_bounds)
        grp_of_win = {}
        for gi, (gw0, gsz_) in enumerate(grp_bounds):
            for w_ in range(gw0, gw0 + gsz_):
                grp_of_win[w_] = gi
        last_chunk_of_grp = {}
        for b in range(NBINS):
            g_ = grp_of_win[win_of_bin[b]]
            last_chunk_of_grp[g_] = max(last_chunk_of_grp.get(g_, -1),
                                        last_chunk_of_bin[b])

        # T_sb tables: [P, NW, 132] (h | b); layer l+1's is built during
        # layer l's edge phase, group by group.
        def node_phase_group(l, g_, T_next, z_src):
            """Build T_next rows for group g_ of layer l (0-based), write T_loc."""
            w0, gsz = grp_bounds[g_]
            HWn = LP[l]["HW"]
            BW = HWn + LP[l]["AW"]  # table row width
            for w_ in range(w0, w0 + gsz):
                if l == 0:
                    hps = pp.tile([P, 132], F32, tag="hps", bufs=4 - ZTPB)
                    pe.matmul(out=hps[:, 0:BW], lhsT=xT_sb[:, w_ * P:(w_ + 1) * P],
                              rhs=w1_sb[:], start=True, stop=True)
                else:
                    ztp = pp.tile([P, P], F32, tag="ztp", bufs=ZTPB)
                    pe.transpose(out=ztp[:], in_=z_src[:, w_ - w0, 0:128],
                                 identity=identf[:])
                    zt_sb = wp.tile([P, P], BF16, tag="ztsb")
                    act.copy(out=zt_sb[:], in_=ztp[:])
                    hps = pp.tile([P, 132], F32, tag="hps", bufs=4 - ZTPB)
                    pe.matmul(out=hps[:, 0:BW], lhsT=zt_sb[:], rhs=wl_sb[l][:],
                              start=True, stop=True)
                act.copy(out=T_next[:, w_, 0:BW], in_=hps[:, 0:BW])
            if l == 0:
                return  # layer-1 table ships as the TG0 param; SBUF copy only
            # batched table write: full windows in one DMA, ragged tail apart
            wfull = gsz - (1 if (w0 + gsz) * P > NPC else 0)
            if wfull > 0:
                sync.dma_start(
                    out=T_loc[l][w0 * P:(w0 + wfull) * P, 0:BW]
                        .rearrange("(w p) e -> p w e", p=P),
                    in_=T_next[:, w0:w0 + wfull, 0:BW])
            if wfull < gsz:
                w_ = w0 + wfull
                nr = NPC - w_ * P
                sync.dma_start(out=T_loc[l][w_ * P:w_ * P + nr, 0:BW],
                               in_=T_next[0:nr, w_, 0:BW])

        PT_AHEAD = dims.get("pt_ahead", 64)
        # prebuild the one-hot cache for the first chunks while the layer-0
        # node phase occupies PE/Act
        for g in range(0, PT_AHEAD, 8):
            vec.tensor_tensor(
                out=pt_all[:, g:g + 8, :],
                in0=dstr[:, g:g + 8].unsqueeze(2).to_broadcast([P, 8, BIN]),
                in1=iotab[:].unsqueeze(1).to_broadcast([P, 8, BIN]),
                op=ALU.is_equal)

        # ---- layer 0 node phase (all groups up front)
        T_sb = wp.tile([P, NW, 132], BF16, tag="tsb")
        for g_ in range(NG):
            node_phase_group(0, g_, T_sb, None)

        for l in range(4):
            HW, AW, RW, EL, GEL = (LP[l][k] for k in ("HW", "AW", "RW", "EL", "GEL"))
            BW = HW + AW

            T_next = None
            if l < 3:
                T_next = wp.tile([P, NW, 132], BF16, name="tnext", tag="tsb")

            grp_tiles = {}
            grp_done = set()

            def open_group(g_):
                t = vp.tile([P, WG, 142], F32, name="wingrp", tag="wingrp")
                grp_tiles[g_] = t
                return t

            def epilogue_group(g_):
                w0, gsz = grp_bounds[g_]
                wg = grp_tiles[g_]
                scr = wp.tile([P, WG, 12], F32, name="scr", tag="scr")
                # self-loop alpha (= b_own [+ eterm means]) -> exp
                if l > 0:
                    sl = [None, (0, 4), (4, 8), (8, 9)][l]
                    vec.tensor_tensor(out=scr[:, 0:gsz, 0:AW],
                                      in0=T_sb[:, w0:w0 + gsz, HW:HW + AW],
                                      in1=loop_sb[:, w0:w0 + gsz, sl[0]:sl[1]],
                                      op=ALU.add)
                else:
                    act.copy(out=scr[:, 0:gsz, 0:AW],
                             in_=T_sb[:, w0:w0 + gsz, HW:HW + AW])
                vec.tensor_scalar_mul(out=scr[:, 0:gsz, 4:4 + AW],
                                      in0=scr[:, 0:gsz, 0:AW], scalar1=0.2)
                vec.tensor_tensor(out=scr[:, 0:gsz, 0:AW], in0=scr[:, 0:gsz, 0:AW],
                                  in1=scr[:, 0:gsz, 4:4 + AW], op=ALU.max)
                act.activation(out=scr[:, 0:gsz, 0:AW], in_=scr[:, 0:gsz, 0:AW],
                               func=AF.Exp)
                # num += h_own * ex_loop
                nt = wp.tile([P, WG, 128], BF16, name="nt", tag="nt")
                vec.tensor_tensor(
                    out=nt[:, 0:gsz, 0:HW].rearrange("p g (c a) -> p g c a", a=AW),
                    in0=T_sb[:, w0:w0 + gsz, 0:HW].rearrange("p g (c a) -> p g c a", a=AW),
                    in1=scr[:, 0:gsz, 0:AW].unsqueeze(2)
                        .to_broadcast([P, gsz, HW // AW, AW]),
                    op=ALU.mult)
                vec.tensor_tensor(out=wg[:, 0:gsz, 0:HW], in0=wg[:, 0:gsz, 0:HW],
                                  in1=nt[:, 0:gsz, 0:HW], op=ALU.add)
                # den -> reciprocal
                vec.tensor_tensor(out=scr[:, 0:gsz, 4:4 + AW],
                                  in0=wg[:, 0:gsz, HW:HW + AW],
                                  in1=scr[:, 0:gsz, 0:AW], op=ALU.add)
                vec.tensor_scalar_add(out=scr[:, 0:gsz, 4:4 + AW],
                                      in0=scr[:, 0:gsz, 4:4 + AW], scalar1=1e-16)
                vec.reciprocal(out=scr[:, 0:gsz, 4:4 + AW], in_=scr[:, 0:gsz, 4:4 + AW])
                if l == 0:
                    vec.tensor_scalar_max(out=scr[:, 0:gsz, 8:9],
                                          in0=wg[:, 0:gsz, 141:142], scalar1=1.0)
                    vec.reciprocal(out=scr[:, 0:gsz, 8:9], in_=scr[:, 0:gsz, 8:9])
                    vec.tensor_tensor(
                        out=loop_sb[:, w0:w0 + gsz, 0:9], in0=wg[:, 0:gsz, 132:141],
                        in1=scr[:, 0:gsz, 8:9].to_broadcast([P, gsz, 9]), op=ALU.mult)
                # z = num * recip(den) + bias [+ relu]
                vec.tensor_tensor(
                    out=wg[:, 0:gsz, 0:HW].rearrange("p g (c a) -> p g c a", a=AW),
                    in0=wg[:, 0:gsz, 0:HW].rearrange("p g (c a) -> p g c a", a=AW),
                    in1=scr[:, 0:gsz, 4:4 + AW].unsqueeze(2)
                        .to_broadcast([P, gsz, HW // AW, AW]),
                    op=ALU.mult)
                vec.tensor_tensor(
                    out=wg[:, 0:gsz, 0:HW], in0=wg[:, 0:gsz, 0:HW],
                    in1=bout_t[l][:, 0:HW].unsqueeze(1).to_broadcast([P, gsz, HW]),
                    op=ALU.add)
                if l < 3:
                    act.activation(out=wg[:, 0:gsz, 0:128], in_=wg[:, 0:gsz, 0:128],
                                   func=AF.Relu)
                    node_phase_group(l + 1, g_, T_next, wg)
                else:
                    nonlocal gsum_ps
                    pool_sb = wp.tile([P, WG, 32], BF16, name="pool_sb", tag="poolsb")
                    act.copy(out=pool_sb[:, 0:gsz, 0:32], in_=wg[:, 0:gsz, 0:32])
                    bt = wp.tile([P, WG, Gn], BF16, name="bt", tag="bt", bufs=1)
                    vec.tensor_tensor(
                        out=bt[:, 0:gsz, :],
                        in0=batcht[:, w0:w0 + gsz].unsqueeze(2).to_broadcast([P, gsz, Gn]),
                        in1=iotagf[:].unsqueeze(1).to_broadcast([P, gsz, Gn]),
                        op=ALU.is_equal)
                    if gsum_ps is None:
                        gsum_ps = gsp.tile([32, Gn], F32, name="gsum_ps")
                    for j_ in range(gsz):
                        n_pool_mm[0] += 1
                        pe.matmul(out=gsum_ps[:], lhsT=pool_sb[:, j_, :],
                                  rhs=bt[:, j_, :],
                                  start=(n_pool_mm[0] == 1),
                                  stop=(n_pool_mm[0] == NW))
                grp_done.add(g_)

            cur_bin_tile = {}
            ss_plan = []
            rem_ = C
            while rem_ > 0:
                n_ = min(SS, rem_)
                ss_plan.append(n_)
                rem_ -= n_
            for t_ in dims.get("ss_tail", (8,)):
                if ss_plan[-1] > t_:
                    ss_plan[-1] -= t_
                    ss_plan.append(t_)
            s0 = 0
            for ss, NCH in enumerate(ss_plan):
                Gt = wp.tile([P, SS, GEL], BF16, tag="gt", bufs=4)
                if GEL == EL:
                    gps.dma_gather(
                        out_ap=Gt[:, 0:NCH, :], in_ap=T_glob[l][:, :],
                        idxs_ap=src16[:, s0 * 8:(s0 + NCH) * 8],
                        num_idxs=NCH * CHUNK, num_idxs_reg=NCH * CHUNK,
                        elem_size=EL, single_packet=False, queue_num=ss % 2)
                else:
                    dma_gather_short(
                        out_ap=Gt[:, 0:NCH, :], in_ap=T_glob[l][:, 0:GEL],
                        idxs_ap=src16[:, s0 * 8:(s0 + NCH) * 8],
                        num_idxs=NCH * CHUNK, elem_size=GEL, elem_step=EL,
                        queue_num=ss % 2)
                if l == 0:
                    # edge-term + mask precompute (feeds rhs cols 132:142 +
                    # later layers' alpha); mask folded into EAT row 5.
                    eaT_sl = eap.tile([5, SS * CHUNK], BF16, name="easl", tag="eat")
                    sync.dma_start(
                        out=eaT_sl[:, 0:NCH * CHUNK],
                        in_=pr["EAT"][:, s0 * CHUNK:(s0 + NCH) * CHUNK])
                    for q0 in range(0, NCH, 16):
                        qn = min(16, NCH - q0)
                        etp = pp.tile([P, 160], F32, tag="etp", bufs=1)
                        for j in range(qn):
                            ci = q0 + j
                            pe.matmul(out=etp[:, j * 10:(j + 1) * 10],
                                      lhsT=eaT_sl[:, ci * CHUNK:(ci + 1) * CHUNK],
                                      rhs=w5x10[:], start=True, stop=True)
                        act.copy(out=etc[:, s0 + q0:s0 + q0 + qn, :]
                                 .rearrange("p a b -> p (a b)"),
                                 in_=etp[:, 0:qn * 10])
                    # staircase one-hots built once, reused by all layers;
                    # built PT_AHEAD chunks ahead so the DVE cost sits in the
                    # pipeline's slack instead of its critical phase
                    pb0 = PT_AHEAD + s0
                    pb1 = min(pb0 + NCH, C)
                    for g in range(pb0, pb1, 8):
                        gn = min(8, pb1 - g)
                        vec.tensor_tensor(
                            out=pt_all[:, g:g + gn, :],
                            in0=dstr[:, g:g + gn].unsqueeze(2).to_broadcast([P, gn, BIN]),
                            in1=iotab[:].unsqueeze(1).to_broadcast([P, gn, BIN]),
                            op=ALU.is_equal)
                # alpha = b[src] (+ eterm) -> leaky relu -> exp
                AT = wp.tile([P, SS, 8], BF16, tag="at", bufs=2)
                if l > 0:
                    sl = [None, (0, 4), (4, 8), (8, 9)][l]
                    vec.tensor_tensor(out=AT[:, 0:NCH, 0:AW],
                                      in0=Gt[:, 0:NCH, HW:HW + AW],
                                      in1=etc[:, s0:s0 + NCH, sl[0]:sl[1]],
                                      op=ALU.add)
                    vec.tensor_scalar_mul(out=AT[:, 0:NCH, AW:2 * AW],
                                          in0=AT[:, 0:NCH, 0:AW], scalar1=0.2)
                    vec.tensor_tensor(out=AT[:, 0:NCH, 0:AW], in0=AT[:, 0:NCH, 0:AW],
                                      in1=AT[:, 0:NCH, AW:2 * AW], op=ALU.max)
                else:
                    vec.tensor_scalar_mul(out=AT[:, 0:NCH, AW:2 * AW],
                                          in0=Gt[:, 0:NCH, HW:HW + AW], scalar1=0.2)
                    vec.tensor_tensor(out=AT[:, 0:NCH, 0:AW],
                                      in0=Gt[:, 0:NCH, HW:HW + AW],
                                      in1=AT[:, 0:NCH, AW:2 * AW], op=ALU.max)
                act.activation(out=Gt[:, 0:NCH, HW:HW + AW], in_=AT[:, 0:NCH, 0:AW],
                               func=AF.Exp)
                HX = HW if (l > 0 or not dims.get("hex_split")) else dims["hex_split"]
                vec.tensor_tensor(
                    out=Gt[:, 0:NCH, 0:HX].rearrange("p s (c a) -> p s c a", a=AW),
                    in0=Gt[:, 0:NCH, 0:HX].rearrange("p s (c a) -> p s c a", a=AW),
                    in1=Gt[:, 0:NCH, HW:HW + AW].unsqueeze(2)
                        .to_broadcast([P, NCH, HX // AW, AW]),
                    op=ALU.mult)
                if HX < HW:
                    gps.tensor_tensor(
                        out=Gt[:, 0:NCH, HX:HW].rearrange("p s (c a) -> p s c a", a=AW),
                        in0=Gt[:, 0:NCH, HX:HW].rearrange("p s (c a) -> p s c a", a=AW),
                        in1=Gt[:, 0:NCH, HW:HW + AW].unsqueeze(2)
                            .to_broadcast([P, NCH, (HW - HX) // AW, AW]),
                        op=ALU.mult)
                if l == 0:
                    # append eterm9|cnt as rhs cols 132:142
                    act.copy(out=Gt[:, 0:NCH, 132:142],
                             in_=etc[:, s0:s0 + NCH, :])
                # scatter matmuls
                for c_i in range(NCH):
                    gc = s0 + c_i
                    b = bin_of_chunk[gc]
                    w_ = win_of_bin[b]
                    g_ = grp_of_win[w_]
                    if g_ not in grp_tiles:
                        open_group(g_)
                    if gc == first_chunk_of_bin[b]:
                        cur_bin_tile[b] = bp.tile([BIN, 142], F32, name="binacc",
                                                  tag="binacc")
                    pe.matmul(out=cur_bin_tile[b][:, 0:RW],
                              lhsT=pt_all[:, gc, :], rhs=Gt[:, c_i, 0:RW],
                              start=(gc == first_chunk_of_bin[b]),
                              stop=(gc == last_chunk_of_bin[b]))
                    if gc == last_chunk_of_bin[b]:
                        j = b % BPW
                        wrel = w_ - grp_bounds[g_][0]
                        act.copy(out=grp_tiles[g_][BIN * j:BIN * (j + 1), wrel, 0:RW],
                                 in_=cur_bin_tile[b][:, 0:RW])
                        del cur_bin_tile[b]
                    if gc == last_chunk_of_grp.get(g_, None):
                        epilogue_group(g_)
                s0 += NCH
            # groups never triggered (e.g. all-empty windows)
            for g_ in range(NG):
                if g_ not in grp_done:
                    if g_ not in grp_tiles:
                        open_group(g_)
                    epilogue_group(g_)
            if l < 3:
                if SIM1:
                    sync.dma_start(out=T_glob[l + 1][0:NPC, :], in_=T_loc[l + 1][:, :])
                else:
                    gps.collective_compute(
                        "AllGather", ALU.bypass, replica_groups=[list(range(NCORES))],
                        ins=[T_loc[l + 1][:, :]], outs=[T_glob[l + 1][:, :]])
                T_sb = T_next

        # ============ readout
        gsum_sb = cp.tile([32, Gn], F32)
        act.copy(out=gsum_sb[:], in_=gsum_ps[:])
        sync.dma_start(out=ar_in[:], in_=gsum_sb[:])
        if SIM1:
            sync.dma_start(out=ar_out[:], in_=ar_in[:])
        else:
            gps.collective_compute("AllReduce", ALU.add,
                                   replica_groups=[list(range(NCORES))],
                                   ins=[ar_in[:]], outs=[ar_out[:]])
        gs = cp.tile([32, Gn], F32)
        sync.dma_start(out=gs[:], in_=ar_out[:])
        vec.tensor_tensor(out=comb[0:32, :], in0=gs[:, :], in1=cntrb[:],
                          op=ALU.mult)
        blt = cp.tile([1, 1], F32)
        vec.memset(blt[:], bl)
        fin = pp.tile([1, Gn], F32, tag="hps", bufs=4 - ZTPB)
        pe.matmul(out=fin[:], lhsT=wlin_sb[:], rhs=comb[:], start=True, stop=True)
        res_sb = cp.tile([1, Gn], F32)
        act.activation(out=res_sb[:], in_=fin[:], func=AF.Sigmoid, bias=blt[:])
        sync.dma_start(out=out_p[:, :], in_=res_sb[:])

    nc.finalize()
    return nc


# ------------------------------------------------------------------ entry
def _run(inputs, trace=False, debug=False):
    dims, shared, per_core = host_prep(inputs)
    nc = build_program(dims, shared)
    in_maps = [{**shared, **pc} for pc in per_core]
    from concourse.bass_utils import run_bass_kernel_spmd
    return run_bass_kernel_spmd(nc, in_maps, list(range(NCORES)), trace=trace)


def kernel(**inputs):
    res = _run(inputs)
    return res.results[0]["out"].reshape(-1).astype(np.float32)


# revision 32
# speedup vs baseline: 1.7075x; 1.1068x over previous
"""EnhancedGAT Trainium2 Bass kernel (8 NeuronCores, SPMD).

Strategy:
  - Edges are sorted by destination node on the host; core k owns dst nodes
    [k*N/8, (k+1)*N/8) and every edge targeting them. Per-core edge lists are
    bucketed into 64-node bins and padded to 128-edge chunks with a per-bin
    chunk count shared across cores (SPMD uniformity). Dummy (padding) edges
    carry dst-offset 64, which falls outside the 64-wide one-hot used by the
    scatter matmuls, so they contribute exactly nothing.
  - Each GAT layer:
      node phase: every core computes a table row [h(128) | b(4)] for its own
        nodes, where b = per-head <h, att_s + att_d> comes directly out of the
        h matmul via 4 extra weight columns W @ A. Rows live in a [NPC, 256]
        bf16 DRAM table (512B stride for the gather); an AllGather replicates
        it to every core.
      edge phase: per 4096-edge superstep one dma_gather pulls the rows for
        the edges' sources; attention coefficients alpha = b[src] (+ edge
        term) are leaky-relu'd and exp'd in place, messages h*ex are scattered
        into per-bin PSUM accumulators via one-hot matmuls. Softmax is
        unnormalized (max-subtraction skipped; alphas are O(0.3)); the divide
        happens per node at the group epilogue, where self-loop contributions
        are added. As soon as a window-group's epilogue finishes, the NEXT
        layer's node phase for those windows runs (transpose + matmul + table
        write), hiding the layer boundary behind the remaining gathers.
  - Layer 1 additionally computes, per edge, the folded edge-attention terms
    for layers 2-4 (eterm = ea @ V + be, with the padding mask folded in as a
    fifth all-ones/zeros EAT row) plus the per-edge mask into an [C,10] SBUF
    cache, and accumulates per-node mean edge-feature terms and in-degrees
    (extra scatter-matmul columns) used by the self-loops of layers 2-4.
  - Final graph mean-pool via one-hot matmuls into a [33, G] accumulator,
    AllReduce across cores, tiny dense readout replicated on every core.
"""
import sys
import numpy as np

sys.path.insert(0, "/opt/trn_rl_repo")

HID = 32
NCORES = 8
P = 128
BIN = 64
SS = 32          # chunks per superstep
CHUNK = 128
ROW = 256        # table row elements (bf16) for layers 1-3 (512B stride)
ROW4 = 128       # layer-4 table row elements


# ----------------------------------------------------------------- host prep
def host_prep(inputs):
    x = np.asarray(inputs["x"], np.float32)
    ei = np.asarray(inputs["edge_index"]).astype(np.int64)
    ea = np.asarray(inputs["edge_attr"], np.float32)
    batch = np.asarray(inputs["batch"]).astype(np.int64)
    desc = np.asarray(inputs["descriptors"], np.float32)

    N = x.shape[0]
    E = ei.shape[1]
    Gn = desc.shape[0]
    NPC = N // NCORES
    NW = -(-NPC // P)
    NBINS = -(-NPC // BIN)

    src_all, dst_all = ei[0], ei[1]
    order = np.argsort(dst_all, kind="stable")
    src_s, dst_s = src_all[order], dst_all[order]
    ea_s = ea[order]
    core_of = dst_s // NPC
    local = dst_s - core_of * NPC
    bin_of = local // BIN

    cnt = np.zeros((NCORES, NBINS), np.int64)
    np.add.at(cnt, (core_of, bin_of), 1)
    cpb = np.max(-(-cnt // CHUNK), axis=0)          # chunks per bin (shared)
    cpb = np.maximum(cpb, 1)                        # every bin gets a chunk
    C_total = int(cpb.sum())
    off = np.zeros(NBINS, np.int64)
    off[1:] = np.cumsum(cpb)[:-1]
    EP = C_total * CHUNK                            # padded edges per core

    per_core = []
    for k in range(NCORES):
        srck = np.zeros(EP, np.int64)
        dstrk = np.full(EP, float(BIN), np.float32)  # dummies -> dead one-hot
        maskk = np.zeros(EP, np.float32)
        eak = np.zeros((EP, 4), np.float32)
        sel = core_of == k
        bins_k = bin_of[sel]
        start = np.searchsorted(bins_k, np.arange(NBINS))
        pos = np.arange(bins_k.size) - start[bins_k]
        slot = off[bins_k] * CHUNK + pos
        srck[slot] = src_s[sel]
        dstrk[slot] = (local[sel] - bins_k * BIN).astype(np.float32)
        maskk[slot] = 1.0
        eak[slot] = ea_s[sel]

        # device layouts: edge e = c*128 + p
        src16 = np.tile(srck.reshape(-1, 16).T.astype(np.int16), (8, 1))
        dstr_d = dstrk.reshape(C_total, P).T.copy()
        import ml_dtypes
        ea5 = np.concatenate([eak.T, maskk[None, :]], axis=0).astype(ml_dtypes.bfloat16)

        xk = x[k * NPC:(k + 1) * NPC]
        xT = np.zeros((8, NW * P), np.float32)
        xT[:, :NPC] = xk.T
        bk = np.full(NW * P, Gn + 5, np.float32)
        bk[:NPC] = batch[k * NPC:(k + 1) * NPC].astype(np.float32)
        batch_d = bk.reshape(NW, P).T.copy()

        per_core.append(dict(SRC16=src16, DSTR=dstr_d, EAT=ea5,
                             XT=xT, BATCH=batch_d))

    # ---- weight folding
    w = {k: np.asarray(v, np.float32) for k, v in inputs.items()
         if k not in ("x", "edge_index", "edge_attr", "batch", "descriptors")}

    def vfold(We, ae, heads):
        Vp = (We.reshape(w["We_enc"].shape[1], heads, HID) * ae[None]).sum(-1)
        return w["We_enc"] @ Vp, w["be_enc"] @ Vp      # [4,heads],[heads]

    V2, bv2 = vfold(w["We2"], w["ae2"], 4)
    V3, bv3 = vfold(w["We3"], w["ae3"], 4)
    V4, bv4 = vfold(w["We4"], w["ae4"], 1)
    # [5,10]: rows = 4 edge-attr dims + mask; cols = 9 eterms + cnt
    W5x10 = np.zeros((5, 10), np.float32)
    W5x10[0:4, 0:9] = np.concatenate([V2, V3, V4], axis=1)
    W5x10[4, 0:9] = np.concatenate([bv2, bv3, bv4])
    W5x10[4, 9] = 1.0

    def padr(v, n):
        o = np.zeros(n, np.float32)
        o[: v.size] = v
        return o

    # channel-major reorder of the 128-wide (4 heads x 32 ch) dimension:
    # new position c*4+a holds old a*32+c. Keeps per-head broadcasts
    # innermost-packed on DVE (2x mode).
    cm = (np.arange(128) % 4) * 32 + np.arange(128) // 4

    def wext(W, att_s, att_d, heads):
        # append per-head b-columns: b_a = h . (att_s+att_d)_a
        att = (att_s + att_d).reshape(-1)  # [heads*HID] head-major
        if heads == 4:
            attc = att[cm]                 # channel-major to match W cols
            A = np.zeros((128, 4), np.float32)
            A[np.arange(128), np.arange(128) % 4] = attc
        else:
            A = att[:, None]               # [32,1]
        return np.concatenate([W, W @ A], axis=1)

    W1e = wext(w["W1"][:, cm], w["as1"], w["ad1"], 4)            # [8,132]
    W2e = wext(w["W2"][cm][:, cm], w["as2"], w["ad2"], 4)        # [128,132]
    W3e = wext(w["W3"][cm][:, cm], w["as3"], w["ad3"], 4)
    W4e = wext(w["W4"][cm], w["as4"], w["ad4"], 1)               # [128,33]

    bout = np.stack([padr(w["b1"][cm], 128), padr(w["b2"][cm], 128),
                     padr(w["b3"][cm], 128), padr(w["b4"], 128)])

    import ml_dtypes
    T0h = (x @ W1e).astype(np.float32)
    pk0 = np.zeros((N, 256), np.uint8)
    pk0[:, 0:8] = T0h[:, 128:132].astype(ml_dtypes.bfloat16).view(np.uint8)
    pk0[:, 8:136] = T0h[:, 0:128].astype(ml_dtypes.float8_e4m3).view(np.uint8)
    TG0 = pk0.view(ml_dtypes.bfloat16)

    gcnt = np.bincount(batch, minlength=Gn).astype(np.float32)
    cntr = (1.0 / np.maximum(gcnt, 1.0))[None, :]           # [1, Gn]
    shared = dict(
        W1=W1e, WL2=W2e, WL3=W3e, WL4=W4e, TG0=TG0,
        W5X10=W5x10, BOUT=bout, CNTR=cntr,
        WD=w["Wd"], BD=w["bd"][:, None], WLIN=w["Wl"], DESCT=desc.T.copy(),
    )
    bl = float(np.asarray(w["bl"]).reshape(-1)[0])

    dims = dict(N=N, E=E, Gn=Gn, NPC=NPC, NW=NW, NBINS=NBINS,
                C=C_total, cpb=cpb, off=off, bl=bl)
    return dims, shared, per_core


# ------------------------------------------------------------- program build
def build_program(dims, shared):
    import concourse.bass as bass
    import concourse.mybir as mybir
    import concourse.tile as tile
    import concourse.bacc as bacc
    from concourse.masks import make_identity
    from contextlib import ExitStack

    F32 = mybir.dt.float32
    FP8 = mybir.dt.float8e4
    BF16 = mybir.dt.bfloat16
    I32 = mybir.dt.int32
    I16 = mybir.dt.int16
    AF = mybir.ActivationFunctionType
    ALU = mybir.AluOpType
    AX = mybir.AxisListType

    N, Gn, NPC, NW, NBINS, C = (dims[k] for k in ("N", "Gn", "NPC", "NW", "NBINS", "C"))
    cpb, off, bl = dims["cpb"], dims["off"], dims["bl"]
    NSS = C // SS
    # layer params: h width, heads, rhs width, gather row elems
    # PK tables pack rows as [b bf16 x4 | h fp8 x128] (136B) in a 256B stride;
    # HX = leading h-columns multiplied on DVE straight from fp8 (1x mode), the
    # rest is cast to bf16 on Act first so the DVE part runs in 2x mode.
    HXD = dims.get("hx", {0: 96, 1: 48, 2: 48})
    PKL = dims.get("pk_layers", (0, 1, 2))
    LP = [dict(HW=128, AW=4, RW=142, EL=128 if 0 in PKL else ROW,
               GEL=68 if 0 in PKL else 132, PK=0 in PKL, HX=HXD[0]),
          dict(HW=128, AW=4, RW=132, EL=128, GEL=68, PK=1 in PKL, HX=HXD[1]),
          dict(HW=128, AW=4, RW=132, EL=128, GEL=68, PK=2 in PKL, HX=HXD[2]),
          dict(HW=32, AW=1, RW=33, EL=ROW4, GEL=34, PK=False, HX=32)]
    for l_ in (1, 2):
        if not LP[l_]["PK"]:
            LP[l_].update(EL=ROW, GEL=ROW)

    nc = bacc.Bacc(num_swdge_queues=2)
    SIM1 = dims.get("sim1", False)

    # ---- params
    pr = {}
    for nm, shp, dt in [("SRC16", [P, C * 8], I16), ("DSTR", [P, C], F32),
                        ("EAT", [5, C * CHUNK], BF16), ("XT", [8, NW * P], F32),
                        ("BATCH", [P, NW], F32), ("W1", [8, 132], F32),
                        ("WL2", [128, 132], F32), ("WL3", [128, 132], F32),
                        ("WL4", [128, 33], F32), ("W5X10", [5, 10], F32),
                        ("BOUT", [4, 128], F32),
                        ("WD", [48, 32], F32), ("BD", [32, 1], F32),
                        ("WLIN", [64, 1], F32), ("DESCT", [48, Gn], F32),
                        ("CNTR", [1, Gn], F32), ("TG0", [N, 128], BF16)]:
        pr[nm] = nc.declare_dram_parameter(nm, shp, dt, isOutput=False)
    out_p = nc.declare_dram_parameter("out", [1, Gn], F32, isOutput=True)
    pr_TG0_ph = pr["TG0"]

    # ---- internal DRAM
    T_loc = [None] + [nc.dram_tensor(f"T_loc{l}", [NPC, LP[l]["EL"]], BF16)
                      for l in range(1, 4)]
    T_glob = [pr_TG0_ph] + [nc.dram_tensor(f"T_glob{l}", [N, LP[l]["EL"]], BF16,
                                           addr_space="Shared")
                            for l in range(1, 4)]
    ar_in = nc.dram_tensor("ar_in", [32, Gn], F32)
    ar_out = nc.dram_tensor("ar_out", [32, Gn], F32, addr_space="Shared")

    # bin/window bookkeeping (compile-time)
    bin_of_chunk = []
    for b in range(NBINS):
        bin_of_chunk += [b] * int(cpb[b])
    BPW = P // BIN  # bins per window
    win_of_bin = [b // BPW for b in range(NBINS)]
    last_chunk_of_bin = {}
    first_chunk_of_bin = {}
    for c_i, b in enumerate(bin_of_chunk):
        last_chunk_of_bin[b] = c_i
        first_chunk_of_bin.setdefault(b, c_i)

    with tile.TileContext(nc) as tc, ExitStack() as ctx:
        cp = ctx.enter_context(tc.tile_pool(name="const", bufs=1))
        wp = ctx.enter_context(tc.tile_pool(name="work", bufs=2))
        vp = ctx.enter_context(tc.tile_pool(name="win", bufs=2))
        pp = ctx.enter_context(tc.tile_pool(name="psum", bufs=2, space="PSUM"))
        bp = ctx.enter_context(tc.tile_pool(name="binp", bufs=2, space="PSUM"))

        sync, gps, vec, act, pe = nc.sync, nc.gpsimd, nc.vector, nc.scalar, nc.tensor

        def dma_gather_short(out_ap, in_ap, idxs_ap, num_idxs, elem_size,
                             elem_step, queue_num):
            from concourse.bass import exact_div
            eng = gps
            _in_ap = eng.lower_ap_dma(in_ap, for_custom_bir_dma=True)
            _idxs_ap = eng.lower_ap(idxs_ap)
            _out_ap = eng.lower_ap(out_ap)
            stride_bytes_256 = exact_div(elem_step * 2, 256)
            return eng.add_instruction(
                mybir.InstDMAGatherAnt(
                    name=eng.bass.get_next_instruction_name(),
                    ins=[*_in_ap, _idxs_ap,
                         eng.lower_val_access(eng.to_reg(num_idxs))],
                    outs=[_out_ap],
                    transpose=False, num_idxs=num_idxs, elem_size=elem_size,
                    stride_bytes_256=stride_bytes_256, gen_mode=0,
                    single_packet=False, queue_num=queue_num,
                    sbuf_tokens_per_rank=0, sbuf_free_dim_per_rank=0,
                    sbuf_free_dim_pad_per_rank=0, sbuf_byte_offset=0))
        ZTPB = dims.get("ztpb", 1)

        # ---- resident tiles
        src16 = cp.tile([P, C * 8], I16)
        sync.dma_start(out=src16[:], in_=pr["SRC16"][:, :])
        dstr = cp.tile([P, C], BF16)
        gps.dma_start(out=dstr[:], in_=pr["DSTR"][:, :])   # f32 -> bf16 cast
        batcht = cp.tile([P, NW], F32)
        sync.dma_start(out=batcht[:], in_=pr["BATCH"][:, :])
        xT_sb = cp.tile([8, NW * P], BF16)
        gps.dma_start(out=xT_sb[:], in_=pr["XT"][:, :])

        iota_i = cp.tile([P, BIN], I32)
        gps.iota(iota_i[:], pattern=[[1, BIN]], base=0, channel_multiplier=0)
        iotab = cp.tile([P, BIN], BF16)
        vec.tensor_copy(iotab[:], iota_i[:])
        iotag_i = cp.tile([P, Gn], I32)
        gps.iota(iotag_i[:], pattern=[[1, Gn]], base=0, channel_multiplier=0)
        iotagf = cp.tile([P, Gn], F32)
        vec.tensor_copy(iotagf[:], iotag_i[:])
        identf = cp.tile([P, P], F32)
        make_identity(nc, identf[:])

        w1_sb = cp.tile([8, 132], BF16)
        gps.dma_start(out=w1_sb[:], in_=pr["W1"][:, :])
        wl_sb = [None,
                 cp.tile([128, 132], BF16, name="wl2", tag="wl2"),
                 cp.tile([128, 132], BF16, name="wl3", tag="wl3"),
                 cp.tile([128, 33], BF16, name="wl4", tag="wl4")]
        gps.dma_start(out=wl_sb[1][:], in_=pr["WL2"][:, :])   # gpsimd casts f32->bf16
        gps.dma_start(out=wl_sb[2][:], in_=pr["WL3"][:, :])
        gps.dma_start(out=wl_sb[3][:], in_=pr["WL4"][:, :])
        w5x10 = cp.tile([5, 10], BF16)
        gps.dma_start(out=w5x10[:], in_=pr["W5X10"][:, :])
        bout_t = []
        for l in range(4):
            t3 = cp.tile([P, 128], F32, tag=f"bout{l}")
            sync.dma_start(out=t3[:], in_=pr["BOUT"][l:l + 1, :].to_broadcast([P, 128]))
            bout_t.append(t3)

        etc = cp.tile([P, C, 10], BF16)      # eterm9 | cnt  per edge
        pt_all = cp.tile([P, C, BIN], BF16)  # one-hot dst rows per edge
        loop_sb = cp.tile([P, NW, 10], F32)
        gsp = ctx.enter_context(tc.tile_pool(name="gsp", bufs=1, space="PSUM"))
        eap = ctx.enter_context(tc.tile_pool(name="eap", bufs=1))
        gsum_ps = None
        n_pool_mm = [0]

        # ---- readout head start: descriptor branch is input-independent
        comb = cp.tile([64, Gn], F32)
        wd_sb = cp.tile([48, 32], F32)
        sync.dma_start(out=wd_sb[:], in_=pr["WD"][:, :])
        desct_sb = cp.tile([48, Gn], F32)
        sync.dma_start(out=desct_sb[:], in_=pr["DESCT"][:, :])
        bd_sb = cp.tile([32, 1], F32)
        sync.dma_start(out=bd_sb[:], in_=pr["BD"][:, :])
        dps = pp.tile([32, Gn], F32, tag="hps", bufs=4 - ZTPB)
        pe.matmul(out=dps[:], lhsT=wd_sb[:], rhs=desct_sb[:], start=True, stop=True)
        act.activation(out=comb[32:64, :], in_=dps[:], func=AF.Relu, bias=bd_sb[:])
        wlin_sb = cp.tile([64, 1], F32)
        sync.dma_start(out=wlin_sb[:], in_=pr["WLIN"][:, :])
        cntrb = cp.tile([32, Gn], F32)
        sync.dma_start(out=cntrb[:], in_=pr["CNTR"][0:1, :].to_broadcast([32, Gn]))

        WG = dims.get("wg", 5)  # max windows per epilogue group
        # non-uniform groups: taper toward the end so the serial layer-boundary
        # tail (last epilogue -> node phase -> AllGather) shrinks
        grp_bounds = []
        w0_ = 0
        while NW - w0_ > 10:
            grp_bounds.append((w0_, WG))
            w0_ += WG
        for t_ in dims.get("taper", (4, 3, 2, 1)):
            if NW - w0_ > t_:
                grp_bounds.append((w0_, t_))
                w0_ += t_
        if NW > w0_:
            grp_bounds.append((w0_, NW - w0_))
        NG = len(grp_bounds)
        grp_of_win = {}
        for gi, (gw0, gsz_) in enumerate(grp_bounds):
            for w_ in range(gw0, gw0 + gsz_):
                grp_of_win[w_] = gi
        last_chunk_of_grp = {}
        for b in range(NBINS):
            g_ = grp_of_win[win_of_bin[b]]
            last_chunk_of_grp[g_] = max(last_chunk_of_grp.get(g_, -1),
                                        last_chunk_of_bin[b])

        # T_sb tables: [P, NW, 132] (h | b); layer l+1's is built during
        # layer l's edge phase, group by group.
        def node_phase_group(l, g_, T_next, z_src):
            """Build T_next rows for group g_ of layer l (0-based), write T_loc."""
            w0, gsz = grp_bounds[g_]
            HWn = LP[l]["HW"]
            BW = HWn + LP[l]["AW"]  # table row width
            for w_ in range(w0, w0 + gsz):
                if l == 0:
                    hps = pp.tile([P, 132], F32, tag="hps", bufs=4 - ZTPB)
                    pe.matmul(out=hps[:, 0:BW], lhsT=xT_sb[:, w_ * P:(w_ + 1) * P],
                              rhs=w1_sb[:], start=True, stop=True)
                else:
                    ztp = pp.tile([P, P], F32, tag="ztp", bufs=ZTPB)
                    pe.transpose(out=ztp[:], in_=z_src[:, w_ - w0, 0:128],
                                 identity=identf[:])
                    zt_sb = wp.tile([P, P], BF16, tag="ztsb")
                    act.copy(out=zt_sb[:], in_=ztp[:])
                    hps = pp.tile([P, 132], F32, tag="hps", bufs=4 - ZTPB)
                    pe.matmul(out=hps[:, 0:BW], lhsT=zt_sb[:], rhs=wl_sb[l][:],
                              start=True, stop=True)
                act.copy(out=T_next[:, w_, 0:BW], in_=hps[:, 0:BW])
                if l > 0 and LP[l]["PK"]:
                    act.copy(out=Tpk[:, w_, 0:4], in_=hps[:, 128:132])
                    act.copy(out=Tpk[:, w_, 4:68].bitcast(FP8), in_=hps[:, 0:128])
            if l == 0:
                return  # layer-1 table ships as the TG0 param; SBUF copy only
            stage, SW = (Tpk, 68) if LP[l]["PK"] else (T_next, BW)
            # batched table write: full windows in one DMA, ragged tail apart
            wfull = gsz - (1 if (w0 + gsz) * P > NPC else 0)
            if wfull > 0:
                sync.dma_start(
                    out=T_loc[l][w0 * P:(w0 + wfull) * P, 0:SW]
                        .rearrange("(w p) e -> p w e", p=P),
                    in_=stage[:, w0:w0 + wfull, 0:SW])
            if wfull < gsz:
                w_ = w0 + wfull
                nr = NPC - w_ * P
                sync.dma_start(out=T_loc[l][w_ * P:w_ * P + nr, 0:SW],
                               in_=stage[0:nr, w_, 0:SW])

        PT_AHEAD = dims.get("pt_ahead", 64)
        # prebuild the one-hot cache for the first chunks while the layer-0
        # node phase occupies PE/Act
        for g in range(0, PT_AHEAD, 8):
            vec.tensor_tensor(
                out=pt_all[:, g:g + 8, :],
                in0=dstr[:, g:g + 8].unsqueeze(2).to_broadcast([P, 8, BIN]),
                in1=iotab[:].unsqueeze(1).to_broadcast([P, 8, BIN]),
                op=ALU.is_equal)

        # ---- layer 0 node phase (all groups up front)
        T_sb = wp.tile([P, NW, 132], BF16, tag="tsb")
        for g_ in range(NG):
            node_phase_group(0, g_, T_sb, None)

        for l in range(4):
            HW, AW, RW, EL, GEL, PK, HX = (
                LP[l][k] for k in ("HW", "AW", "RW", "EL", "GEL", "PK", "HX"))
            BW = HW + AW

            T_next = None
            if l < 3:
                T_next = wp.tile([P, NW, 132], BF16, name="tnext", tag="tsb")
                if LP[l + 1]["PK"]:
                    Tpk = wp.tile([P, NW, 68], BF16, name="tpk", tag="tpk", bufs=1)

            grp_tiles = {}
            grp_done = set()

            def open_group(g_):
                t = vp.tile([P, WG, 142], F32, name="wingrp", tag="wingrp")
                grp_tiles[g_] = t
                return t

            def epilogue_group(g_):
                w0, gsz = grp_bounds[g_]
                wg = grp_tiles[g_]
                scr = wp.tile([P, WG, 12], F32, name="scr", tag="scr")
                # self-loop alpha (= b_own [+ eterm means]) -> exp
                if l > 0:
                    sl = [None, (0, 4), (4, 8), (8, 9)][l]
                    vec.tensor_tensor(out=scr[:, 0:gsz, 0:AW],
                                      in0=T_sb[:, w0:w0 + gsz, HW:HW + AW],
                                      in1=loop_sb[:, w0:w0 + gsz, sl[0]:sl[1]],
                                      op=ALU.add)
                else:
                    act.copy(out=scr[:, 0:gsz, 0:AW],
                             in_=T_sb[:, w0:w0 + gsz, HW:HW + AW])
                vec.tensor_scalar_mul(out=scr[:, 0:gsz, 4:4 + AW],
                                      in0=scr[:, 0:gsz, 0:AW], scalar1=0.2)
                vec.tensor_tensor(out=scr[:, 0:gsz, 0:AW], in0=scr[:, 0:gsz, 0:AW],
                                  in1=scr[:, 0:gsz, 4:4 + AW], op=ALU.max)
                act.activation(out=scr[:, 0:gsz, 0:AW], in_=scr[:, 0:gsz, 0:AW],
                               func=AF.Exp)
                # num += h_own * ex_loop
                nt = wp.tile([P, WG, 128], BF16, name="nt", tag="nt")
                vec.tensor_tensor(
                    out=nt[:, 0:gsz, 0:HW].rearrange("p g (c a) -> p g c a", a=AW),
                    in0=T_sb[:, w0:w0 + gsz, 0:HW].rearrange("p g (c a) -> p g c a", a=AW),
                    in1=scr[:, 0:gsz, 0:AW].unsqueeze(2)
                        .to_broadcast([P, gsz, HW // AW, AW]),
                    op=ALU.mult)
                vec.tensor_tensor(out=wg[:, 0:gsz, 0:HW], in0=wg[:, 0:gsz, 0:HW],
                                  in1=nt[:, 0:gsz, 0:HW], op=ALU.add)
                # den -> reciprocal
                vec.tensor_tensor(out=scr[:, 0:gsz, 4:4 + AW],
                                  in0=wg[:, 0:gsz, HW:HW + AW],
                                  in1=scr[:, 0:gsz, 0:AW], op=ALU.add)
                vec.tensor_scalar_add(out=scr[:, 0:gsz, 4:4 + AW],
                                      in0=scr[:, 0:gsz, 4:4 + AW], scalar1=1e-16)
                vec.reciprocal(out=scr[:, 0:gsz, 4:4 + AW], in_=scr[:, 0:gsz, 4:4 + AW])
                if l == 0:
                    vec.tensor_scalar_max(out=scr[:, 0:gsz, 8:9],
                                          in0=wg[:, 0:gsz, 141:142], scalar1=1.0)
                    vec.reciprocal(out=scr[:, 0:gsz, 8:9], in_=scr[:, 0:gsz, 8:9])
                    vec.tensor_tensor(
                        out=loop_sb[:, w0:w0 + gsz, 0:9], in0=wg[:, 0:gsz, 132:141],
                        in1=scr[:, 0:gsz, 8:9].to_broadcast([P, gsz, 9]), op=ALU.mult)
                # z = num * recip(den) + bias [+ relu]
                vec.tensor_tensor(
                    out=wg[:, 0:gsz, 0:HW].rearrange("p g (c a) -> p g c a", a=AW),
                    in0=wg[:, 0:gsz, 0:HW].rearrange("p g (c a) -> p g c a", a=AW),
                    in1=scr[:, 0:gsz, 4:4 + AW].unsqueeze(2)
                        .to_broadcast([P, gsz, HW // AW, AW]),
                    op=ALU.mult)
                vec.tensor_tensor(
                    out=wg[:, 0:gsz, 0:HW], in0=wg[:, 0:gsz, 0:HW],
                    in1=bout_t[l][:, 0:HW].unsqueeze(1).to_broadcast([P, gsz, HW]),
                    op=ALU.add)
                if l < 3:
                    act.activation(out=wg[:, 0:gsz, 0:128], in_=wg[:, 0:gsz, 0:128],
                                   func=AF.Relu)
                    node_phase_group(l + 1, g_, T_next, wg)
                else:
                    nonlocal gsum_ps
                    pool_sb = wp.tile([P, WG, 32], BF16, name="pool_sb", tag="poolsb")
                    act.copy(out=pool_sb[:, 0:gsz, 0:32], in_=wg[:, 0:gsz, 0:32])
                    bt = wp.tile([P, WG, Gn], BF16, name="bt", tag="bt", bufs=1)
                    vec.tensor_tensor(
                        out=bt[:, 0:gsz, :],
                        in0=batcht[:, w0:w0 + gsz].unsqueeze(2).to_broadcast([P, gsz, Gn]),
                        in1=iotagf[:].unsqueeze(1).to_broadcast([P, gsz, Gn]),
                        op=ALU.is_equal)
                    if gsum_ps is None:
                        gsum_ps = gsp.tile([32, Gn], F32, name="gsum_ps")
                    for j_ in range(gsz):
                        n_pool_mm[0] += 1
                        pe.matmul(out=gsum_ps[:], lhsT=pool_sb[:, j_, :],
                                  rhs=bt[:, j_, :],
                                  start=(n_pool_mm[0] == 1),
                                  stop=(n_pool_mm[0] == NW))
                grp_done.add(g_)

            cur_bin_tile = {}
            ss_plan = []
            rem_ = C
            while rem_ > 0:
                n_ = min(SS, rem_)
                ss_plan.append(n_)
                rem_ -= n_
            for t_ in dims.get("ss_tail", (8,)):
                if ss_plan[-1] > t_:
                    ss_plan[-1] -= t_
                    ss_plan.append(t_)
            s0 = 0
            GW = GEL if PK else max(GEL, RW)
            for ss, NCH in enumerate(ss_plan):
                Gt = wp.tile([P, SS, GW], BF16, tag="gt" if GW > 68 else "gtp", bufs=4)
                if GEL == EL:
                    gps.dma_gather(
                        out_ap=Gt[:, 0:NCH, 0:GEL], in_ap=T_glob[l][:, :],
                        idxs_ap=src16[:, s0 * 8:(s0 + NCH) * 8],
                        num_idxs=NCH * CHUNK, num_idxs_reg=NCH * CHUNK,
                        elem_size=EL, single_packet=False, queue_num=ss % 2)
                else:
                    dma_gather_short(
                        out_ap=Gt[:, 0:NCH, 0:GEL], in_ap=T_glob[l][:, 0:GEL],
                        idxs_ap=src16[:, s0 * 8:(s0 + NCH) * 8],
                        num_idxs=NCH * CHUNK, elem_size=GEL, elem_step=EL,
                        queue_num=ss % 2)
                if l == 0:
                    # edge-term + mask precompute (feeds rhs cols 132:142 +
                    # later layers' alpha); mask folded into EAT row 5.
                    eaT_sl = eap.tile([5, SS * CHUNK], BF16, name="easl", tag="eat")
                    sync.dma_start(
                        out=eaT_sl[:, 0:NCH * CHUNK],
                        in_=pr["EAT"][:, s0 * CHUNK:(s0 + NCH) * CHUNK])
                    for q0 in range(0, NCH, 16):
                        qn = min(16, NCH - q0)
                        etp = pp.tile([P, 160], F32, tag="etp", bufs=1)
                        for j in range(qn):
                            ci = q0 + j
                            pe.matmul(out=etp[:, j * 10:(j + 1) * 10],
                                      lhsT=eaT_sl[:, ci * CHUNK:(ci + 1) * CHUNK],
                                      rhs=w5x10[:], start=True, stop=True)
                        act.copy(out=etc[:, s0 + q0:s0 + q0 + qn, :]
                                 .rearrange("p a b -> p (a b)"),
                                 in_=etp[:, 0:qn * 10])
                    # staircase one-hots built once, reused by all layers;
                    # built PT_AHEAD chunks ahead so the DVE cost sits in the
                    # pipeline's slack instead of its critical phase
                    pb0 = PT_AHEAD + s0
                    pb1 = min(pb0 + NCH, C)
                    for g in range(pb0, pb1, 8):
                        gn = min(8, pb1 - g)
                        vec.tensor_tensor(
                            out=pt_all[:, g:g + gn, :],
                            in0=dstr[:, g:g + gn].unsqueeze(2).to_broadcast([P, gn, BIN]),
                            in1=iotab[:].unsqueeze(1).to_broadcast([P, gn, BIN]),
                            op=ALU.is_equal)
                # alpha = b[src] (+ eterm) -> leaky relu -> exp
                AT = wp.tile([P, SS, 8], BF16, tag="at", bufs=2)
                if PK:
                    SCT = wp.tile([P, SS, 142], BF16, tag="rhs", bufs=3)
                    BS = 0            # b slot in the packed gathered row
                else:
                    SCT = Gt
                    BS = HW
                if l > 0:
                    sl = [None, (0, 4), (4, 8), (8, 9)][l]
                    vec.tensor_tensor(out=AT[:, 0:NCH, 0:AW],
                                      in0=Gt[:, 0:NCH, BS:BS + AW],
                                      in1=etc[:, s0:s0 + NCH, sl[0]:sl[1]],
                                      op=ALU.add)
                    vec.tensor_scalar_mul(out=AT[:, 0:NCH, AW:2 * AW],
                                          in0=AT[:, 0:NCH, 0:AW], scalar1=0.2)
                    vec.tensor_tensor(out=AT[:, 0:NCH, 0:AW], in0=AT[:, 0:NCH, 0:AW],
                                      in1=AT[:, 0:NCH, AW:2 * AW], op=ALU.max)
                else:
                    vec.tensor_scalar_mul(out=AT[:, 0:NCH, AW:2 * AW],
                                          in0=Gt[:, 0:NCH, BS:BS + AW], scalar1=0.2)
                    vec.tensor_tensor(out=AT[:, 0:NCH, 0:AW],
                                      in0=Gt[:, 0:NCH, BS:BS + AW],
                                      in1=AT[:, 0:NCH, AW:2 * AW], op=ALU.max)
                act.activation(out=SCT[:, 0:NCH, HW:HW + AW], in_=AT[:, 0:NCH, 0:AW],
                               func=AF.Exp)
                if PK:
                    # h x ex: leading HX columns straight from fp8 on DVE (1x);
                    # the rest cast to bf16 on Act, then multiplied in 2x mode
                    vec.tensor_tensor(
                        out=SCT[:, 0:NCH, 0:HX].rearrange("p s (c a) -> p s c a", a=AW),
                        in0=Gt[:, 0:NCH, 4:4 + HX // 2].bitcast(FP8)
                            .rearrange("p s (c a) -> p s c a", a=AW),
                        in1=SCT[:, 0:NCH, HW:HW + AW].unsqueeze(2)
                            .to_broadcast([P, NCH, HX // AW, AW]),
                        op=ALU.mult)
                    if HX < HW:
                        act.copy(out=SCT[:, 0:NCH, HX:HW],
                                 in_=Gt[:, 0:NCH, 4 + HX // 2:4 + HW // 2].bitcast(FP8))
                        vec.tensor_tensor(
                            out=SCT[:, 0:NCH, HX:HW].rearrange("p s (c a) -> p s c a", a=AW),
                            in0=SCT[:, 0:NCH, HX:HW].rearrange("p s (c a) -> p s c a", a=AW),
                            in1=SCT[:, 0:NCH, HW:HW + AW].unsqueeze(2)
                                .to_broadcast([P, NCH, (HW - HX) // AW, AW]),
                            op=ALU.mult)
                else:
                    vec.tensor_tensor(
                        out=SCT[:, 0:NCH, 0:HW].rearrange("p s (c a) -> p s c a", a=AW),
                        in0=SCT[:, 0:NCH, 0:HW].rearrange("p s (c a) -> p s c a", a=AW),
                        in1=SCT[:, 0:NCH, HW:HW + AW].unsqueeze(2)
                            .to_broadcast([P, NCH, HW // AW, AW]),
                        op=ALU.mult)
                if l == 0:
                    # append eterm9|cnt as rhs cols 132:142
                    act.copy(out=SCT[:, 0:NCH, 132:142],
                             in_=etc[:, s0:s0 + NCH, :])
                # scatter matmuls
                for c_i in range(NCH):
                    gc = s0 + c_i
                    b = bin_of_chunk[gc]
                    w_ = win_of_bin[b]
                    g_ = grp_of_win[w_]
                    if g_ not in grp_tiles:
                        open_group(g_)
                    if gc == first_chunk_of_bin[b]:
                        cur_bin_tile[b] = bp.tile([BIN, 142], F32, name="binacc",
                                                  tag="binacc")
                    pe.matmul(out=cur_bin_tile[b][:, 0:RW],
                              lhsT=pt_all[:, gc, :], rhs=SCT[:, c_i, 0:RW],
                              start=(gc == first_chunk_of_bin[b]),
                              stop=(gc == last_chunk_of_bin[b]))
                    if gc == last_chunk_of_bin[b]:
                        j = b % BPW
                        wrel = w_ - grp_bounds[g_][0]
                        act.copy(out=grp_tiles[g_][BIN * j:BIN * (j + 1), wrel, 0:RW],
                                 in_=cur_bin_tile[b][:, 0:RW])
                        del cur_bin_tile[b]
                    if gc == last_chunk_of_grp.get(g_, None):
                        epilogue_group(g_)
                s0 += NCH
            # groups never triggered (e.g. all-empty windows)
            for g_ in range(NG):
                if g_ not in grp_done:
                    if g_ not in grp_tiles:
                        open_group(g_)
                    epilogue_group(g_)
            if l < 3:
                if SIM1:
                    sync.dma_start(out=T_glob[l + 1][0:NPC, :], in_=T_loc[l + 1][:, :])
                else:
                    gps.collective_compute(
                        "AllGather", ALU.bypass, replica_groups=[list(range(NCORES))],
                        ins=[T_loc[l + 1][:, :]], outs=[T_glob[l + 1][:, :]])
                T_sb = T_next

        # ============ readout
        gsum_sb = cp.tile([32, Gn], F32)
        act.copy(out=gsum_sb[:], in_=gsum_ps[:])
        sync.dma_start(out=ar_in[:], in_=gsum_sb[:])
        if SIM1:
            sync.dma_start(out=ar_out[:], in_=ar_in[:])
        else:
            gps.collective_compute("AllReduce", ALU.add,
                                   replica_groups=[list(range(NCORES))],
                                   ins=[ar_in[:]], outs=[ar_out[:]])
        gs = cp.tile([32, Gn], F32)
        sync.dma_start(out=gs[:], in_=ar_out[:])
        vec.tensor_tensor(out=comb[0:32, :], in0=gs[:, :], in1=cntrb[:],
                          op=ALU.mult)
        blt = cp.tile([1, 1], F32)
        vec.memset(blt[:], bl)
        fin = pp.tile([1, Gn], F32, tag="hps", bufs=4 - ZTPB)
        pe.matmul(out=fin[:], lhsT=wlin_sb[:], rhs=comb[:], start=True, stop=True)
        res_sb = cp.tile([1, Gn], F32)
        act.activation(out=res_sb[:], in_=fin[:], func=AF.Sigmoid, bias=blt[:])
        sync.dma_start(out=out_p[:, :], in_=res_sb[:])

    nc.finalize()
    return nc


# ------------------------------------------------------------------ entry
def _run(inputs, trace=False, debug=False):
    dims, shared, per_core = host_prep(inputs)
    nc = build_program(dims, shared)
    in_maps = [{**shared, **pc} for pc in per_core]
    from concourse.bass_utils import run_bass_kernel_spmd
    return run_bass_kernel_spmd(nc, in_maps, list(range(NCORES)), trace=trace)


def kernel(**inputs):
    res = _run(inputs)
    return res.results[0]["out"].reshape(-1).astype(np.float32)
